# revision 1
# baseline (speedup 1.0000x reference)
"""Trainium2 Bass kernel for nn_Attention_Rel_Scl (B=8,S=1024,E=1024,H=16).

Data-parallel over batch: one batch element per NeuronCore (8 cores).

v6: v5 + fp8e4 DoubleRow matmuls for Q/K projections and PV.
  - exp(QK^T/sqrt(E)) is the *stationary* operand of PV / colsum / biasV
    matmuls, so those cost only (out free size) PE cycles and the result
    lands directly in natural [row, feature] orientation (no transposes,
    no gathers). V carries an interleaved 1.0 column per head so PV and
    the softmax denominator come from one moving stream.
  - Emission interleaves 2 QK+exp J-steps between every ~2us PE chunk
    (projection half-chains, PV half-blocks): the in-order engines then
    pace each other without head-of-line stalls; Act (the 133us exp
    budget) starts ~15us in and stays ~full.
  - QT/KT/VE psum->SBUF copies run on GpSimd (Pool) so the DVE's
    reciprocal (which waits on PV groups) never blocks them.
  - T2 (bias table, 63.7KB/part) is DMA-filled into the region freed by
    the projection inputs, overlapping the back half of stage 2.
  - bias[h,i,j] = flat[(16368-1024h) + 1024*(i%16) - 16*(i//16) + j]
    (flat = rel_table.reshape(-1), clip never fires); rows processed in
    order f -> SIGMA[f] = 16*(63-f%64) + f//64 make the bias block for
    (hh, F, J) the T2 view at offset 15360-1024*hh+2048*F+128*J with
    ap [[1,128],[1024,2],[16,64]], T2[p,w] = flat[p+w].
  - LayerNorm in natural layout; combine-add + normalize-apply on Pool,
    bn_stats/aggr/recip on DVE, Sqrt on Act. Contiguous output DMA; host
    un-permutes rows (SIGMA).
"""

import sys

if "/opt/trn_rl_repo" not in sys.path:
    sys.path.insert(0, "/opt/trn_rl_repo")

import numpy as np

B, S, E, H = 8, 1024, 1024, 16
D = E // H          # 64 head dim
P = 128             # partitions
G = H // 2          # 8 head pairs
NBLK = S // P       # 8 key/query blocks
KBLK = E // P       # 8 contraction blocks
EPS = 1e-3
SCALE = float(E) ** -0.5
FLAT = (2 * S - 1) * H   # 32752
T2W = 32625              # max free offset 32624 (+p<=127 -> 32751 = FLAT-1)
DE = D + 1               # 65: V column block plus ones column

_f = np.arange(S)
SIGMA = 16 * (63 - _f % 64) + _f // 64

_BUILT = {}


def _build(trivial_ln: bool):
    import concourse.bass as bass
    import concourse.tile as tile
    from concourse import bacc, mybir
    from contextlib import ExitStack

    fp16 = mybir.dt.float16
    fp32 = mybir.dt.float32
    Exp = mybir.ActivationFunctionType.Exp
    Sqrt = mybir.ActivationFunctionType.Sqrt
    mult = mybir.AluOpType.mult
    add = mybir.AluOpType.add
    sub = mybir.AluOpType.subtract

    nc = bacc.Bacc("TRN2", target_bir_lowering=False, debug=False,
                   num_devices=8)

    fp8 = mybir.dt.float8e4
    DRow = mybir.MatmulPerfMode.DoubleRow
    xt16 = nc.dram_tensor("xt16", [E, S], fp16, kind="ExternalInput").ap()
    xt8 = nc.dram_tensor("xt8", [E, S], fp8, kind="ExternalInput").ap()
    xtp8 = nc.dram_tensor("xtp8", [E, S], fp8, kind="ExternalInput").ap()
    wq8 = nc.dram_tensor("wq8", [E, E], fp8, kind="ExternalInput").ap()
    wk8 = nc.dram_tensor("wk8", [E, E], fp8, kind="ExternalInput").ap()
    wv16 = nc.dram_tensor("wv16", [E, E], fp16, kind="ExternalInput").ap()
    wv8 = nc.dram_tensor("wv8", [E, E], fp8, kind="ExternalInput").ap()
    flat16 = nc.dram_tensor("flat16", [FLAT], fp16, kind="ExternalInput").ap()
    if not trivial_ln:
        gam = nc.dram_tensor("gamma", [1, E], fp32, kind="ExternalInput").ap()
        bet = nc.dram_tensor("beta", [1, E], fp32, kind="ExternalInput").ap()
    out = nc.dram_tensor("out", [S, E], fp32, kind="ExternalOutput").ap()

    with tile.TileContext(nc) as tc, ExitStack() as ctx:
        persist = ctx.enter_context(tc.tile_pool(name="persist", bufs=1))
        QT = persist.tile([P, G, S], fp16, name="QT")
        KT = persist.tile([P, G, S], fp16, name="KT")
        VE = persist.tile([P, NBLK, H * DE], fp16, name="VE")
        VE8 = persist.tile([P, NBLK, H * DE], fp8, name="VE8")
        natSB = persist.tile([P, NBLK, E], fp32, name="natSB")
        srecSB = persist.tile([P, G, 2, NBLK], fp32, name="srecSB")
        epsT = persist.tile([P, 1], fp32, name="epsT")

        nc.vector.memset(epsT, EPS)
        nc.vector.memset(
            bass.AP(tensor=VE.tensor, offset=VE.offset + D,
                    ap=[VE.ap[0], [H * DE, NBLK], [DE, H]]),
            1.0)
        nc.vector.memset(
            bass.AP(tensor=VE8.tensor, offset=VE8.offset + D,
                    ap=[VE8.ap[0], [H * DE, NBLK], [DE, H]]),
            32.0)

        if not trivial_ln:
            gamT = persist.tile([P, E], fp32, name="gamT")
            betT = persist.tile([P, E], fp32, name="betT")
            nc.sync.dma_start(
                out=gamT,
                in_=bass.AP(tensor=gam.tensor, offset=0, ap=[[0, P], [1, E]]),
            )
            nc.sync.dma_start(
                out=betT,
                in_=bass.AP(tensor=bet.tensor, offset=0, ap=[[0, P], [1, E]]),
            )

        expp = ctx.enter_context(tc.tile_pool(name="expp", bufs=4))
        psQK = ctx.enter_context(
            tc.tile_pool(name="psQK", bufs=2, space="PSUM"))
        pvp = ctx.enter_context(
            tc.tile_pool(name="pvp", bufs=1, space="PSUM"))

        eP = {}
        pools = {}

        # ---- emission helpers: each returns a list of closures ("chunks");
        # E-units (one QK J-step + exp) are interleaved between chunks.
        def proj_chunks(g, w8get, dst, rhs8get):
            # fp8 DoubleRow: contraction 1024 as 4 steps of 2x128.
            # Per-ic [P,512] psum tiles (bufs=2) let the DVE copy of ic0
            # overlap the matmuls of ic1 / the next chain.
            def go():
                w8, rhs8 = w8get(), rhs8get()
                for ic in range(2):
                    pt = pools["psProj"].tile([P, 512], fp32, tag="proj",
                                              name="pt")
                    for kp in range(4):
                        nc.tensor.matmul(
                            pt,
                            w8[:, 2 * kp:2 * kp + 2, g * P:(g + 1) * P],
                            rhs8[:, 2 * kp:2 * kp + 2,
                                 ic * 512:(ic + 1) * 512],
                            start=(kp == 0), stop=(kp == 3),
                            perf_mode=DRow, skip_group_check=True,
                        )
                    nc.vector.tensor_copy(
                        dst[:, g, ic * 512:(ic + 1) * 512], pt)
            return [go]

        def v8proj_chunks(jb):
            # fp8 DR V projection feeding VE8 (PV path) only
            def mk(ic):
                def go():
                    bt = pvp.tile([P, NBLK, P], fp32, tag="pv", name="pv")
                    pt = bass.AP(tensor=bt.tensor, offset=bt.offset,
                                 ap=[bt.ap[0], [1, 512]])
                    for kp in range(4):
                        nc.tensor.matmul(
                            pt,
                            x8T[:, 2 * kp:2 * kp + 2, jb * P:(jb + 1) * P],
                            wv8_sb[:, 2 * kp:2 * kp + 2,
                                   ic * 512:(ic + 1) * 512],
                            start=(kp == 0), stop=(kp == 3),
                            perf_mode=DRow, skip_group_check=True,
                        )
                    dstv8 = bass.AP(
                        tensor=VE8.tensor,
                        offset=VE8.offset + jb * (H * DE) + ic * 8 * DE,
                        ap=[VE8.ap[0], [DE, 8], [1, D]],
                    )
                    nc.vector.tensor_copy(dstv8, pt)
                return go
            return [mk(0), mk(1)]

        def vproj_chunks(jb):
            # V runs on the pv psum ring (idle until the first PV at
            # iter 2), in parallel with the Q/K ring. The fp8 copy of V
            # (for DoubleRow PV) is derived from VE on GpSimd.
            def mk(ic):
                def go():
                    bt = pvp.tile([P, NBLK, P], fp32, tag="pv", name="pv")
                    pt = bass.AP(tensor=bt.tensor, offset=bt.offset,
                                 ap=[bt.ap[0], [1, 512]])
                    for kb in range(KBLK):
                        nc.tensor.matmul(
                            pt,
                            xT[:, kb, jb * P:(jb + 1) * P],
                            wv_sb[:, kb, ic * 512:(ic + 1) * 512],
                            start=(kb == 0), stop=(kb == KBLK - 1),
                            skip_group_check=True,
                        )
                    dstv = bass.AP(
                        tensor=VE.tensor,
                        offset=VE.offset + jb * (H * DE) + ic * 8 * DE,
                        ap=[VE.ap[0], [DE, 8], [1, D]],
                    )
                    nc.vector.tensor_copy(dstv, pt)
                return go
            return [mk(0), mk(1)]

        def pv_chunks(g, half):
            u = 2 * g + half
            hh = u
            state = {}

            def mk(fh):
                def go(st):
                    if fh == 0:
                        st["pv"] = pvp.tile([P, NBLK, P], fp32, tag="pv", name="pv")
                    pv = st["pv"]
                    for F in range(4 * fh, 4 * fh + 4):
                        for Jp in range(4):
                            nc.tensor.matmul(
                                pv[:, F, 0:DE],
                                eP[u][:, 2 * Jp:2 * Jp + 2,
                                      F * P:(F + 1) * P],
                                VE8[:, 2 * Jp:2 * Jp + 2,
                                    hh * DE:(hh + 1) * DE],
                                start=(Jp == 0), stop=(Jp == 3),
                                perf_mode=DRow, skip_group_check=True,
                            )
                    if fh == 1:
                        del eP[u]
                        srec = srecSB[:, g, half, :]
                        nc.vector.reciprocal(
                            srec,
                            bass.AP(tensor=pv.tensor, offset=pv.offset + D,
                                    ap=[pv.ap[0], [P, NBLK]]))
                        natv = bass.AP(
                            tensor=natSB.tensor,
                            offset=natSB.offset + hh * D,
                            ap=[natSB.ap[0], [E, NBLK], [1, D]],
                        )
                        pvv = bass.AP(tensor=pv.tensor, offset=pv.offset,
                                      ap=[pv.ap[0], [P, NBLK], [1, D]])
                        srecb = bass.AP(
                            tensor=srecSB.tensor,
                            offset=srecSB.offset + u * NBLK,
                            ap=[srecSB.ap[0], [1, NBLK], [0, D]],
                        )
                        nc.vector.tensor_tensor(natv, pvv, srecb, mult)
                return go
            return [lambda f=mk(0): f(state), lambda f=mk(1): f(state)]

        def e_units(g):
            units = []
            for half in range(2):
                for J in range(NBLK):
                    def go(half=half, J=J):
                        u = 2 * g + half
                        if J == 0:
                            eP[u] = expp.tile([P, NBLK, S], fp8, tag="ept", name="eP")
                        lo = D * half
                        pa = psQK.tile([P, E], fp32, tag="qk", name="pa")
                        for ic in range(2):
                            nc.tensor.matmul(
                                pa[:, ic * 512:(ic + 1) * 512],
                                KT[lo:lo + D, g, J * P:(J + 1) * P],
                                QT[lo:lo + D, g, ic * 512:(ic + 1) * 512],
                                start=True, stop=True,
                                skip_group_check=True,
                            )
                        nc.scalar.activation(
                            out=eP[u][:, J, :], in_=pa, func=Exp,
                            scale=SCALE / 1024.0)
                    units.append(go)
            return units

        def emit_interleaved(chunks, units, gates=None):
            # spread E-units evenly between chunks; unit k may only be
            # emitted once gates[k] chunks are done (WAR: the eP slot it
            # allocates must have its reader PV already emitted).
            nc_, nu = len(chunks), len(units)
            if gates is None:
                gates = [0] * nu
            ui = 0
            for ci, ch in enumerate(chunks):
                ch()
                done = ci + 1
                want = done * nu // nc_
                while ui < want and ui < nu and gates[ui] <= done:
                    units[ui]()
                    ui += 1
            while ui < nu:
                units[ui]()
                ui += 1

        # ---- fused stage 1+2 ----
        with tc.tile_pool(name="psProj", bufs=2, space="PSUM") as psProj, \
             tc.tile_pool(name="s1fix", bufs=1) as s1fix:
            pools["psProj"] = psProj
            xT = s1fix.tile([P, KBLK, S], fp16, name="xT")
            x8T = s1fix.tile([P, KBLK, S], fp8, name="x8T")
            x8Tp = s1fix.tile([P, KBLK, S], fp8, name="x8Tp")
            w8q = s1fix.tile([P, KBLK, E], fp8, name="w8q")
            w8k = s1fix.tile([P, KBLK, E], fp8, name="w8k")
            wv_sb = s1fix.tile([P, KBLK, E], fp16, name="wv_sb")
            wv8_sb = s1fix.tile([P, KBLK, E], fp8, name="wv8_sb")
            nc.sync.dma_start(
                out=x8Tp, in_=xtp8.rearrange("(kb kp) s -> kp kb s", kp=P))
            nc.sync.dma_start(
                out=w8q, in_=wq8.rearrange("(kb kp) e -> kp kb e", kp=P))
            nc.sync.dma_start(
                out=x8T, in_=xt8.rearrange("(kb kp) s -> kp kb s", kp=P))
            nc.sync.dma_start(
                out=w8k, in_=wk8.rearrange("(kb kp) e -> kp kb e", kp=P))
            nc.sync.dma_start(
                out=wv8_sb, in_=wv8.rearrange("(kb kp) e -> kp kb e", kp=P))
            nc.sync.dma_start(
                out=xT, in_=xt16.rearrange("(kb kp) s -> kp kb s", kp=P))
            nc.sync.dma_start(
                out=wv_sb,
                in_=wv16.rearrange("(kb kp) e -> kp kb e", kp=P),
            )

            # Warmup feeds Act immediately: Q/K(0,1) projections first,
            # then E(0) units interleaved with the V chains; remaining
            # Q/K projections ride iter 1 alongside E(1). fp8 DR makes
            # projections cheap enough that the s1fix region (and the T2
            # fill) frees by ~60us into the run.
            for g in (0, 1):
                for c in proj_chunks(g, lambda: w8q, QT, lambda: x8Tp):
                    c()
                for c in proj_chunks(g, lambda: w8k, KT, lambda: x8T):
                    c()
            chunks = []
            for jb in range(4):
                chunks += v8proj_chunks(jb)
            emit_interleaved(chunks, e_units(0))
            chunks = []
            for g in range(2, G):
                chunks += proj_chunks(g, lambda: w8q, QT, lambda: x8Tp)
                chunks += proj_chunks(g, lambda: w8k, KT, lambda: x8T)
            for jb in range(4, 8):
                chunks += v8proj_chunks(jb)
            emit_interleaved(chunks, e_units(1))
            v16 = {2: [0, 1, 2], 3: [3, 4, 5], 4: [6, 7]}
            for i in range(2, 5):
                chunks = pv_chunks(i - 2, 0) + pv_chunks(i - 2, 1)
                for jb in v16[i]:
                    chunks += vproj_chunks(jb)
                emit_interleaved(chunks, e_units(i),
                                 [2] * 8 + [4] * 8)

        # ---- tail of stage 2 + stage 3 (T2 reuses the s1fix region) ----
        with tc.tile_pool(name="t2p", bufs=1) as t2p, \
             tc.tile_pool(name="ln", bufs=2) as ln:
            T2 = t2p.tile([P, T2W], fp16, name="T2")
            nc.sync.dma_start(
                out=T2,
                in_=bass.AP(tensor=flat16.tensor, offset=0,
                            ap=[[1, P], [1, T2W]]),
            )
            # iters 5..7 + PV trail of the pipeline
            for i in range(5, G):
                chunks = pv_chunks(i - 2, 0) + pv_chunks(i - 2, 1)
                emit_interleaved(chunks, e_units(i), [2] * 8 + [4] * 8)
            for c in pv_chunks(G - 2, 0) + pv_chunks(G - 2, 1):
                c()
            for c in pv_chunks(G - 1, 0) + pv_chunks(G - 1, 1):
                c()

            with tc.tile_pool(name="bps", bufs=1, space="PSUM") as bps:
                for F in range(NBLK):
                    # double-buffer the bias accumulator across the bps pool
                    # and the (now idle) pv pool's bank pair
                    if F % 2 == 0:
                        bias_ps = bps.tile([P, E], fp32, tag="bias",
                                           name="bias")
                    else:
                        bt = pvp.tile([P, NBLK, P], fp32, tag="pv",
                                      name="pv")
                        bias_ps = bass.AP(tensor=bt.tensor, offset=bt.offset,
                                          ap=[bt.ap[0], [1, E]])
                    for hh in range(H):
                        base = 15360 - 1024 * hh + 2048 * F
                        for J in range(NBLK):
                            t2st = bass.AP(
                                tensor=T2.tensor,
                                offset=T2.offset + base + P * J,
                                ap=[T2.ap[0], [1024, 2], [16, 64]],
                            )
                            nc.tensor.matmul(
                                bias_ps[:, hh * D:(hh + 1) * D], t2st,
                                VE[:, J, hh * DE:hh * DE + D],
                                start=(J == 0), stop=(J == NBLK - 1),
                                skip_group_check=True,
                            )
                    comb = ln.tile([P, E], fp32, tag="comb", name="comb")
                    nc.vector.tensor_tensor(comb, natSB[:, F, :], bias_ps,
                                            add)
                    stats = ln.tile([P, 2, 6], fp32, tag="stats", name="stats")
                    mv = ln.tile([P, 2], fp32, tag="mv", name="mv")
                    for c in range(2):
                        nc.vector.bn_stats(stats[:, c, :],
                                           comb[:, c * 512:(c + 1) * 512])
                    nc.vector.bn_aggr(mv, stats)
                    rstd = ln.tile([P, 1], fp32, tag="rstd", name="rstd")
                    murs = ln.tile([P, 1], fp32, tag="murs", name="murs")
                    nc.scalar.activation(out=rstd, in_=mv[:, 1:2],
                                         func=Sqrt, bias=epsT, scale=1.0)
                    nc.vector.reciprocal(rstd, rstd)
                    nc.vector.tensor_tensor(murs, mv[:, 0:1], rstd, mult)
                    of = ln.tile([P, E], fp32, tag="of", name="of")
                    nc.gpsimd.tensor_scalar(of, comb, rstd, murs,
                                            op0=mult, op1=sub)
                    if not trivial_ln:
                        nc.vector.tensor_tensor(of, of, gamT, mult)
                        nc.vector.tensor_tensor(of, of, betT, add)
                    nc.sync.dma_start(out[F * P:(F + 1) * P, :], of)

    nc.compile()
    return nc


def get_nc(trivial_ln: bool = True):
    if trivial_ln not in _BUILT:
        _BUILT[trivial_ln] = _build(trivial_ln)
    return _BUILT[trivial_ln]


def make_in_maps(inputs):
    x = np.asarray(inputs["x"])
    rel = np.asarray(inputs["rel_table"])
    gamma = np.asarray(inputs["gamma"])
    beta = np.asarray(inputs["beta"])
    trivial_ln = bool(np.all(gamma == 1.0) and np.all(beta == 0.0))

    import ml_dtypes
    f8 = ml_dtypes.float8_e4m3fn
    x16 = x.astype(np.float16)
    xt16 = np.ascontiguousarray(x16.transpose(0, 2, 1))          # (B, E, S)
    xt8 = np.ascontiguousarray(x.transpose(0, 2, 1).astype(f8))
    xtp8 = np.ascontiguousarray(x[:, SIGMA, :].transpose(0, 2, 1).astype(f8))
    # q/k weights pre-scaled by 32 (fp8 sweet spot); exp scale absorbs 1/1024
    wq8 = (np.asarray(inputs["Wq"]) * 32.0).astype(f8)
    wk8 = (np.asarray(inputs["Wk"]) * 32.0).astype(f8)
    wv16 = np.asarray(inputs["Wv"]).astype(np.float16)
    wv8 = (np.asarray(inputs["Wv"]) * 32.0).astype(f8)
    flat16 = np.ascontiguousarray(rel.reshape(-1).astype(np.float16))

    in_maps = []
    for b in range(x.shape[0]):
        m = {"xt16": xt16[b], "xt8": xt8[b], "xtp8": xtp8[b],
             "wq8": wq8, "wk8": wk8, "wv16": wv16, "wv8": wv8,
             "flat16": flat16}
        if not trivial_ln:
            m["gamma"] = gamma.reshape(1, E).astype(np.float32)
            m["beta"] = beta.reshape(1, E).astype(np.float32)
        in_maps.append(m)
    return in_maps, trivial_ln


def unpermute(raw):
    """raw: (..., S, E) rows in processing order -> natural order."""
    fixed = np.empty_like(raw)
    fixed[..., SIGMA, :] = raw
    return fixed


def kernel(**inputs) -> np.ndarray:
    from concourse import bass_utils

    in_maps, trivial_ln = make_in_maps(inputs)
    nc = get_nc(trivial_ln)
    res = bass_utils.run_bass_kernel_spmd(nc, in_maps,
                                          core_ids=list(range(len(in_maps))))
    outs = np.stack([r["out"] for r in res.results])
    return unpermute(outs).astype(np.float32)



# revision 19
# speedup vs baseline: 1.1133x; 1.1133x over previous
"""Trainium2 Bass kernel for nn_Attention_Rel_Scl (B=8,S=1024,E=1024,H=16).

Data-parallel over batch: one batch element per NeuronCore (8 cores).

v7: v6 + (a) QK^T itself as fp8 DoubleRow: QT/KT are fp8 [P, G, 2, S]
  with the r=1 slice a DMA duplicate of r=0, so the DR matmul contracts
  the 64 head dims twice (exp scale absorbs the 2x) at 0.5 cycles/row;
  (b) bias@V moved inside the Act(exp) window: per-head bias units
  (T2 streamed in 4 slices) accumulate in [P,512] psum and add into
  natSB (now fp16) right after that head's PV, so the old 27us post-exp
  bias tail disappears; LN stats read natSB directly (comb removed).

v6: v5 + fp8e4 DoubleRow matmuls for Q/K projections and PV.
  - exp(QK^T/sqrt(E)) is the *stationary* operand of PV / colsum / biasV
    matmuls, so those cost only (out free size) PE cycles and the result
    lands directly in natural [row, feature] orientation (no transposes,
    no gathers). V carries an interleaved 1.0 column per head so PV and
    the softmax denominator come from one moving stream.
  - Emission interleaves 2 QK+exp J-steps between every ~2us PE chunk
    (projection half-chains, PV half-blocks): the in-order engines then
    pace each other without head-of-line stalls; Act (the 133us exp
    budget) starts ~15us in and stays ~full.
  - QT/KT/VE psum->SBUF copies run on GpSimd (Pool) so the DVE's
    reciprocal (which waits on PV groups) never blocks them.
  - T2 (bias table, 63.7KB/part) is DMA-filled into the region freed by
    the projection inputs, overlapping the back half of stage 2.
  - bias[h,i,j] = flat[(16368-1024h) + 1024*(i%16) - 16*(i//16) + j]
    (flat = rel_table.reshape(-1), clip never fires); rows processed in
    order f -> SIGMA[f] = 16*(63-f%64) + f//64 make the bias block for
    (hh, F, J) the T2 view at offset 15360-1024*hh+2048*F+128*J with
    ap [[1,128],[1024,2],[16,64]], T2[p,w] = flat[p+w].
  - LayerNorm in natural layout; combine-add + normalize-apply on Pool,
    bn_stats/aggr/recip on DVE, Sqrt on Act. Contiguous output DMA; host
    un-permutes rows (SIGMA).
"""

import sys

if "/opt/trn_rl_repo" not in sys.path:
    sys.path.insert(0, "/opt/trn_rl_repo")

import numpy as np

B, S, E, H = 8, 1024, 1024, 16
D = E // H          # 64 head dim
P = 128             # partitions
G = H // 2          # 8 head pairs
NBLK = S // P       # 8 key/query blocks
KBLK = E // P       # 8 contraction blocks
EPS = 1e-3
SCALE = float(E) ** -0.5
FLAT = (2 * S - 1) * H   # 32752
T2W = 32625              # max free offset 32624 (+p<=127 -> 32751 = FLAT-1)
DE = D + 1               # 65: V column block plus ones column

_f = np.arange(S)
SIGMA = 16 * (63 - _f % 64) + _f // 64

_BUILT = {}


def _build(trivial_ln: bool):
    import concourse.bass as bass
    import concourse.tile as tile
    from concourse import bacc, mybir
    from contextlib import ExitStack

    fp16 = mybir.dt.float16
    fp32 = mybir.dt.float32
    Exp = mybir.ActivationFunctionType.Exp
    Sqrt = mybir.ActivationFunctionType.Sqrt
    Identity = mybir.ActivationFunctionType.Identity
    mult = mybir.AluOpType.mult
    add = mybir.AluOpType.add
    sub = mybir.AluOpType.subtract

    nc = bacc.Bacc("TRN2", target_bir_lowering=False, debug=False,
                   num_devices=8)

    fp8 = mybir.dt.float8e4
    DRow = mybir.MatmulPerfMode.DoubleRow
    xt16 = nc.dram_tensor("xt16", [E, S], fp16, kind="ExternalInput").ap()
    xt8 = nc.dram_tensor("xt8", [E, S], fp8, kind="ExternalInput").ap()
    xtp8 = nc.dram_tensor("xtp8", [E, S], fp8, kind="ExternalInput").ap()
    wq8 = nc.dram_tensor("wq8", [E, E], fp8, kind="ExternalInput").ap()
    wk8 = nc.dram_tensor("wk8", [E, E], fp8, kind="ExternalInput").ap()
    wv16 = nc.dram_tensor("wv16", [E, E], fp16, kind="ExternalInput").ap()
    wv8 = nc.dram_tensor("wv8", [E, E], fp8, kind="ExternalInput").ap()
    flat16 = nc.dram_tensor("flat16", [FLAT], fp16, kind="ExternalInput").ap()
    if not trivial_ln:
        gam = nc.dram_tensor("gamma", [1, E], fp32, kind="ExternalInput").ap()
        bet = nc.dram_tensor("beta", [1, E], fp32, kind="ExternalInput").ap()
    # fp16 output (host casts to fp32): LN output is ~N(0,1), fp16
    # rounding is ~5e-4 relative — halves the out-DMA tail.
    out = nc.dram_tensor("out", [S, E], fp16, kind="ExternalOutput").ap()

    with tile.TileContext(nc) as tc, ExitStack() as ctx:
        persist = ctx.enter_context(tc.tile_pool(name="persist", bufs=1))
        QT = persist.tile([P, G, 2, S], fp8, name="QT")
        KT = persist.tile([P, G, 2, S], fp8, name="KT")
        VE = persist.tile([P, NBLK, H * DE], fp16, name="VE")
        VE8 = persist.tile([P, NBLK, H * DE], fp8, name="VE8")
        natSB = persist.tile([P, NBLK, E], fp16, name="natSB")
        srecSB = persist.tile([P, G, 2, NBLK], fp32, name="srecSB")
        epsT = persist.tile([P, 1], fp32, name="epsT")
        # per-(F, head) bn_stats groups, filled as each head's bias-add
        # lands; bn_aggr over the 16 groups at the tail
        statsA = persist.tile([P, NBLK, H, 6], fp32, name="statsA")

        nc.vector.memset(epsT, EPS)
        nc.vector.memset(
            bass.AP(tensor=VE.tensor, offset=VE.offset + D,
                    ap=[VE.ap[0], [H * DE, NBLK], [DE, H]]),
            1.0)
        nc.vector.memset(
            bass.AP(tensor=VE8.tensor, offset=VE8.offset + D,
                    ap=[VE8.ap[0], [H * DE, NBLK], [DE, H]]),
            32.0)

        if not trivial_ln:
            gamT = persist.tile([P, E], fp32, name="gamT")
            betT = persist.tile([P, E], fp32, name="betT")
            nc.sync.dma_start(
                out=gamT,
                in_=bass.AP(tensor=gam.tensor, offset=0, ap=[[0, P], [1, E]]),
            )
            nc.sync.dma_start(
                out=betT,
                in_=bass.AP(tensor=bet.tensor, offset=0, ap=[[0, P], [1, E]]),
            )

        expp = ctx.enter_context(tc.tile_pool(name="expp", bufs=5))
        psQK = ctx.enter_context(
            tc.tile_pool(name="psQK", bufs=2, space="PSUM"))
        pvp = ctx.enter_context(
            tc.tile_pool(name="pvp", bufs=1, space="PSUM"))

        eP = {}
        pools = {}

        # ---- emission helpers: each returns a list of closures ("chunks");
        # E-units (one QK J-step + exp) are interleaved between chunks.
        def proj_chunks(g, w8get, dst, rhs8get):
            # fp8 DoubleRow: contraction 1024 as 4 steps of 2x128.
            # Per-ic [P,512] psum tiles (bufs=2) let the DVE copy of ic0
            # overlap the matmuls of ic1 / the next chain.
            # dst is [P, G, 2, S] fp8; the r=1 slice is a DMA duplicate of
            # r=0 so QK can run as a DoubleRow matmul (contracting the 64
            # head dims twice; exp scale absorbs the factor 2).
            def go():
                w8, rhs8 = w8get(), rhs8get()
                for ic in range(2):
                    pt = pools["psProj"].tile([P, 512], fp32, tag="proj",
                                              name="pt")
                    for kp in range(4):
                        nc.tensor.matmul(
                            pt,
                            w8[:, 2 * kp:2 * kp + 2, g * P:(g + 1) * P],
                            rhs8[:, 2 * kp:2 * kp + 2,
                                 ic * 512:(ic + 1) * 512],
                            start=(kp == 0), stop=(kp == 3),
                            perf_mode=DRow, skip_group_check=True,
                        )
                    nc.vector.tensor_copy(
                        dst[:, g, 0, ic * 512:(ic + 1) * 512], pt)
                    # GPSIMD cannot read PSUM on hw: duplicate from SBUF
                    nc.gpsimd.tensor_copy(
                        dst[:, g, 1, ic * 512:(ic + 1) * 512],
                        dst[:, g, 0, ic * 512:(ic + 1) * 512])
            return [go]

        def v8proj_chunks(jb):
            # fp8 DR V projection feeding VE8 (PV path) only
            def mk(ic):
                def go():
                    bt = pvp.tile([P, NBLK, P], fp32, tag="pv", name="pv")
                    pt = bass.AP(tensor=bt.tensor, offset=bt.offset,
                                 ap=[bt.ap[0], [1, 512]])
                    for kp in range(4):
                        nc.tensor.matmul(
                            pt,
                            x8T[:, 2 * kp:2 * kp + 2, jb * P:(jb + 1) * P],
                            wv8_sb[:, 2 * kp:2 * kp + 2,
                                   ic * 512:(ic + 1) * 512],
                            start=(kp == 0), stop=(kp == 3),
                            perf_mode=DRow, skip_group_check=True,
                        )
                    dstv8 = bass.AP(
                        tensor=VE8.tensor,
                        offset=VE8.offset + jb * (H * DE) + ic * 8 * DE,
                        ap=[VE8.ap[0], [DE, 8], [1, D]],
                    )
                    nc.vector.tensor_copy(dstv8, pt)
                return go
            return [mk(0), mk(1)]

        def vproj_chunks(jb):
            # V runs on the pv psum ring (idle until the first PV at
            # iter 2), in parallel with the Q/K ring. The fp8 copy of V
            # (for DoubleRow PV) is derived from VE on GpSimd.
            def mk(ic):
                def go():
                    bt = pvp.tile([P, NBLK, P], fp32, tag="pv", name="pv")
                    pt = bass.AP(tensor=bt.tensor, offset=bt.offset,
                                 ap=[bt.ap[0], [1, 512]])
                    for kb in range(KBLK):
                        nc.tensor.matmul(
                            pt,
                            xT[:, kb, jb * P:(jb + 1) * P],
                            wv_sb[:, kb, ic * 512:(ic + 1) * 512],
                            start=(kb == 0), stop=(kb == KBLK - 1),
                            skip_group_check=True,
                        )
                    dstv = bass.AP(
                        tensor=VE.tensor,
                        offset=VE.offset + jb * (H * DE) + ic * 8 * DE,
                        ap=[VE.ap[0], [DE, 8], [1, D]],
                    )
                    nc.vector.tensor_copy(dstv, pt)
                return go
            return [mk(0), mk(1)]

        def pv_chunks(g, half):
            u = 2 * g + half
            hh = u
            state = {}

            def mk(fh):
                def go(st):
                    if fh == 0:
                        st["pv"] = pvp.tile([P, NBLK, P], fp32, tag="pv", name="pv")
                    pv = st["pv"]
                    for F in range(4 * fh, 4 * fh + 4):
                        for Jp in range(4):
                            nc.tensor.matmul(
                                pv[:, F, 0:DE],
                                eP[u][:, 2 * Jp:2 * Jp + 2,
                                      F * P:(F + 1) * P],
                                VE8[:, 2 * Jp:2 * Jp + 2,
                                    hh * DE:(hh + 1) * DE],
                                start=(Jp == 0), stop=(Jp == 3),
                                perf_mode=DRow, skip_group_check=True,
                            )
                    if fh == 1:
                        del eP[u]
                        srec = srecSB[:, g, half, :]
                        nc.vector.reciprocal(
                            srec,
                            bass.AP(tensor=pv.tensor, offset=pv.offset + D,
                                    ap=[pv.ap[0], [P, NBLK]]))
                        natv = bass.AP(
                            tensor=natSB.tensor,
                            offset=natSB.offset + hh * D,
                            ap=[natSB.ap[0], [E, NBLK], [1, D]],
                        )
                        pvv = bass.AP(tensor=pv.tensor, offset=pv.offset,
                                      ap=[pv.ap[0], [P, NBLK], [1, D]])
                        srecb = bass.AP(
                            tensor=srecSB.tensor,
                            offset=srecSB.offset + u * NBLK,
                            ap=[srecSB.ap[0], [1, NBLK], [0, D]],
                        )
                        nc.vector.tensor_tensor(natv, pvv, srecb, mult)
                return go
            return [lambda f=mk(0): f(state), lambda f=mk(1): f(state)]

        def e_units(g):
            units = []
            for half in range(2):
                for J in range(NBLK):
                    def go(half=half, J=J):
                        u = 2 * g + half
                        if J == 0:
                            eP[u] = expp.tile([P, NBLK, S], fp8, tag="ept", name="eP")
                        lo = D * half
                        pa = psQK.tile([P, E], fp32, tag="qk", name="pa")
                        for ic in range(2):
                            nc.tensor.matmul(
                                pa[:, ic * 512:(ic + 1) * 512],
                                KT[lo:lo + D, g, :, J * P:(J + 1) * P],
                                QT[lo:lo + D, g, :,
                                   ic * 512:(ic + 1) * 512],
                                start=True, stop=True,
                                perf_mode=DRow, skip_group_check=True,
                            )
                        nc.scalar.activation(
                            out=eP[u][:, J, :], in_=pa, func=Exp,
                            scale=SCALE / 2048.0)
                    units.append(go)
            return units

        def emit_interleaved(chunks, units, gates=None):
            # spread E-units evenly between chunks; unit k may only be
            # emitted once gates[k] chunks are done (WAR: the eP slot it
            # allocates must have its reader PV already emitted).
            nc_, nu = len(chunks), len(units)
            if gates is None:
                gates = [0] * nu
            ui = 0
            for ci, ch in enumerate(chunks):
                ch()
                done = ci + 1
                want = done * nu // nc_
                while ui < want and ui < nu and gates[ui] <= done:
                    units[ui]()
                    ui += 1
            while ui < nu:
                units[ui]()
                ui += 1

        # ---- fused stage 1+2 ----
        with tc.tile_pool(name="psProj", bufs=2, space="PSUM") as psProj, \
             tc.tile_pool(name="s1fix", bufs=1) as s1fix:
            pools["psProj"] = psProj
            xT = s1fix.tile([P, KBLK, S], fp16, name="xT")
            x8T = s1fix.tile([P, KBLK, S], fp8, name="x8T")
            x8Tp = s1fix.tile([P, KBLK, S], fp8, name="x8Tp")
            w8q = s1fix.tile([P, KBLK, E], fp8, name="w8q")
            w8k = s1fix.tile([P, KBLK, E], fp8, name="w8k")
            wv_sb = s1fix.tile([P, KBLK, E], fp16, name="wv_sb")
            wv8_sb = s1fix.tile([P, KBLK, E], fp8, name="wv8_sb")
            # Input DMAs serialize on the DMA-engine device, so arrival
            # order = creation order. Load per-g column blocks of Wq/Wk so
            # the g0 QK chain (and the first exp) is gated by ~7us of DMA
            # instead of ~12us.
            wqr = wq8.rearrange("(kb kp) e -> kp kb e", kp=P)
            wkr = wk8.rearrange("(kb kp) e -> kp kb e", kp=P)
            nc.sync.dma_start(
                out=x8Tp, in_=xtp8.rearrange("(kb kp) s -> kp kb s", kp=P))
            nc.sync.dma_start(out=w8q[:, :, 0:P], in_=wqr[:, :, 0:P])
            nc.sync.dma_start(
                out=x8T, in_=xt8.rearrange("(kb kp) s -> kp kb s", kp=P))
            nc.sync.dma_start(out=w8k[:, :, 0:P], in_=wkr[:, :, 0:P])
            for g in (1,):
                nc.sync.dma_start(out=w8q[:, :, g * P:(g + 1) * P],
                                  in_=wqr[:, :, g * P:(g + 1) * P])
                nc.sync.dma_start(out=w8k[:, :, g * P:(g + 1) * P],
                                  in_=wkr[:, :, g * P:(g + 1) * P])
            nc.sync.dma_start(
                out=wv8_sb, in_=wv8.rearrange("(kb kp) e -> kp kb e", kp=P))
            for g in range(2, G):
                nc.sync.dma_start(out=w8q[:, :, g * P:(g + 1) * P],
                                  in_=wqr[:, :, g * P:(g + 1) * P])
                nc.sync.dma_start(out=w8k[:, :, g * P:(g + 1) * P],
                                  in_=wkr[:, :, g * P:(g + 1) * P])
            nc.sync.dma_start(
                out=xT, in_=xt16.rearrange("(kb kp) s -> kp kb s", kp=P))
            nc.sync.dma_start(
                out=wv_sb,
                in_=wv16.rearrange("(kb kp) e -> kp kb e", kp=P),
            )

            # Warmup feeds Act immediately: Q/K(0,1) projections first,
            # then E(0) units interleaved with the V chains; remaining
            # Q/K projections ride iter 1 alongside E(1). fp8 DR makes
            # projections cheap enough that the s1fix region (and the T2
            # fill) frees by ~60us into the run.
            for g in (0, 1):
                for c in proj_chunks(g, lambda: w8q, QT, lambda: x8Tp):
                    c()
                for c in proj_chunks(g, lambda: w8k, KT, lambda: x8T):
                    c()
            chunks = []
            for jb in range(4):
                chunks += v8proj_chunks(jb)
            emit_interleaved(chunks, e_units(0))
            chunks = []
            for g in range(2, G):
                chunks += proj_chunks(g, lambda: w8q, QT, lambda: x8Tp)
                chunks += proj_chunks(g, lambda: w8k, KT, lambda: x8T)
            for jb in range(4, 8):
                chunks += v8proj_chunks(jb)
            emit_interleaved(chunks, e_units(1))
            v16 = {2: [0, 1, 2], 3: [3, 4, 5], 4: [6, 7]}
            for i in range(2, 5):
                chunks = pv_chunks(i - 2, 0) + pv_chunks(i - 2, 1)
                for jb in v16[i]:
                    chunks += vproj_chunks(jb)
                emit_interleaved(chunks, e_units(i),
                                 [2] * 8 + [4] * 8)

        # ---- tail of stage 2 + stage 3 (T2 reuses the s1fix region) ----
        with tc.tile_pool(name="t2p", bufs=1) as t2p, \
             tc.tile_pool(name="lns", bufs=8) as lns, \
             tc.tile_pool(name="lnof", bufs=3) as lnof, \
             tc.tile_pool(name="bps", bufs=2, space="PSUM") as bps:
            T2 = t2p.tile([P, T2W], fp16, name="T2")
            # 4-slice fill, high offsets first: bias for head hh reads
            # [15360-1024hh, 32752-1024hh), so hh 0..7 only need the top
            # three slices and can start one slice-DMA earlier.
            for a, b in ((24576, T2W), (16384, 24576), (8192, 16384),
                         (0, 8192)):
                nc.sync.dma_start(
                    out=T2[:, a:b],
                    in_=bass.AP(tensor=flat16.tensor, offset=a,
                                ap=[[1, P], [1, b - a]]),
                )

            def bias_units(hh):
                # bias@V for head hh over all 8 F blocks, accumulated in a
                # [P, 512] psum tile (64-col slice per F), then added into
                # natSB. Runs inside the Act window instead of after it.
                # Emit only after pv_chunks for head hh (natv write order
                # on the in-order DVE queue).
                def go():
                    bt = bps.tile([P, NBLK * D], fp32, tag="bias",
                                  name="bias")
                    for F in range(NBLK):
                        base = 15360 - 1024 * hh + 2048 * F
                        for J in range(NBLK):
                            t2st = bass.AP(
                                tensor=T2.tensor,
                                offset=T2.offset + base + P * J,
                                ap=[T2.ap[0], [16, P]],
                            )
                            nc.tensor.matmul(
                                bt[:, F * D:(F + 1) * D], t2st,
                                VE[:, J, hh * DE:hh * DE + D],
                                start=(J == 0), stop=(J == NBLK - 1),
                                skip_group_check=True,
                            )
                    nat = bass.AP(
                        tensor=natSB.tensor, offset=natSB.offset + hh * D,
                        ap=[natSB.ap[0], [E, NBLK], [1, D]],
                    )
                    nc.vector.tensor_tensor(nat, nat, bt, add)
                    for F in range(NBLK):
                        nc.vector.bn_stats(
                            statsA[:, F, hh, :],
                            natSB[:, F, hh * D:(hh + 1) * D])
                return go

            # iters 5..7 + PV trail of the pipeline. Bias units are created
            # AFTER all attention work so the scheduler only backfills them
            # into PE-idle slots (created earlier they outrank the next
            # iter's QK matmuls in priority and get hoisted into solid
            # blocks that starve Act).
            for i in range(5, G):
                chunks = pv_chunks(i - 2, 0) + pv_chunks(i - 2, 1)
                emit_interleaved(chunks, e_units(i), [2] * 8 + [4] * 8)
            for c in pv_chunks(G - 2, 0) + pv_chunks(G - 2, 1):
                c()
            for c in pv_chunks(G - 1, 0) + pv_chunks(G - 1, 1):
                c()
            for hh in range(H):
                bias_units(hh)()

            # Two-pass LN tail: all stats→rstd/murs first (Act queue then
            # holds Sqrt0..7 before any apply, no head-of-line blocking),
            # then the applies alternate Pool / Act with out-DMA per F.
            rstds, murss = [], []
            for F in range(NBLK):
                mv = lns.tile([P, 2], fp32, tag="mv", name="mv")
                nc.vector.bn_aggr(mv, statsA[:, F, :, :])
                rstd = lns.tile([P, 1], fp32, tag="rstd", name="rstd")
                murs = lns.tile([P, 1], fp32, tag="murs", name="murs")
                nc.scalar.activation(out=rstd, in_=mv[:, 1:2],
                                     func=Sqrt, bias=epsT, scale=1.0)
                nc.vector.reciprocal(rstd, rstd)
                if F % 2 == 0:
                    # Pool apply: of = natSB*rstd - murs
                    nc.vector.tensor_tensor(murs, mv[:, 0:1], rstd, mult)
                else:
                    # Act apply: of = natSB*rstd + (-murs)
                    nc.vector.tensor_scalar(murs, mv[:, 0:1], rstd, -1.0,
                                            op0=mult, op1=mult)
                rstds.append(rstd)
                murss.append(murs)
            for F in range(NBLK):
                of = lnof.tile([P, E], fp32, tag="of", name="of")
                if F % 2 == 0:
                    nc.gpsimd.tensor_scalar(of, natSB[:, F, :], rstds[F],
                                            murss[F], op0=mult, op1=sub)
                else:
                    nc.scalar.activation(out=of, in_=natSB[:, F, :],
                                         func=Identity, bias=murss[F],
                                         scale=rstds[F])
                if not trivial_ln:
                    nc.vector.tensor_tensor(of, of, gamT, mult)
                    nc.vector.tensor_tensor(of, of, betT, add)
                nc.sync.dma_start(out[F * P:(F + 1) * P, :], of)

    nc.compile()
    return nc


def get_nc(trivial_ln: bool = True):
    if trivial_ln not in _BUILT:
        _BUILT[trivial_ln] = _build(trivial_ln)
    return _BUILT[trivial_ln]


def make_in_maps(inputs):
    x = np.asarray(inputs["x"])
    rel = np.asarray(inputs["rel_table"])
    gamma = np.asarray(inputs["gamma"])
    beta = np.asarray(inputs["beta"])
    trivial_ln = bool(np.all(gamma == 1.0) and np.all(beta == 0.0))

    import ml_dtypes
    f8 = ml_dtypes.float8_e4m3fn
    x16 = x.astype(np.float16)
    xt16 = np.ascontiguousarray(x16.transpose(0, 2, 1))          # (B, E, S)
    xt8 = np.ascontiguousarray(x.transpose(0, 2, 1).astype(f8))
    xtp8 = np.ascontiguousarray(x[:, SIGMA, :].transpose(0, 2, 1).astype(f8))
    # q/k weights pre-scaled by 32 (fp8 sweet spot); exp scale absorbs 1/1024
    wq8 = (np.asarray(inputs["Wq"]) * 32.0).astype(f8)
    wk8 = (np.asarray(inputs["Wk"]) * 32.0).astype(f8)
    wv16 = np.asarray(inputs["Wv"]).astype(np.float16)
    wv8 = (np.asarray(inputs["Wv"]) * 32.0).astype(f8)
    flat16 = np.ascontiguousarray(rel.reshape(-1).astype(np.float16))

    in_maps = []
    for b in range(x.shape[0]):
        m = {"xt16": xt16[b], "xt8": xt8[b], "xtp8": xtp8[b],
             "wq8": wq8, "wk8": wk8, "wv16": wv16, "wv8": wv8,
             "flat16": flat16}
        if not trivial_ln:
            m["gamma"] = gamma.reshape(1, E).astype(np.float32)
            m["beta"] = beta.reshape(1, E).astype(np.float32)
        in_maps.append(m)
    return in_maps, trivial_ln


def unpermute(raw):
    """raw: (..., S, E) rows in processing order -> natural order."""
    fixed = np.empty_like(raw)
    fixed[..., SIGMA, :] = raw
    return fixed


def kernel(**inputs) -> np.ndarray:
    from concourse import bass_utils

    in_maps, trivial_ln = make_in_maps(inputs)
    nc = get_nc(trivial_ln)
    res = bass_utils.run_bass_kernel_spmd(nc, in_maps,
                                          core_ids=list(range(len(in_maps))))
    outs = np.stack([r["out"] for r in res.results])
    return unpermute(outs).astype(np.float32)



# revision 22
# speedup vs baseline: 1.1541x; 1.0367x over previous
"""Trainium2 Bass kernel for nn_Attention_Rel_Scl (B=8,S=1024,E=1024,H=16).

Data-parallel over batch: one batch element per NeuronCore (8 cores).

v7: v6 + (a) QK^T itself as fp8 DoubleRow: QT/KT are fp8 [P, G, 2, S]
  with the r=1 slice a DMA duplicate of r=0, so the DR matmul contracts
  the 64 head dims twice (exp scale absorbs the 2x) at 0.5 cycles/row;
  (b) bias@V moved inside the Act(exp) window: per-head bias units
  (T2 streamed in 4 slices) accumulate in [P,512] psum and add into
  natSB (now fp16) right after that head's PV, so the old 27us post-exp
  bias tail disappears; LN stats read natSB directly (comb removed).

v6: v5 + fp8e4 DoubleRow matmuls for Q/K projections and PV.
  - exp(QK^T/sqrt(E)) is the *stationary* operand of PV / colsum / biasV
    matmuls, so those cost only (out free size) PE cycles and the result
    lands directly in natural [row, feature] orientation (no transposes,
    no gathers). V carries an interleaved 1.0 column per head so PV and
    the softmax denominator come from one moving stream.
  - Emission interleaves 2 QK+exp J-steps between every ~2us PE chunk
    (projection half-chains, PV half-blocks): the in-order engines then
    pace each other without head-of-line stalls; Act (the 133us exp
    budget) starts ~15us in and stays ~full.
  - QT/KT/VE psum->SBUF copies run on GpSimd (Pool) so the DVE's
    reciprocal (which waits on PV groups) never blocks them.
  - T2 (bias table, 63.7KB/part) is DMA-filled into the region freed by
    the projection inputs, overlapping the back half of stage 2.
  - bias[h,i,j] = flat[(16368-1024h) + 1024*(i%16) - 16*(i//16) + j]
    (flat = rel_table.reshape(-1), clip never fires); rows processed in
    order f -> SIGMA[f] = 16*(63-f%64) + f//64 make the bias block for
    (hh, F, J) the T2 view at offset 15360-1024*hh+2048*F+128*J with
    ap [[1,128],[1024,2],[16,64]], T2[p,w] = flat[p+w].
  - LayerNorm in natural layout; combine-add + normalize-apply on Pool,
    bn_stats/aggr/recip on DVE, Sqrt on Act. Contiguous output DMA; host
    un-permutes rows (SIGMA).
"""

import sys

if "/opt/trn_rl_repo" not in sys.path:
    sys.path.insert(0, "/opt/trn_rl_repo")

import numpy as np

B, S, E, H = 8, 1024, 1024, 16
D = E // H          # 64 head dim
P = 128             # partitions
G = H // 2          # 8 head pairs
NBLK = S // P       # 8 key/query blocks
KBLK = E // P       # 8 contraction blocks
EPS = 1e-3
SCALE = float(E) ** -0.5
FLAT = (2 * S - 1) * H   # 32752
T2W = 32625              # max free offset 32624 (+p<=127 -> 32751 = FLAT-1)
DE = D + 1               # 65: V column block plus ones column

_f = np.arange(S)
SIGMA = 16 * (63 - _f % 64) + _f // 64

_BUILT = {}


def _build(trivial_ln: bool):
    import concourse.bass as bass
    import concourse.tile as tile
    from concourse import bacc, mybir
    from contextlib import ExitStack

    fp16 = mybir.dt.float16
    fp32 = mybir.dt.float32
    Exp = mybir.ActivationFunctionType.Exp
    Sqrt = mybir.ActivationFunctionType.Sqrt
    Identity = mybir.ActivationFunctionType.Identity
    mult = mybir.AluOpType.mult
    add = mybir.AluOpType.add
    sub = mybir.AluOpType.subtract

    nc = bacc.Bacc("TRN2", target_bir_lowering=False, debug=False,
                   num_devices=8)

    fp8 = mybir.dt.float8e4
    DRow = mybir.MatmulPerfMode.DoubleRow
    xt16 = nc.dram_tensor("xt16", [E, S], fp16, kind="ExternalInput").ap()
    xt8 = nc.dram_tensor("xt8", [E, S], fp8, kind="ExternalInput").ap()
    xtp8 = nc.dram_tensor("xtp8", [E, S], fp8, kind="ExternalInput").ap()
    wq8 = nc.dram_tensor("wq8", [E, E], fp8, kind="ExternalInput").ap()
    wk8 = nc.dram_tensor("wk8", [E, E], fp8, kind="ExternalInput").ap()
    wv16 = nc.dram_tensor("wv16", [E, E], fp16, kind="ExternalInput").ap()
    wv8 = nc.dram_tensor("wv8", [E, E], fp8, kind="ExternalInput").ap()
    flat16 = nc.dram_tensor("flat16", [FLAT], fp16, kind="ExternalInput").ap()
    if not trivial_ln:
        gam = nc.dram_tensor("gamma", [1, E], fp32, kind="ExternalInput").ap()
        bet = nc.dram_tensor("beta", [1, E], fp32, kind="ExternalInput").ap()
    # fp16 output (host casts to fp32): LN output is ~N(0,1), fp16
    # rounding is ~5e-4 relative — halves the out-DMA tail.
    out = nc.dram_tensor("out", [S, E], fp16, kind="ExternalOutput").ap()

    with tile.TileContext(nc) as tc, ExitStack() as ctx:
        persist = ctx.enter_context(tc.tile_pool(name="persist", bufs=1))
        QT = persist.tile([P, G, 2, S], fp8, name="QT")
        KT = persist.tile([P, G, 2, S], fp8, name="KT")
        VE = persist.tile([P, NBLK, H * DE], fp16, name="VE")
        VE8 = persist.tile([P, NBLK, H * DE], fp8, name="VE8")
        natSB = persist.tile([P, NBLK, E], fp16, name="natSB")
        srecSB = persist.tile([P, G, 2, NBLK], fp32, name="srecSB")
        epsT = persist.tile([P, 1], fp32, name="epsT")
        # per-(F, head) bn_stats groups, filled as each head's bias-add
        # lands; bn_aggr over the 16 groups at the tail
        statsA = persist.tile([P, NBLK, H, 6], fp32, name="statsA")

        nc.vector.memset(epsT, EPS)
        nc.vector.memset(
            bass.AP(tensor=VE.tensor, offset=VE.offset + D,
                    ap=[VE.ap[0], [H * DE, NBLK], [DE, H]]),
            1.0)
        nc.vector.memset(
            bass.AP(tensor=VE8.tensor, offset=VE8.offset + D,
                    ap=[VE8.ap[0], [H * DE, NBLK], [DE, H]]),
            32.0)

        if not trivial_ln:
            gamT = persist.tile([P, E], fp32, name="gamT")
            betT = persist.tile([P, E], fp32, name="betT")
            nc.sync.dma_start(
                out=gamT,
                in_=bass.AP(tensor=gam.tensor, offset=0, ap=[[0, P], [1, E]]),
            )
            nc.sync.dma_start(
                out=betT,
                in_=bass.AP(tensor=bet.tensor, offset=0, ap=[[0, P], [1, E]]),
            )

        expp = ctx.enter_context(tc.tile_pool(name="expp", bufs=5))
        psQK = ctx.enter_context(
            tc.tile_pool(name="psQK", bufs=2, space="PSUM"))
        pvp = ctx.enter_context(
            tc.tile_pool(name="pvp", bufs=1, space="PSUM"))

        eP = {}
        pools = {}

        # ---- emission helpers: each returns a list of closures ("chunks");
        # E-units (one QK J-step + exp) are interleaved between chunks.
        def proj_chunks(g, w8get, dst, rhs8get):
            # fp8 DoubleRow: contraction 1024 as 4 steps of 2x128.
            # Per-ic [P,512] psum tiles (bufs=2) let the DVE copy of ic0
            # overlap the matmuls of ic1 / the next chain.
            # dst is [P, G, 2, S] fp8; the r=1 slice is a DMA duplicate of
            # r=0 so QK can run as a DoubleRow matmul (contracting the 64
            # head dims twice; exp scale absorbs the factor 2).
            def go():
                w8, rhs8 = w8get(), rhs8get()
                for ic in range(2):
                    pt = pools["psProj"].tile([P, 512], fp32, tag="proj",
                                              name="pt")
                    for kp in range(4):
                        nc.tensor.matmul(
                            pt,
                            w8[:, 2 * kp:2 * kp + 2, g * P:(g + 1) * P],
                            rhs8[:, 2 * kp:2 * kp + 2,
                                 ic * 512:(ic + 1) * 512],
                            start=(kp == 0), stop=(kp == 3),
                            perf_mode=DRow, skip_group_check=True,
                        )
                    nc.vector.tensor_copy(
                        dst[:, g, 0, ic * 512:(ic + 1) * 512], pt)
                    # duplicate r=0 -> r=1 (GPSIMD cannot read PSUM on hw).
                    # g0/g1 feed the first exps: keep their dup on DVE to
                    # avoid the cross-engine hop on the warmup critical path.
                    eng = nc.vector if g < 2 else nc.gpsimd
                    eng.tensor_copy(
                        dst[:, g, 1, ic * 512:(ic + 1) * 512],
                        dst[:, g, 0, ic * 512:(ic + 1) * 512])
            return [go]

        def v8proj_chunks(jb):
            # fp8 DR V projection feeding VE8 (PV path) only
            def mk(ic):
                def go():
                    bt = pvp.tile([P, NBLK, P], fp32, tag="pv", name="pv")
                    pt = bass.AP(tensor=bt.tensor, offset=bt.offset,
                                 ap=[bt.ap[0], [1, 512]])
                    for kp in range(4):
                        nc.tensor.matmul(
                            pt,
                            x8T[:, 2 * kp:2 * kp + 2, jb * P:(jb + 1) * P],
                            wv8_sb[:, 2 * kp:2 * kp + 2,
                                   ic * 512:(ic + 1) * 512],
                            start=(kp == 0), stop=(kp == 3),
                            perf_mode=DRow, skip_group_check=True,
                        )
                    dstv8 = bass.AP(
                        tensor=VE8.tensor,
                        offset=VE8.offset + jb * (H * DE) + ic * 8 * DE,
                        ap=[VE8.ap[0], [DE, 8], [1, D]],
                    )
                    nc.vector.tensor_copy(dstv8, pt)
                return go
            return [mk(0), mk(1)]

        def vproj_chunks(jb):
            # V runs on the pv psum ring (idle until the first PV at
            # iter 2), in parallel with the Q/K ring. The fp8 copy of V
            # (for DoubleRow PV) is derived from VE on GpSimd.
            def mk(ic):
                def go():
                    bt = pvp.tile([P, NBLK, P], fp32, tag="pv", name="pv")
                    pt = bass.AP(tensor=bt.tensor, offset=bt.offset,
                                 ap=[bt.ap[0], [1, 512]])
                    for kb in range(KBLK):
                        nc.tensor.matmul(
                            pt,
                            xT[:, kb, jb * P:(jb + 1) * P],
                            wv_sb[:, kb, ic * 512:(ic + 1) * 512],
                            start=(kb == 0), stop=(kb == KBLK - 1),
                            skip_group_check=True,
                        )
                    dstv = bass.AP(
                        tensor=VE.tensor,
                        offset=VE.offset + jb * (H * DE) + ic * 8 * DE,
                        ap=[VE.ap[0], [DE, 8], [1, D]],
                    )
                    nc.vector.tensor_copy(dstv, pt)
                return go
            return [mk(0), mk(1)]

        def pv_chunks(g, half):
            u = 2 * g + half
            hh = u
            state = {}

            def mk(fh):
                def go(st):
                    if fh == 0:
                        st["pv"] = pvp.tile([P, NBLK, P], fp32, tag="pv", name="pv")
                    pv = st["pv"]
                    for F in range(4 * fh, 4 * fh + 4):
                        for Jp in range(4):
                            nc.tensor.matmul(
                                pv[:, F, 0:DE],
                                eP[u][:, 2 * Jp:2 * Jp + 2,
                                      F * P:(F + 1) * P],
                                VE8[:, 2 * Jp:2 * Jp + 2,
                                    hh * DE:(hh + 1) * DE],
                                start=(Jp == 0), stop=(Jp == 3),
                                perf_mode=DRow, skip_group_check=True,
                            )
                    if fh == 1:
                        del eP[u]
                        srec = srecSB[:, g, half, :]
                        nc.vector.reciprocal(
                            srec,
                            bass.AP(tensor=pv.tensor, offset=pv.offset + D,
                                    ap=[pv.ap[0], [P, NBLK]]))
                        natv = bass.AP(
                            tensor=natSB.tensor,
                            offset=natSB.offset + hh * D,
                            ap=[natSB.ap[0], [E, NBLK], [1, D]],
                        )
                        pvv = bass.AP(tensor=pv.tensor, offset=pv.offset,
                                      ap=[pv.ap[0], [P, NBLK], [1, D]])
                        srecb = bass.AP(
                            tensor=srecSB.tensor,
                            offset=srecSB.offset + u * NBLK,
                            ap=[srecSB.ap[0], [1, NBLK], [0, D]],
                        )
                        nc.vector.tensor_tensor(natv, pvv, srecb, mult)
                return go
            return [lambda f=mk(0): f(state), lambda f=mk(1): f(state)]

        def e_units(g):
            units = []
            for half in range(2):
                for J in range(NBLK):
                    def go(half=half, J=J):
                        u = 2 * g + half
                        if J == 0:
                            eP[u] = expp.tile([P, NBLK, S], fp8, tag="ept", name="eP")
                        lo = D * half
                        pa = psQK.tile([P, E], fp32, tag="qk", name="pa")
                        for ic in range(2):
                            nc.tensor.matmul(
                                pa[:, ic * 512:(ic + 1) * 512],
                                KT[lo:lo + D, g, :, J * P:(J + 1) * P],
                                QT[lo:lo + D, g, :,
                                   ic * 512:(ic + 1) * 512],
                                start=True, stop=True,
                                perf_mode=DRow, skip_group_check=True,
                            )
                        nc.scalar.activation(
                            out=eP[u][:, J, :], in_=pa, func=Exp,
                            scale=SCALE / 2048.0)
                    units.append(go)
            return units

        def emit_interleaved(chunks, units, gates=None):
            # spread E-units evenly between chunks; unit k may only be
            # emitted once gates[k] chunks are done (WAR: the eP slot it
            # allocates must have its reader PV already emitted).
            nc_, nu = len(chunks), len(units)
            if gates is None:
                gates = [0] * nu
            ui = 0
            for ci, ch in enumerate(chunks):
                ch()
                done = ci + 1
                want = done * nu // nc_
                while ui < want and ui < nu and gates[ui] <= done:
                    units[ui]()
                    ui += 1
            while ui < nu:
                units[ui]()
                ui += 1

        # ---- fused stage 1+2 ----
        with tc.tile_pool(name="psProj", bufs=2, space="PSUM") as psProj, \
             tc.tile_pool(name="s1fix", bufs=1) as s1fix:
            pools["psProj"] = psProj
            xT = s1fix.tile([P, KBLK, S], fp16, name="xT")
            x8T = s1fix.tile([P, KBLK, S], fp8, name="x8T")
            x8Tp = s1fix.tile([P, KBLK, S], fp8, name="x8Tp")
            w8q = s1fix.tile([P, KBLK, E], fp8, name="w8q")
            w8k = s1fix.tile([P, KBLK, E], fp8, name="w8k")
            wv_sb = s1fix.tile([P, KBLK, E], fp16, name="wv_sb")
            wv8_sb = s1fix.tile([P, KBLK, E], fp8, name="wv8_sb")
            # Input DMAs serialize on the DMA-engine device, so arrival
            # order = creation order. Load per-g column blocks of Wq/Wk so
            # the g0 QK chain (and the first exp) is gated by ~7us of DMA
            # instead of ~12us.
            wqr = wq8.rearrange("(kb kp) e -> kp kb e", kp=P)
            wkr = wk8.rearrange("(kb kp) e -> kp kb e", kp=P)
            nc.sync.dma_start(
                out=x8Tp, in_=xtp8.rearrange("(kb kp) s -> kp kb s", kp=P))
            nc.sync.dma_start(out=w8q[:, :, 0:P], in_=wqr[:, :, 0:P])
            nc.sync.dma_start(
                out=x8T, in_=xt8.rearrange("(kb kp) s -> kp kb s", kp=P))
            nc.sync.dma_start(out=w8k[:, :, 0:P], in_=wkr[:, :, 0:P])
            for g in (1,):
                nc.sync.dma_start(out=w8q[:, :, g * P:(g + 1) * P],
                                  in_=wqr[:, :, g * P:(g + 1) * P])
                nc.sync.dma_start(out=w8k[:, :, g * P:(g + 1) * P],
                                  in_=wkr[:, :, g * P:(g + 1) * P])
            nc.sync.dma_start(
                out=wv8_sb, in_=wv8.rearrange("(kb kp) e -> kp kb e", kp=P))
            for g in range(2, G):
                nc.sync.dma_start(out=w8q[:, :, g * P:(g + 1) * P],
                                  in_=wqr[:, :, g * P:(g + 1) * P])
                nc.sync.dma_start(out=w8k[:, :, g * P:(g + 1) * P],
                                  in_=wkr[:, :, g * P:(g + 1) * P])
            nc.sync.dma_start(
                out=xT, in_=xt16.rearrange("(kb kp) s -> kp kb s", kp=P))
            nc.sync.dma_start(
                out=wv_sb,
                in_=wv16.rearrange("(kb kp) e -> kp kb e", kp=P),
            )

            # Warmup feeds Act immediately: Q/K(0,1) projections first,
            # then E(0) units interleaved with the V chains; remaining
            # Q/K projections ride iter 1 alongside E(1). fp8 DR makes
            # projections cheap enough that the s1fix region (and the T2
            # fill) frees by ~60us into the run.
            for g in (0, 1):
                for c in proj_chunks(g, lambda: w8q, QT, lambda: x8Tp):
                    c()
                for c in proj_chunks(g, lambda: w8k, KT, lambda: x8T):
                    c()
            chunks = []
            for jb in range(4):
                chunks += v8proj_chunks(jb)
            emit_interleaved(chunks, e_units(0))
            chunks = []
            for g in range(2, G):
                chunks += proj_chunks(g, lambda: w8q, QT, lambda: x8Tp)
                chunks += proj_chunks(g, lambda: w8k, KT, lambda: x8T)
            for jb in range(4, 8):
                chunks += v8proj_chunks(jb)
            emit_interleaved(chunks, e_units(1))
            v16 = {2: [0, 1, 2], 3: [3, 4, 5], 4: [6, 7]}
            for i in range(2, 5):
                chunks = pv_chunks(i - 2, 0) + pv_chunks(i - 2, 1)
                for jb in v16[i]:
                    chunks += vproj_chunks(jb)
                emit_interleaved(chunks, e_units(i),
                                 [2] * 8 + [4] * 8)

        # ---- tail of stage 2 + stage 3 (T2 reuses the s1fix region) ----
        with tc.tile_pool(name="t2p", bufs=1) as t2p, \
             tc.tile_pool(name="lns", bufs=8) as lns, \
             tc.tile_pool(name="lnof", bufs=8) as lnof, \
             tc.tile_pool(name="bps", bufs=2, space="PSUM") as bps:
            T2 = t2p.tile([P, T2W], fp16, name="T2")
            # 4-slice fill, high offsets first: bias for head hh reads
            # [15360-1024hh, 32752-1024hh), so hh 0..7 only need the top
            # three slices and can start one slice-DMA earlier.
            for a, b in ((24576, T2W), (16384, 24576), (8192, 16384),
                         (0, 8192)):
                nc.sync.dma_start(
                    out=T2[:, a:b],
                    in_=bass.AP(tensor=flat16.tensor, offset=a,
                                ap=[[1, P], [1, b - a]]),
                )

            def bias_units(hh):
                # bias@V for head hh over all 8 F blocks, accumulated in a
                # [P, 512] psum tile (64-col slice per F), then added into
                # natSB. Runs inside the Act window instead of after it.
                # Emit only after pv_chunks for head hh (natv write order
                # on the in-order DVE queue).
                def go():
                    bt = bps.tile([P, NBLK * D], fp32, tag="bias",
                                  name="bias")
                    for F in range(NBLK):
                        base = 15360 - 1024 * hh + 2048 * F
                        for J in range(NBLK):
                            t2st = bass.AP(
                                tensor=T2.tensor,
                                offset=T2.offset + base + P * J,
                                ap=[T2.ap[0], [16, P]],
                            )
                            nc.tensor.matmul(
                                bt[:, F * D:(F + 1) * D], t2st,
                                VE[:, J, hh * DE:hh * DE + D],
                                start=(J == 0), stop=(J == NBLK - 1),
                                skip_group_check=True,
                            )
                    nat = bass.AP(
                        tensor=natSB.tensor, offset=natSB.offset + hh * D,
                        ap=[natSB.ap[0], [E, NBLK], [1, D]],
                    )
                    nc.vector.tensor_tensor(nat, nat, bt, add)
                    for F in range(NBLK):
                        nc.vector.bn_stats(
                            statsA[:, F, hh, :],
                            natSB[:, F, hh * D:(hh + 1) * D])
                return go

            # iters 5..7 + PV trail of the pipeline. Bias units are created
            # AFTER all attention work so the scheduler only backfills them
            # into PE-idle slots (created earlier they outrank the next
            # iter's QK matmuls in priority and get hoisted into solid
            # blocks that starve Act).
            for i in range(5, G):
                chunks = pv_chunks(i - 2, 0) + pv_chunks(i - 2, 1)
                emit_interleaved(chunks, e_units(i), [2] * 8 + [4] * 8)
            for c in pv_chunks(G - 2, 0) + pv_chunks(G - 2, 1):
                c()
            for c in pv_chunks(G - 1, 0) + pv_chunks(G - 1, 1):
                c()
            for hh in range(H):
                bias_units(hh)()

            # Two-pass LN tail: all stats→rstd/murs first (Act queue then
            # holds Sqrt0..7 before any apply, no head-of-line blocking),
            # then the applies alternate Pool / Act with out-DMA per F.
            rstds, murss = [], []
            for F in range(NBLK):
                mv = lns.tile([P, 2], fp32, tag="mv", name="mv")
                nc.vector.bn_aggr(mv, statsA[:, F, :, :])
                rstd = lns.tile([P, 1], fp32, tag="rstd", name="rstd")
                murs = lns.tile([P, 1], fp32, tag="murs", name="murs")
                nc.scalar.activation(out=rstd, in_=mv[:, 1:2],
                                     func=Sqrt, bias=epsT, scale=1.0)
                nc.vector.reciprocal(rstd, rstd)
                if F % 2 == 0:
                    # Pool apply: of = natSB*rstd - murs
                    nc.vector.tensor_tensor(murs, mv[:, 0:1], rstd, mult)
                else:
                    # Act apply: of = natSB*rstd + (-murs)
                    nc.vector.tensor_scalar(murs, mv[:, 0:1], rstd, -1.0,
                                            op0=mult, op1=mult)
                rstds.append(rstd)
                murss.append(murs)
            for F in range(NBLK):
                of = lnof.tile([P, E], fp16, tag="of", name="of")
                if F % 2 == 0:
                    nc.gpsimd.tensor_scalar(of, natSB[:, F, :], rstds[F],
                                            murss[F], op0=mult, op1=sub)
                else:
                    nc.scalar.activation(out=of, in_=natSB[:, F, :],
                                         func=Identity, bias=murss[F],
                                         scale=rstds[F])
                if not trivial_ln:
                    nc.vector.tensor_tensor(of, of, gamT, mult)
                    nc.vector.tensor_tensor(of, of, betT, add)
                nc.sync.dma_start(out[F * P:(F + 1) * P, :], of)

    nc.compile()
    return nc


def get_nc(trivial_ln: bool = True):
    if trivial_ln not in _BUILT:
        _BUILT[trivial_ln] = _build(trivial_ln)
    return _BUILT[trivial_ln]


def make_in_maps(inputs):
    x = np.asarray(inputs["x"])
    rel = np.asarray(inputs["rel_table"])
    gamma = np.asarray(inputs["gamma"])
    beta = np.asarray(inputs["beta"])
    trivial_ln = bool(np.all(gamma == 1.0) and np.all(beta == 0.0))

    import ml_dtypes
    f8 = ml_dtypes.float8_e4m3fn
    x16 = x.astype(np.float16)
    xt16 = np.ascontiguousarray(x16.transpose(0, 2, 1))          # (B, E, S)
    xt8 = np.ascontiguousarray(x.transpose(0, 2, 1).astype(f8))
    xtp8 = np.ascontiguousarray(x[:, SIGMA, :].transpose(0, 2, 1).astype(f8))
    # q/k weights pre-scaled by 32 (fp8 sweet spot); exp scale absorbs 1/1024
    wq8 = (np.asarray(inputs["Wq"]) * 32.0).astype(f8)
    wk8 = (np.asarray(inputs["Wk"]) * 32.0).astype(f8)
    wv16 = np.asarray(inputs["Wv"]).astype(np.float16)
    wv8 = (np.asarray(inputs["Wv"]) * 32.0).astype(f8)
    flat16 = np.ascontiguousarray(rel.reshape(-1).astype(np.float16))

    in_maps = []
    for b in range(x.shape[0]):
        m = {"xt16": xt16[b], "xt8": xt8[b], "xtp8": xtp8[b],
             "wq8": wq8, "wk8": wk8, "wv16": wv16, "wv8": wv8,
             "flat16": flat16}
        if not trivial_ln:
            m["gamma"] = gamma.reshape(1, E).astype(np.float32)
            m["beta"] = beta.reshape(1, E).astype(np.float32)
        in_maps.append(m)
    return in_maps, trivial_ln


def unpermute(raw):
    """raw: (..., S, E) rows in processing order -> natural order."""
    fixed = np.empty_like(raw)
    fixed[..., SIGMA, :] = raw
    return fixed


def kernel(**inputs) -> np.ndarray:
    from concourse import bass_utils

    in_maps, trivial_ln = make_in_maps(inputs)
    nc = get_nc(trivial_ln)
    res = bass_utils.run_bass_kernel_spmd(nc, in_maps,
                                          core_ids=list(range(len(in_maps))))
    outs = np.stack([r["out"] for r in res.results])
    return unpermute(outs).astype(np.float32)



# revision 43
# speedup vs baseline: 1.2023x; 1.0417x over previous
"""Trainium2 Bass kernel for nn_Attention_Rel_Scl (B=8,S=1024,E=1024,H=16).

Data-parallel over batch: one batch element per NeuronCore (8 cores).

v7: v6 + (a) QK^T itself as fp8 DoubleRow: QT/KT are fp8 [P, G, 2, S]
  with the r=1 slice a DMA duplicate of r=0, so the DR matmul contracts
  the 64 head dims twice (exp scale absorbs the 2x) at 0.5 cycles/row;
  (b) bias@V moved inside the Act(exp) window: per-head bias units
  (T2 streamed in 4 slices) accumulate in [P,512] psum and add into
  natSB (now fp16) right after that head's PV, so the old 27us post-exp
  bias tail disappears; LN stats read natSB directly (comb removed).

v6: v5 + fp8e4 DoubleRow matmuls for Q/K projections and PV.
  - exp(QK^T/sqrt(E)) is the *stationary* operand of PV / colsum / biasV
    matmuls, so those cost only (out free size) PE cycles and the result
    lands directly in natural [row, feature] orientation (no transposes,
    no gathers). V carries an interleaved 1.0 column per head so PV and
    the softmax denominator come from one moving stream.
  - Emission interleaves 2 QK+exp J-steps between every ~2us PE chunk
    (projection half-chains, PV half-blocks): the in-order engines then
    pace each other without head-of-line stalls; Act (the 133us exp
    budget) starts ~15us in and stays ~full.
  - QT/KT/VE psum->SBUF copies run on GpSimd (Pool) so the DVE's
    reciprocal (which waits on PV groups) never blocks them.
  - T2 (bias table, 63.7KB/part) is DMA-filled into the region freed by
    the projection inputs, overlapping the back half of stage 2.
  - bias[h,i,j] = flat[(16368-1024h) + 1024*(i%16) - 16*(i//16) + j]
    (flat = rel_table.reshape(-1), clip never fires); rows processed in
    order f -> SIGMA[f] = 16*(63-f%64) + f//64 make the bias block for
    (hh, F, J) the T2 view at offset 15360-1024*hh+2048*F+128*J with
    ap [[1,128],[1024,2],[16,64]], T2[p,w] = flat[p+w].
  - LayerNorm in natural layout; combine-add + normalize-apply on Pool,
    bn_stats/aggr/recip on DVE, Sqrt on Act. Contiguous output DMA; host
    un-permutes rows (SIGMA).
"""

import sys

if "/opt/trn_rl_repo" not in sys.path:
    sys.path.insert(0, "/opt/trn_rl_repo")

import numpy as np

B, S, E, H = 8, 1024, 1024, 16
D = E // H          # 64 head dim
P = 128             # partitions
G = H // 2          # 8 head pairs
NBLK = S // P       # 8 key/query blocks
KBLK = E // P       # 8 contraction blocks
EPS = 1e-3
SCALE = float(E) ** -0.5
FLAT = (2 * S - 1) * H   # 32752
T2W = 32625              # max free offset 32624 (+p<=127 -> 32751 = FLAT-1)
DE = D + 1               # 65: V column block plus ones column

_f = np.arange(S)
SIGMA = 16 * (63 - _f % 64) + _f // 64

_BUILT = {}


def _build(trivial_ln: bool):
    import concourse.bass as bass
    import concourse.tile as tile
    from concourse import bacc, mybir
    from contextlib import ExitStack

    fp16 = mybir.dt.float16
    fp32 = mybir.dt.float32
    Exp = mybir.ActivationFunctionType.Exp
    Sqrt = mybir.ActivationFunctionType.Sqrt
    Identity = mybir.ActivationFunctionType.Identity
    mult = mybir.AluOpType.mult
    add = mybir.AluOpType.add
    sub = mybir.AluOpType.subtract

    nc = bacc.Bacc("TRN2", target_bir_lowering=False, debug=False,
                   num_devices=8)

    fp8 = mybir.dt.float8e4
    DRow = mybir.MatmulPerfMode.DoubleRow
    xt16 = nc.dram_tensor("xt16", [E, S], fp16, kind="ExternalInput").ap()
    xt8 = nc.dram_tensor("xt8", [E, S], fp8, kind="ExternalInput").ap()
    xtp8 = nc.dram_tensor("xtp8", [E, S], fp8, kind="ExternalInput").ap()
    wq8 = nc.dram_tensor("wq8", [E, E], fp8, kind="ExternalInput").ap()
    wk8 = nc.dram_tensor("wk8", [E, E], fp8, kind="ExternalInput").ap()
    wv16 = nc.dram_tensor("wv16", [E, E], fp16, kind="ExternalInput").ap()
    wv8 = nc.dram_tensor("wv8", [E, E], fp8, kind="ExternalInput").ap()
    flat16 = nc.dram_tensor("flat16", [FLAT], fp16, kind="ExternalInput").ap()
    if not trivial_ln:
        gam = nc.dram_tensor("gamma", [1, E], fp32, kind="ExternalInput").ap()
        bet = nc.dram_tensor("beta", [1, E], fp32, kind="ExternalInput").ap()
    # fp16 output (host casts to fp32): LN output is ~N(0,1), fp16
    # rounding is ~5e-4 relative — halves the out-DMA tail.
    out = nc.dram_tensor("out", [S, E], fp16, kind="ExternalOutput").ap()

    with tile.TileContext(nc) as tc, ExitStack() as ctx:
        persist = ctx.enter_context(tc.tile_pool(name="persist", bufs=1))
        QT = persist.tile([P, G, 2, S], fp8, name="QT")
        KT = persist.tile([P, G, 2, S], fp8, name="KT")
        VE = persist.tile([P, NBLK, H * DE], fp16, name="VE")
        VE8 = persist.tile([P, NBLK, H * DE], fp8, name="VE8")
        natSB = persist.tile([P, NBLK, E], fp16, name="natSB")
        srecSB = persist.tile([P, G, 2, NBLK], fp32, name="srecSB")
        epsT = persist.tile([P, 1], fp32, name="epsT")

        nc.vector.memset(epsT, EPS)
        nc.vector.memset(
            bass.AP(tensor=VE.tensor, offset=VE.offset + D,
                    ap=[VE.ap[0], [H * DE, NBLK], [DE, H]]),
            1.0)
        nc.vector.memset(
            bass.AP(tensor=VE8.tensor, offset=VE8.offset + D,
                    ap=[VE8.ap[0], [H * DE, NBLK], [DE, H]]),
            32.0)

        if not trivial_ln:
            gamT = persist.tile([P, E], fp32, name="gamT")
            betT = persist.tile([P, E], fp32, name="betT")
            nc.sync.dma_start(
                out=gamT,
                in_=bass.AP(tensor=gam.tensor, offset=0, ap=[[0, P], [1, E]]),
            )
            nc.sync.dma_start(
                out=betT,
                in_=bass.AP(tensor=bet.tensor, offset=0, ap=[[0, P], [1, E]]),
            )

        expp = ctx.enter_context(tc.tile_pool(name="expp", bufs=4))
        # T2 bias table, split in two sliding windows so each can load as
        # early as SBUF frees: T2b (the high-offset window, 27.7KB) fits
        # beside the stage-1 inputs and loads right after them; T2a
        # (39.7KB) reuses the Q/K-input region that dies after iter 1.
        # Group (hh, F) with base = 15360-1024*hh+2048*F reads
        # T2a[base + 128J + p + 16w] if base <= 17408 (view max 20463),
        # else T2b at offset base-18432 (flat index 18432 + ...).
        T2AW = 20352
        T2BO = 18432
        T2BW = FLAT - T2BO - 127   # 14193
        t2bp = ctx.enter_context(tc.tile_pool(name="t2bp", bufs=1))
        T2b = t2bp.tile([P, T2BW], fp16, name="T2b")
        psQK = ctx.enter_context(
            tc.tile_pool(name="psQK", bufs=2, space="PSUM"))
        pvp = ctx.enter_context(
            tc.tile_pool(name="pvp", bufs=1, space="PSUM"))

        eP = {}
        pools = {}

        # ---- emission helpers: each returns a list of closures ("chunks");
        # E-units (one QK J-step + exp) are interleaved between chunks.
        def proj_chunks(g, w8get, dst, rhs8get):
            # fp8 DoubleRow: contraction 1024 as 4 steps of 2x128.
            # Per-ic [P,512] psum tiles (bufs=2) let the DVE copy of ic0
            # overlap the matmuls of ic1 / the next chain.
            # dst is [P, G, 2, S] fp8; the r=1 slice is a DMA duplicate of
            # r=0 so QK can run as a DoubleRow matmul (contracting the 64
            # head dims twice; exp scale absorbs the factor 2).
            def go():
                w8, rhs8 = w8get(), rhs8get()
                for ic in range(2):
                    pt = pools["psProj"].tile([P, 512], fp32, tag="proj",
                                              name="pt")
                    for kp in range(4):
                        nc.tensor.matmul(
                            pt,
                            w8[:, 2 * kp:2 * kp + 2, g * P:(g + 1) * P],
                            rhs8[:, 2 * kp:2 * kp + 2,
                                 ic * 512:(ic + 1) * 512],
                            start=(kp == 0), stop=(kp == 3),
                            perf_mode=DRow, skip_group_check=True,
                        )
                    nc.vector.tensor_copy(
                        dst[:, g, 0, ic * 512:(ic + 1) * 512], pt)
                    # duplicate r=0 -> r=1 (GPSIMD cannot read PSUM on hw).
                    # g0/g1 feed the first exps: keep their dup on DVE to
                    # avoid the cross-engine hop on the warmup critical path.
                    eng = nc.vector if g < 2 else nc.gpsimd
                    eng.tensor_copy(
                        dst[:, g, 1, ic * 512:(ic + 1) * 512],
                        dst[:, g, 0, ic * 512:(ic + 1) * 512])
            return [go]

        def v8proj_chunks(jb):
            # fp8 DR V projection feeding VE8 (PV path) only
            def mk(ic):
                def go():
                    bt = pvp.tile([P, NBLK, P], fp32, tag="pv", name="pv")
                    pt = bass.AP(tensor=bt.tensor, offset=bt.offset,
                                 ap=[bt.ap[0], [1, 512]])
                    for kp in range(4):
                        nc.tensor.matmul(
                            pt,
                            x8T[:, 2 * kp:2 * kp + 2, jb * P:(jb + 1) * P],
                            wv8_sb[:, 2 * kp:2 * kp + 2,
                                   ic * 512:(ic + 1) * 512],
                            start=(kp == 0), stop=(kp == 3),
                            perf_mode=DRow, skip_group_check=True,
                        )
                    dstv8 = bass.AP(
                        tensor=VE8.tensor,
                        offset=VE8.offset + jb * (H * DE) + ic * 8 * DE,
                        ap=[VE8.ap[0], [DE, 8], [1, D]],
                    )
                    nc.vector.tensor_copy(dstv8, pt)
                return go
            return [mk(0), mk(1)]

        def vproj_chunks(jb):
            # V16 runs on the psProj ring (idle after iter 1), decoupled
            # from the PV ring so all 16 chunks can finish by ~iter 3 and
            # unblock the bias matmuls (which read all of VE).
            def mk(ic):
                def go():
                    pt = pools["psProj"].tile([P, 512], fp32, tag="proj",
                                              name="pt")
                    for kb in range(KBLK):
                        nc.tensor.matmul(
                            pt,
                            xT[:, kb, jb * P:(jb + 1) * P],
                            wv_sb[:, kb, ic * 512:(ic + 1) * 512],
                            start=(kb == 0), stop=(kb == KBLK - 1),
                            skip_group_check=True,
                        )
                    dstv = bass.AP(
                        tensor=VE.tensor,
                        offset=VE.offset + jb * (H * DE) + ic * 8 * DE,
                        ap=[VE.ap[0], [DE, 8], [1, D]],
                    )
                    nc.vector.tensor_copy(dstv, pt)
                return go
            return [mk(0), mk(1)]

        def pv_chunks(g, half):
            u = 2 * g + half
            hh = u
            state = {}

            def mk(fh):
                def go(st):
                    if fh == 0:
                        st["pv"] = pvp.tile([P, NBLK, P], fp32, tag="pv", name="pv")
                    pv = st["pv"]
                    for F in range(4 * fh, 4 * fh + 4):
                        for Jp in range(4):
                            nc.tensor.matmul(
                                pv[:, F, 0:DE],
                                eP[u][:, 2 * Jp:2 * Jp + 2,
                                      F * P:(F + 1) * P],
                                VE8[:, 2 * Jp:2 * Jp + 2,
                                    hh * DE:(hh + 1) * DE],
                                start=(Jp == 0), stop=(Jp == 3),
                                perf_mode=DRow, skip_group_check=True,
                            )
                    if fh == 1:
                        del eP[u]
                        srec = srecSB[:, g, half, :]
                        nc.vector.reciprocal(
                            srec,
                            bass.AP(tensor=pv.tensor, offset=pv.offset + D,
                                    ap=[pv.ap[0], [P, NBLK]]))
                        natv = bass.AP(
                            tensor=natSB.tensor,
                            offset=natSB.offset + hh * D,
                            ap=[natSB.ap[0], [E, NBLK], [1, D]],
                        )
                        pvv = bass.AP(tensor=pv.tensor, offset=pv.offset,
                                      ap=[pv.ap[0], [P, NBLK], [1, D]])
                        srecb = bass.AP(
                            tensor=srecSB.tensor,
                            offset=srecSB.offset + u * NBLK,
                            ap=[srecSB.ap[0], [1, NBLK], [0, D]],
                        )
                        if u < 6:
                            # early heads: attn@V lands first, the bias
                            # unit later ADDS into natSB.
                            nc.vector.tensor_tensor(natv, pvv, srecb, mult)
                        else:
                            # late heads: the bias COPY (created earlier,
                            # eligible early) already filled natSB; add
                            # the normalized attention on top, then stats.
                            tmp = pools["tmpp"].tile([P, NBLK * D], fp32,
                                                     tag="tmp", name="tmp")
                            nc.vector.tensor_tensor(tmp, pvv, srecb, mult)
                            nc.vector.tensor_tensor(natv, natv, tmp, add)
                            for F in range(NBLK):
                                nc.vector.bn_stats(
                                    statsA[:, F, u, :],
                                    natSB[:, F, u * D:(u + 1) * D])
                return go
            return [lambda f=mk(0): f(state), lambda f=mk(1): f(state)]

        def e_units(g):
            units = []
            for half in range(2):
                for J in range(NBLK):
                    def go(half=half, J=J):
                        u = 2 * g + half
                        if J == 0:
                            eP[u] = expp.tile([P, NBLK, S], fp8, tag="ept", name="eP")
                        lo = D * half
                        pa = psQK.tile([P, E], fp32, tag="qk", name="pa")
                        for ic in range(2):
                            nc.tensor.matmul(
                                pa[:, ic * 512:(ic + 1) * 512],
                                KT[lo:lo + D, g, :, J * P:(J + 1) * P],
                                QT[lo:lo + D, g, :,
                                   ic * 512:(ic + 1) * 512],
                                start=True, stop=True,
                                perf_mode=DRow, skip_group_check=True,
                            )
                        nc.scalar.activation(
                            out=eP[u][:, J, :], in_=pa, func=Exp,
                            scale=SCALE / 2048.0)
                    units.append(go)
            return units

        def emit_interleaved(chunks, units, gates=None):
            # spread E-units evenly between chunks; unit k may only be
            # emitted once gates[k] chunks are done (WAR: the eP slot it
            # allocates must have its reader PV already emitted).
            nc_, nu = len(chunks), len(units)
            if gates is None:
                gates = [0] * nu
            ui = 0
            for ci, ch in enumerate(chunks):
                ch()
                done = ci + 1
                want = done * nu // nc_
                while ui < want and ui < nu and gates[ui] <= done:
                    units[ui]()
                    ui += 1
            while ui < nu:
                units[ui]()
                ui += 1

        # ---- fused stage 1+2 ----
        with tc.tile_pool(name="psProj", bufs=2, space="PSUM") as psProj, \
             tc.tile_pool(name="s1fix", bufs=1) as s1fix:
            pools["psProj"] = psProj
            xT = s1fix.tile([P, KBLK, S], fp16, name="xT")
            x8T = s1fix.tile([P, KBLK, S], fp8, name="x8T")
            x8Tp = s1fix.tile([P, KBLK, S], fp8, name="x8Tp")
            w8q = s1fix.tile([P, KBLK, E], fp8, name="w8q")
            w8k = s1fix.tile([P, KBLK, E], fp8, name="w8k")
            wv_sb = s1fix.tile([P, KBLK, E], fp16, name="wv_sb")
            wv8_sb = s1fix.tile([P, KBLK, E], fp8, name="wv8_sb")
            # Input DMAs serialize on the DMA-engine device, so arrival
            # order = creation order. Load per-g column blocks of Wq/Wk so
            # the g0 QK chain (and the first exp) is gated by ~7us of DMA
            # instead of ~12us.
            wqr = wq8.rearrange("(kb kp) e -> kp kb e", kp=P)
            wkr = wk8.rearrange("(kb kp) e -> kp kb e", kp=P)
            nc.sync.dma_start(
                out=x8Tp, in_=xtp8.rearrange("(kb kp) s -> kp kb s", kp=P))
            nc.sync.dma_start(out=w8q[:, :, 0:P], in_=wqr[:, :, 0:P])
            nc.sync.dma_start(
                out=x8T, in_=xt8.rearrange("(kb kp) s -> kp kb s", kp=P))
            nc.sync.dma_start(out=w8k[:, :, 0:P], in_=wkr[:, :, 0:P])
            for g in (1,):
                nc.sync.dma_start(out=w8q[:, :, g * P:(g + 1) * P],
                                  in_=wqr[:, :, g * P:(g + 1) * P])
                nc.sync.dma_start(out=w8k[:, :, g * P:(g + 1) * P],
                                  in_=wkr[:, :, g * P:(g + 1) * P])
            nc.sync.dma_start(
                out=wv8_sb, in_=wv8.rearrange("(kb kp) e -> kp kb e", kp=P))
            for g in range(2, G):
                nc.sync.dma_start(out=w8q[:, :, g * P:(g + 1) * P],
                                  in_=wqr[:, :, g * P:(g + 1) * P])
                nc.sync.dma_start(out=w8k[:, :, g * P:(g + 1) * P],
                                  in_=wkr[:, :, g * P:(g + 1) * P])
            nc.sync.dma_start(
                out=xT, in_=xt16.rearrange("(kb kp) s -> kp kb s", kp=P))
            nc.sync.dma_start(
                out=wv_sb,
                in_=wv16.rearrange("(kb kp) e -> kp kb e", kp=P),
            )
            # T2b has no region conflicts: loads right after the inputs
            nc.sync.dma_start(
                out=T2b,
                in_=bass.AP(tensor=flat16.tensor, offset=T2BO,
                            ap=[[1, P], [1, T2BW]]),
            )

            # Warmup feeds Act immediately: Q/K(0,1) projections first,
            # then E(0) units interleaved with the V chains; remaining
            # Q/K projections ride iter 1 alongside E(1). fp8 DR makes
            # projections cheap enough that the s1fix region (and the T2
            # fill) frees by ~60us into the run.
            for g in (0, 1):
                for c in proj_chunks(g, lambda: w8q, QT, lambda: x8Tp):
                    c()
                for c in proj_chunks(g, lambda: w8k, KT, lambda: x8T):
                    c()
            chunks = []
            for jb in range(4):
                chunks += v8proj_chunks(jb)
            emit_interleaved(chunks, e_units(0))
            chunks = []
            for g in range(2, G):
                chunks += proj_chunks(g, lambda: w8q, QT, lambda: x8Tp)
                chunks += proj_chunks(g, lambda: w8k, KT, lambda: x8T)
            for jb in range(4, 8):
                chunks += v8proj_chunks(jb)
            emit_interleaved(chunks, e_units(1))
            v16 = {2: [0, 1, 2, 3], 3: [4, 5, 6, 7], 4: []}
            for i in range(2, 5):
                chunks = pv_chunks(i - 2, 0) + pv_chunks(i - 2, 1)
                for jb in v16[i]:
                    chunks += vproj_chunks(jb)
                emit_interleaved(chunks, e_units(i),
                                 [2] * 8 + [4] * 8)

        # ---- tail of stage 2 + stage 3 (T2 reuses the s1fix region) ----
        with tc.tile_pool(name="t2p", bufs=1) as t2p, \
             tc.tile_pool(name="lns", bufs=8) as lns, \
             tc.tile_pool(name="lnof", bufs=8) as lnof, \
             tc.tile_pool(name="bps", bufs=2, space="PSUM") as bps, \
             tc.tile_pool(name="tmpp", bufs=3) as tmpp:
            pools["tmpp"] = tmpp
            statsA = t2p.tile([P, NBLK, H, 6], fp32, name="statsA")
            # T2a (covers groups with base <= 17408) reuses the freed
            # s1fix region; 3-slice fill starts as soon as vproj's last
            # read of xT/wv_sb retires (~iter 3 with vproj on psProj).
            T2a = t2p.tile([P, T2AW], fp16, name="T2a")
            for a, b in ((0, 6784), (6784, 13568), (13568, T2AW)):
                nc.sync.dma_start(
                    out=T2a[:, a:b],
                    in_=bass.AP(tensor=flat16.tensor, offset=a,
                                ap=[[1, P], [1, b - a]]),
                )

            def bias_units(hh):
                # bias@V for head hh over all 8 F blocks, accumulated in a
                # [P, 512] psum tile (64-col slice per F). Late heads
                # (>=6, created before their PV) COPY into natSB — the PV
                # then adds normalized attention on top; early heads (<6,
                # created last) ADD into natSB behind their natv + stats.
                def go():
                    bt = bps.tile([P, NBLK * D], fp32, tag="bias",
                                  name="bias")
                    for F in range(NBLK):
                        base = 15360 - 1024 * hh + 2048 * F
                        if base <= 17408:
                            tsr, off = T2a, base
                        else:
                            tsr, off = T2b, base - T2BO
                        for J in range(NBLK):
                            t2st = bass.AP(
                                tensor=tsr.tensor,
                                offset=tsr.offset + off + P * J,
                                ap=[tsr.ap[0], [16, P]],
                            )
                            nc.tensor.matmul(
                                bt[:, F * D:(F + 1) * D], t2st,
                                VE[:, J, hh * DE:hh * DE + D],
                                start=(J == 0), stop=(J == NBLK - 1),
                                skip_group_check=True,
                            )
                    nat = bass.AP(
                        tensor=natSB.tensor, offset=natSB.offset + hh * D,
                        ap=[natSB.ap[0], [E, NBLK], [1, D]],
                    )
                    if hh >= 6:
                        nc.vector.tensor_copy(nat, bt)
                    else:
                        nc.vector.tensor_tensor(nat, nat, bt, add)
                        for F in range(NBLK):
                            nc.vector.bn_stats(
                                statsA[:, F, hh, :],
                                natSB[:, F, hh * D:(hh + 1) * D])
                return go

            # iters 5..7 with bias units spread as chunks (PE order is
            # ~creation order, so each iter carries only what its Act
            # window affords). Copy-scheme heads (>=6) are created just
            # before the iter holding their PV; add-scheme heads (0-5)
            # ride along (their natv landed in iters 2-4).
            # The two copy-scheme units for THIS iter's PV heads must be
            # created BEFORE the pv chunks (the PV add reads natSB on top
            # of the bias copy); the eP-slot gates shift by 2 accordingly.
            biassched = {5: ([6, 7], [12, 0, 1]), 6: ([8, 9], [13, 2, 3]),
                         7: ([10, 11], [14, 15, 4, 5])}
            for i in range(5, G):
                pre, post = biassched[i]
                chunks = ([bias_units(hh) for hh in pre]
                          + pv_chunks(i - 2, 0) + pv_chunks(i - 2, 1)
                          + [bias_units(hh) for hh in post])
                emit_interleaved(chunks, e_units(i), [4] * 8 + [6] * 8)
            for c in pv_chunks(G - 2, 0) + pv_chunks(G - 2, 1):
                c()
            for c in pv_chunks(G - 1, 0) + pv_chunks(G - 1, 1):
                c()

            # LN front: aggr/sqrt/recip/murs per F
            mvs, rstds, murss = [], [], []
            for F in range(NBLK):
                mv = lns.tile([P, 2], fp32, tag="mv", name="mv")
                nc.vector.bn_aggr(mv, statsA[:, F, :, :])
                rstd = lns.tile([P, 1], fp32, tag="rstd", name="rstd")
                nc.scalar.activation(out=rstd, in_=mv[:, 1:2],
                                     func=Sqrt, bias=epsT, scale=1.0)
                mvs.append(mv)
                rstds.append(rstd)
            for F in range(NBLK):
                murs = lns.tile([P, 1], fp32, tag="murs", name="murs")
                nc.vector.reciprocal(rstds[F], rstds[F])
                if F % 2 == 0:
                    nc.vector.tensor_tensor(murs, mvs[F][:, 0:1],
                                            rstds[F], mult)
                else:
                    nc.vector.tensor_scalar(murs, mvs[F][:, 0:1],
                                            rstds[F], -1.0,
                                            op0=mult, op1=mult)
                murss.append(murs)

            # LN pass 2: applies alternate Pool / Act with out-DMA per F.
            for F in range(NBLK):
                of = lnof.tile([P, E], fp16, tag="of", name="of")
                if F % 2 == 0:
                    nc.gpsimd.tensor_scalar(of, natSB[:, F, :], rstds[F],
                                            murss[F], op0=mult, op1=sub)
                else:
                    nc.scalar.activation(out=of, in_=natSB[:, F, :],
                                         func=Identity, bias=murss[F],
                                         scale=rstds[F])
                if not trivial_ln:
                    nc.vector.tensor_tensor(of, of, gamT, mult)
                    nc.vector.tensor_tensor(of, of, betT, add)
                nc.sync.dma_start(out[F * P:(F + 1) * P, :], of)

    nc.compile()
    return nc


def get_nc(trivial_ln: bool = True):
    if trivial_ln not in _BUILT:
        _BUILT[trivial_ln] = _build(trivial_ln)
    return _BUILT[trivial_ln]


def make_in_maps(inputs):
    x = np.asarray(inputs["x"])
    rel = np.asarray(inputs["rel_table"])
    gamma = np.asarray(inputs["gamma"])
    beta = np.asarray(inputs["beta"])
    trivial_ln = bool(np.all(gamma == 1.0) and np.all(beta == 0.0))

    import ml_dtypes
    f8 = ml_dtypes.float8_e4m3fn
    x16 = x.astype(np.float16)
    xt16 = np.ascontiguousarray(x16.transpose(0, 2, 1))          # (B, E, S)
    xt8 = np.ascontiguousarray(x.transpose(0, 2, 1).astype(f8))
    xtp8 = np.ascontiguousarray(x[:, SIGMA, :].transpose(0, 2, 1).astype(f8))
    # q/k weights pre-scaled by 32 (fp8 sweet spot); exp scale absorbs 1/1024
    wq8 = (np.asarray(inputs["Wq"]) * 32.0).astype(f8)
    wk8 = (np.asarray(inputs["Wk"]) * 32.0).astype(f8)
    wv16 = np.asarray(inputs["Wv"]).astype(np.float16)
    wv8 = (np.asarray(inputs["Wv"]) * 32.0).astype(f8)
    flat16 = np.ascontiguousarray(rel.reshape(-1).astype(np.float16))

    in_maps = []
    for b in range(x.shape[0]):
        m = {"xt16": xt16[b], "xt8": xt8[b], "xtp8": xtp8[b],
             "wq8": wq8, "wk8": wk8, "wv16": wv16, "wv8": wv8,
             "flat16": flat16}
        if not trivial_ln:
            m["gamma"] = gamma.reshape(1, E).astype(np.float32)
            m["beta"] = beta.reshape(1, E).astype(np.float32)
        in_maps.append(m)
    return in_maps, trivial_ln


def unpermute(raw):
    """raw: (..., S, E) rows in processing order -> natural order."""
    fixed = np.empty_like(raw)
    fixed[..., SIGMA, :] = raw
    return fixed


def kernel(**inputs) -> np.ndarray:
    from concourse import bass_utils

    in_maps, trivial_ln = make_in_maps(inputs)
    nc = get_nc(trivial_ln)
    res = bass_utils.run_bass_kernel_spmd(nc, in_maps,
                                          core_ids=list(range(len(in_maps))))
    outs = np.stack([r["out"] for r in res.results])
    return unpermute(outs).astype(np.float32)



# revision 45
# speedup vs baseline: 1.2118x; 1.0079x over previous
"""Trainium2 Bass kernel for nn_Attention_Rel_Scl (B=8,S=1024,E=1024,H=16).

Data-parallel over batch: one batch element per NeuronCore (8 cores).

v8 (164976ns model, from v6's 198352): Act/exp (133us busy) is the
hard floor — TimelineSim charges matmuls out_free x 0.4167ns x cpr
(fp8e4 DoubleRow cpr=0.5, contraction length free) and activations
free_size x 0.833ns; psum (16KB) caps exp tiles at [128,1024].
  (a) QK^T as fp8 DoubleRow: QT/KT fp8 [P, G, 2, S], r=1 a copy of r=0
  (DVE for g0/g1 warmup, Pool after; Pool cannot read PSUM so the dup
  chains off the SBUF r=0 slice). The DR matmul contracts the 64 head
  dims twice; exp scale absorbs the 2x. PE 139->112us.
  (b) bias@V inside the exp window: per-head [P,512] psum accumulation.
  T2 split in two sliding windows (T2b 27.7KB loads at t~16us beside
  the stage-1 inputs, T2a 39.7KB reuses s1fix right after vproj), and
  vproj runs on the psProj ring so VE completes ~iter 3. Heads 6-15:
  bias COPIES into natSB (fp16) before their PV, whose add rides on
  top (tmp = pv*srec; natSB += tmp) — drains psum without waiting
  natv; heads 0-5 add after their early natv. Scheduler note: PE static
  order ~= creation order, so bias units are spread per-iter as chunks
  sized to each iter's Act budget (a block created too early starves
  Act for its full duration).
  (c) tail: per-(F,head) bn_stats ride each bias-add; LN front fused
  into head 15's PV; applies alternate Pool / Act(Identity, bias=-mu*
  rstd); fp16 'of'/output (host casts) halves the out-DMA; wide lnof
  ring avoids WAR ping-pong. Tail ~15us after the last exp.
  First exp ~13us (w8q/w8k loaded per-g block; DMA device serializes).

v6: v5 + fp8e4 DoubleRow matmuls for Q/K projections and PV.
  - exp(QK^T/sqrt(E)) is the *stationary* operand of PV / colsum / biasV
    matmuls, so those cost only (out free size) PE cycles and the result
    lands directly in natural [row, feature] orientation (no transposes,
    no gathers). V carries an interleaved 1.0 column per head so PV and
    the softmax denominator come from one moving stream.
  - Emission interleaves 2 QK+exp J-steps between every ~2us PE chunk
    (projection half-chains, PV half-blocks): the in-order engines then
    pace each other without head-of-line stalls; Act (the 133us exp
    budget) starts ~15us in and stays ~full.
  - QT/KT/VE psum->SBUF copies run on GpSimd (Pool) so the DVE's
    reciprocal (which waits on PV groups) never blocks them.
  - T2 (bias table, 63.7KB/part) is DMA-filled into the region freed by
    the projection inputs, overlapping the back half of stage 2.
  - bias[h,i,j] = flat[(16368-1024h) + 1024*(i%16) - 16*(i//16) + j]
    (flat = rel_table.reshape(-1), clip never fires); rows processed in
    order f -> SIGMA[f] = 16*(63-f%64) + f//64 make the bias block for
    (hh, F, J) the T2 view at offset 15360-1024*hh+2048*F+128*J with
    ap [[1,128],[1024,2],[16,64]], T2[p,w] = flat[p+w].
  - LayerNorm in natural layout; combine-add + normalize-apply on Pool,
    bn_stats/aggr/recip on DVE, Sqrt on Act. Contiguous output DMA; host
    un-permutes rows (SIGMA).
"""

import sys

if "/opt/trn_rl_repo" not in sys.path:
    sys.path.insert(0, "/opt/trn_rl_repo")

import numpy as np

B, S, E, H = 8, 1024, 1024, 16
D = E // H          # 64 head dim
P = 128             # partitions
G = H // 2          # 8 head pairs
NBLK = S // P       # 8 key/query blocks
KBLK = E // P       # 8 contraction blocks
EPS = 1e-3
SCALE = float(E) ** -0.5
FLAT = (2 * S - 1) * H   # 32752
T2W = 32625              # max free offset 32624 (+p<=127 -> 32751 = FLAT-1)
DE = D + 1               # 65: V column block plus ones column

_f = np.arange(S)
SIGMA = 16 * (63 - _f % 64) + _f // 64

_BUILT = {}


def _build(trivial_ln: bool):
    import concourse.bass as bass
    import concourse.tile as tile
    from concourse import bacc, mybir
    from contextlib import ExitStack

    fp16 = mybir.dt.float16
    fp32 = mybir.dt.float32
    Exp = mybir.ActivationFunctionType.Exp
    Sqrt = mybir.ActivationFunctionType.Sqrt
    Identity = mybir.ActivationFunctionType.Identity
    mult = mybir.AluOpType.mult
    add = mybir.AluOpType.add
    sub = mybir.AluOpType.subtract

    nc = bacc.Bacc("TRN2", target_bir_lowering=False, debug=False,
                   num_devices=8)

    fp8 = mybir.dt.float8e4
    DRow = mybir.MatmulPerfMode.DoubleRow
    xt16 = nc.dram_tensor("xt16", [E, S], fp16, kind="ExternalInput").ap()
    xt8 = nc.dram_tensor("xt8", [E, S], fp8, kind="ExternalInput").ap()
    xtp8 = nc.dram_tensor("xtp8", [E, S], fp8, kind="ExternalInput").ap()
    wq8 = nc.dram_tensor("wq8", [E, E], fp8, kind="ExternalInput").ap()
    wk8 = nc.dram_tensor("wk8", [E, E], fp8, kind="ExternalInput").ap()
    wv16 = nc.dram_tensor("wv16", [E, E], fp16, kind="ExternalInput").ap()
    wv8 = nc.dram_tensor("wv8", [E, E], fp8, kind="ExternalInput").ap()
    flat16 = nc.dram_tensor("flat16", [FLAT], fp16, kind="ExternalInput").ap()
    if not trivial_ln:
        gam = nc.dram_tensor("gamma", [1, E], fp32, kind="ExternalInput").ap()
        bet = nc.dram_tensor("beta", [1, E], fp32, kind="ExternalInput").ap()
    # fp16 output (host casts to fp32): LN output is ~N(0,1), fp16
    # rounding is ~5e-4 relative — halves the out-DMA tail.
    out = nc.dram_tensor("out", [S, E], fp16, kind="ExternalOutput").ap()

    with tile.TileContext(nc) as tc, ExitStack() as ctx:
        persist = ctx.enter_context(tc.tile_pool(name="persist", bufs=1))
        QT = persist.tile([P, G, 2, S], fp8, name="QT")
        KT = persist.tile([P, G, 2, S], fp8, name="KT")
        VE = persist.tile([P, NBLK, H * DE], fp16, name="VE")
        VE8 = persist.tile([P, NBLK, H * DE], fp8, name="VE8")
        natSB = persist.tile([P, NBLK, E], fp16, name="natSB")
        srecSB = persist.tile([P, G, 2, NBLK], fp32, name="srecSB")
        epsT = persist.tile([P, 1], fp32, name="epsT")

        nc.vector.memset(epsT, EPS)
        nc.vector.memset(
            bass.AP(tensor=VE.tensor, offset=VE.offset + D,
                    ap=[VE.ap[0], [H * DE, NBLK], [DE, H]]),
            1.0)
        nc.vector.memset(
            bass.AP(tensor=VE8.tensor, offset=VE8.offset + D,
                    ap=[VE8.ap[0], [H * DE, NBLK], [DE, H]]),
            32.0)

        if not trivial_ln:
            gamT = persist.tile([P, E], fp32, name="gamT")
            betT = persist.tile([P, E], fp32, name="betT")
            nc.sync.dma_start(
                out=gamT,
                in_=bass.AP(tensor=gam.tensor, offset=0, ap=[[0, P], [1, E]]),
            )
            nc.sync.dma_start(
                out=betT,
                in_=bass.AP(tensor=bet.tensor, offset=0, ap=[[0, P], [1, E]]),
            )

        expp = ctx.enter_context(tc.tile_pool(name="expp", bufs=4))
        # T2 bias table, split in two sliding windows so each can load as
        # early as SBUF frees: T2b (the high-offset window, 27.7KB) fits
        # beside the stage-1 inputs and loads right after them; T2a
        # (39.7KB) reuses the Q/K-input region that dies after iter 1.
        # Group (hh, F) with base = 15360-1024*hh+2048*F reads
        # T2a[base + 128J + p + 16w] if base <= 17408 (view max 20463),
        # else T2b at offset base-18432 (flat index 18432 + ...).
        T2AW = 20352
        T2BO = 18432
        T2BW = FLAT - T2BO - 127   # 14193
        t2bp = ctx.enter_context(tc.tile_pool(name="t2bp", bufs=1))
        T2b = t2bp.tile([P, T2BW], fp16, name="T2b")
        psQK = ctx.enter_context(
            tc.tile_pool(name="psQK", bufs=2, space="PSUM"))
        pvp = ctx.enter_context(
            tc.tile_pool(name="pvp", bufs=1, space="PSUM"))

        eP = {}
        pools = {}

        # ---- emission helpers: each returns a list of closures ("chunks");
        # E-units (one QK J-step + exp) are interleaved between chunks.
        def proj_chunks(g, w8get, dst, rhs8get):
            # fp8 DoubleRow: contraction 1024 as 4 steps of 2x128.
            # Per-ic [P,512] psum tiles (bufs=2) let the DVE copy of ic0
            # overlap the matmuls of ic1 / the next chain.
            # dst is [P, G, 2, S] fp8; the r=1 slice is a DMA duplicate of
            # r=0 so QK can run as a DoubleRow matmul (contracting the 64
            # head dims twice; exp scale absorbs the factor 2).
            def go():
                w8, rhs8 = w8get(), rhs8get()
                for ic in range(2):
                    pt = pools["psProj"].tile([P, 512], fp32, tag="proj",
                                              name="pt")
                    for kp in range(4):
                        nc.tensor.matmul(
                            pt,
                            w8[:, 2 * kp:2 * kp + 2, g * P:(g + 1) * P],
                            rhs8[:, 2 * kp:2 * kp + 2,
                                 ic * 512:(ic + 1) * 512],
                            start=(kp == 0), stop=(kp == 3),
                            perf_mode=DRow, skip_group_check=True,
                        )
                    nc.vector.tensor_copy(
                        dst[:, g, 0, ic * 512:(ic + 1) * 512], pt)
                    # duplicate r=0 -> r=1 (GPSIMD cannot read PSUM on hw).
                    # g0/g1 feed the first exps: keep their dup on DVE to
                    # avoid the cross-engine hop on the warmup critical path.
                    eng = nc.vector if g < 2 else nc.gpsimd
                    eng.tensor_copy(
                        dst[:, g, 1, ic * 512:(ic + 1) * 512],
                        dst[:, g, 0, ic * 512:(ic + 1) * 512])
            return [go]

        def v8proj_chunks(jb):
            # fp8 DR V projection feeding VE8 (PV path) only
            def mk(ic):
                def go():
                    bt = pvp.tile([P, NBLK, P], fp32, tag="pv", name="pv")
                    pt = bass.AP(tensor=bt.tensor, offset=bt.offset,
                                 ap=[bt.ap[0], [1, 512]])
                    for kp in range(4):
                        nc.tensor.matmul(
                            pt,
                            x8T[:, 2 * kp:2 * kp + 2, jb * P:(jb + 1) * P],
                            wv8_sb[:, 2 * kp:2 * kp + 2,
                                   ic * 512:(ic + 1) * 512],
                            start=(kp == 0), stop=(kp == 3),
                            perf_mode=DRow, skip_group_check=True,
                        )
                    dstv8 = bass.AP(
                        tensor=VE8.tensor,
                        offset=VE8.offset + jb * (H * DE) + ic * 8 * DE,
                        ap=[VE8.ap[0], [DE, 8], [1, D]],
                    )
                    nc.vector.tensor_copy(dstv8, pt)
                return go
            return [mk(0), mk(1)]

        def vproj_chunks(jb):
            # V16 runs on the psProj ring (idle after iter 1), decoupled
            # from the PV ring so all 16 chunks can finish by ~iter 3 and
            # unblock the bias matmuls (which read all of VE).
            def mk(ic):
                def go():
                    pt = pools["psProj"].tile([P, 512], fp32, tag="proj",
                                              name="pt")
                    for kb in range(KBLK):
                        nc.tensor.matmul(
                            pt,
                            xT[:, kb, jb * P:(jb + 1) * P],
                            wv_sb[:, kb, ic * 512:(ic + 1) * 512],
                            start=(kb == 0), stop=(kb == KBLK - 1),
                            skip_group_check=True,
                        )
                    dstv = bass.AP(
                        tensor=VE.tensor,
                        offset=VE.offset + jb * (H * DE) + ic * 8 * DE,
                        ap=[VE.ap[0], [DE, 8], [1, D]],
                    )
                    nc.vector.tensor_copy(dstv, pt)
                return go
            return [mk(0), mk(1)]

        def pv_chunks(g, half):
            u = 2 * g + half
            hh = u
            state = {}

            def mk(fh):
                def go(st):
                    if fh == 0:
                        st["pv"] = pvp.tile([P, NBLK, P], fp32, tag="pv", name="pv")
                    pv = st["pv"]
                    for F in range(4 * fh, 4 * fh + 4):
                        for Jp in range(4):
                            nc.tensor.matmul(
                                pv[:, F, 0:DE],
                                eP[u][:, 2 * Jp:2 * Jp + 2,
                                      F * P:(F + 1) * P],
                                VE8[:, 2 * Jp:2 * Jp + 2,
                                    hh * DE:(hh + 1) * DE],
                                start=(Jp == 0), stop=(Jp == 3),
                                perf_mode=DRow, skip_group_check=True,
                            )
                    if fh == 1:
                        del eP[u]
                        srec = srecSB[:, g, half, :]
                        nc.vector.reciprocal(
                            srec,
                            bass.AP(tensor=pv.tensor, offset=pv.offset + D,
                                    ap=[pv.ap[0], [P, NBLK]]))
                        natv = bass.AP(
                            tensor=natSB.tensor,
                            offset=natSB.offset + hh * D,
                            ap=[natSB.ap[0], [E, NBLK], [1, D]],
                        )
                        pvv = bass.AP(tensor=pv.tensor, offset=pv.offset,
                                      ap=[pv.ap[0], [P, NBLK], [1, D]])
                        srecb = bass.AP(
                            tensor=srecSB.tensor,
                            offset=srecSB.offset + u * NBLK,
                            ap=[srecSB.ap[0], [1, NBLK], [0, D]],
                        )
                        if u < 6:
                            # early heads: attn@V lands first, the bias
                            # unit later ADDS into natSB.
                            nc.vector.tensor_tensor(natv, pvv, srecb, mult)
                        else:
                            # late heads: the bias COPY (created earlier,
                            # eligible early) already filled natSB; add
                            # the normalized attention on top, then stats.
                            tmp = pools["tmpp"].tile([P, NBLK * D], fp32,
                                                     tag="tmp", name="tmp")
                            nc.vector.tensor_tensor(tmp, pvv, srecb, mult)
                            nc.vector.tensor_tensor(natv, natv, tmp, add)
                            for F in range(NBLK):
                                nc.vector.bn_stats(
                                    statsA[:, F, u, :],
                                    natSB[:, F, u * D:(u + 1) * D])
                return go
            return [lambda f=mk(0): f(state), lambda f=mk(1): f(state)]

        def e_units(g):
            units = []
            for half in range(2):
                for J in range(NBLK):
                    def go(half=half, J=J):
                        u = 2 * g + half
                        if J == 0:
                            eP[u] = expp.tile([P, NBLK, S], fp8, tag="ept", name="eP")
                        lo = D * half
                        pa = psQK.tile([P, E], fp32, tag="qk", name="pa")
                        for ic in range(2):
                            nc.tensor.matmul(
                                pa[:, ic * 512:(ic + 1) * 512],
                                KT[lo:lo + D, g, :, J * P:(J + 1) * P],
                                QT[lo:lo + D, g, :,
                                   ic * 512:(ic + 1) * 512],
                                start=True, stop=True,
                                perf_mode=DRow, skip_group_check=True,
                            )
                        nc.scalar.activation(
                            out=eP[u][:, J, :], in_=pa, func=Exp,
                            scale=SCALE / 2048.0)
                    units.append(go)
            return units

        def emit_interleaved(chunks, units, gates=None):
            # spread E-units evenly between chunks; unit k may only be
            # emitted once gates[k] chunks are done (WAR: the eP slot it
            # allocates must have its reader PV already emitted).
            nc_, nu = len(chunks), len(units)
            if gates is None:
                gates = [0] * nu
            ui = 0
            for ci, ch in enumerate(chunks):
                ch()
                done = ci + 1
                want = done * nu // nc_
                while ui < want and ui < nu and gates[ui] <= done:
                    units[ui]()
                    ui += 1
            while ui < nu:
                units[ui]()
                ui += 1

        # ---- fused stage 1+2 ----
        with tc.tile_pool(name="psProj", bufs=2, space="PSUM") as psProj, \
             tc.tile_pool(name="s1fix", bufs=1) as s1fix:
            pools["psProj"] = psProj
            xT = s1fix.tile([P, KBLK, S], fp16, name="xT")
            x8T = s1fix.tile([P, KBLK, S], fp8, name="x8T")
            x8Tp = s1fix.tile([P, KBLK, S], fp8, name="x8Tp")
            w8q = s1fix.tile([P, KBLK, E], fp8, name="w8q")
            w8k = s1fix.tile([P, KBLK, E], fp8, name="w8k")
            wv_sb = s1fix.tile([P, KBLK, E], fp16, name="wv_sb")
            wv8_sb = s1fix.tile([P, KBLK, E], fp8, name="wv8_sb")
            # Input DMAs serialize on the DMA-engine device, so arrival
            # order = creation order. Load per-g column blocks of Wq/Wk so
            # the g0 QK chain (and the first exp) is gated by ~7us of DMA
            # instead of ~12us.
            wqr = wq8.rearrange("(kb kp) e -> kp kb e", kp=P)
            wkr = wk8.rearrange("(kb kp) e -> kp kb e", kp=P)
            nc.sync.dma_start(
                out=x8Tp, in_=xtp8.rearrange("(kb kp) s -> kp kb s", kp=P))
            nc.sync.dma_start(out=w8q[:, :, 0:P], in_=wqr[:, :, 0:P])
            nc.sync.dma_start(
                out=x8T, in_=xt8.rearrange("(kb kp) s -> kp kb s", kp=P))
            nc.sync.dma_start(out=w8k[:, :, 0:P], in_=wkr[:, :, 0:P])
            for g in (1,):
                nc.sync.dma_start(out=w8q[:, :, g * P:(g + 1) * P],
                                  in_=wqr[:, :, g * P:(g + 1) * P])
                nc.sync.dma_start(out=w8k[:, :, g * P:(g + 1) * P],
                                  in_=wkr[:, :, g * P:(g + 1) * P])
            nc.sync.dma_start(
                out=wv8_sb, in_=wv8.rearrange("(kb kp) e -> kp kb e", kp=P))
            for g in range(2, G):
                nc.sync.dma_start(out=w8q[:, :, g * P:(g + 1) * P],
                                  in_=wqr[:, :, g * P:(g + 1) * P])
                nc.sync.dma_start(out=w8k[:, :, g * P:(g + 1) * P],
                                  in_=wkr[:, :, g * P:(g + 1) * P])
            nc.sync.dma_start(
                out=xT, in_=xt16.rearrange("(kb kp) s -> kp kb s", kp=P))
            nc.sync.dma_start(
                out=wv_sb,
                in_=wv16.rearrange("(kb kp) e -> kp kb e", kp=P),
            )
            # T2b has no region conflicts: loads right after the inputs
            nc.sync.dma_start(
                out=T2b,
                in_=bass.AP(tensor=flat16.tensor, offset=T2BO,
                            ap=[[1, P], [1, T2BW]]),
            )

            # Warmup feeds Act immediately: Q/K(0,1) projections first,
            # then E(0) units interleaved with the V chains; remaining
            # Q/K projections ride iter 1 alongside E(1). fp8 DR makes
            # projections cheap enough that the s1fix region (and the T2
            # fill) frees by ~60us into the run.
            for g in (0, 1):
                for c in proj_chunks(g, lambda: w8q, QT, lambda: x8Tp):
                    c()
                for c in proj_chunks(g, lambda: w8k, KT, lambda: x8T):
                    c()
            chunks = []
            for jb in range(4):
                chunks += v8proj_chunks(jb)
            emit_interleaved(chunks, e_units(0))
            chunks = []
            for g in range(2, G):
                chunks += proj_chunks(g, lambda: w8q, QT, lambda: x8Tp)
                chunks += proj_chunks(g, lambda: w8k, KT, lambda: x8T)
            for jb in range(4, 8):
                chunks += v8proj_chunks(jb)
            emit_interleaved(chunks, e_units(1))
            v16 = {2: [0, 1, 2, 3], 3: [4, 5, 6, 7], 4: []}
            for i in range(2, 5):
                chunks = pv_chunks(i - 2, 0) + pv_chunks(i - 2, 1)
                for jb in v16[i]:
                    chunks += vproj_chunks(jb)
                emit_interleaved(chunks, e_units(i),
                                 [2] * 8 + [4] * 8)

        # ---- tail of stage 2 + stage 3 (T2 reuses the s1fix region) ----
        with tc.tile_pool(name="t2p", bufs=1) as t2p, \
             tc.tile_pool(name="lns", bufs=8) as lns, \
             tc.tile_pool(name="lnof", bufs=8) as lnof, \
             tc.tile_pool(name="bps", bufs=2, space="PSUM") as bps, \
             tc.tile_pool(name="tmpp", bufs=3) as tmpp:
            pools["tmpp"] = tmpp
            statsA = t2p.tile([P, NBLK, H, 6], fp32, name="statsA")
            # T2a (covers groups with base <= 17408) reuses the freed
            # s1fix region; 3-slice fill starts as soon as vproj's last
            # read of xT/wv_sb retires (~iter 3 with vproj on psProj).
            T2a = t2p.tile([P, T2AW], fp16, name="T2a")
            for a, b in ((0, 6784), (6784, 13568), (13568, T2AW)):
                nc.sync.dma_start(
                    out=T2a[:, a:b],
                    in_=bass.AP(tensor=flat16.tensor, offset=a,
                                ap=[[1, P], [1, b - a]]),
                )

            def bias_units(hh):
                # bias@V for head hh over all 8 F blocks, accumulated in a
                # [P, 512] psum tile (64-col slice per F). Late heads
                # (>=6, created before their PV) COPY into natSB — the PV
                # then adds normalized attention on top; early heads (<6,
                # created last) ADD into natSB behind their natv + stats.
                def go():
                    bt = bps.tile([P, NBLK * D], fp32, tag="bias",
                                  name="bias")
                    for F in range(NBLK):
                        base = 15360 - 1024 * hh + 2048 * F
                        if base <= 17408:
                            tsr, off = T2a, base
                        else:
                            tsr, off = T2b, base - T2BO
                        for J in range(NBLK):
                            t2st = bass.AP(
                                tensor=tsr.tensor,
                                offset=tsr.offset + off + P * J,
                                ap=[tsr.ap[0], [16, P]],
                            )
                            nc.tensor.matmul(
                                bt[:, F * D:(F + 1) * D], t2st,
                                VE[:, J, hh * DE:hh * DE + D],
                                start=(J == 0), stop=(J == NBLK - 1),
                                skip_group_check=True,
                            )
                    nat = bass.AP(
                        tensor=natSB.tensor, offset=natSB.offset + hh * D,
                        ap=[natSB.ap[0], [E, NBLK], [1, D]],
                    )
                    if hh >= 6:
                        nc.vector.tensor_copy(nat, bt)
                    else:
                        nc.vector.tensor_tensor(nat, nat, bt, add)
                        for F in range(NBLK):
                            nc.vector.bn_stats(
                                statsA[:, F, hh, :],
                                natSB[:, F, hh * D:(hh + 1) * D])
                return go

            # iters 5..7 with bias units spread as chunks (PE order is
            # ~creation order, so each iter carries only what its Act
            # window affords). Copy-scheme heads (>=6) are created just
            # before the iter holding their PV; add-scheme heads (0-5)
            # ride along (their natv landed in iters 2-4).
            # The two copy-scheme units for THIS iter's PV heads must be
            # created BEFORE the pv chunks (the PV add reads natSB on top
            # of the bias copy); the eP-slot gates shift by 2 accordingly.
            biassched = {5: ([6, 7], [12, 0, 1]), 6: ([8, 9], [13, 2, 3]),
                         7: ([10, 11], [14, 15, 4, 5])}
            for i in range(5, G):
                pre, post = biassched[i]
                chunks = ([bias_units(pre[0])] + pv_chunks(i - 2, 0)
                          + [bias_units(pre[1])] + pv_chunks(i - 2, 1)
                          + [bias_units(hh) for hh in post])
                emit_interleaved(chunks, e_units(i), [3] * 8 + [6] * 8)
            for c in pv_chunks(G - 2, 0) + pv_chunks(G - 2, 1):
                c()
            for c in pv_chunks(G - 1, 0) + pv_chunks(G - 1, 1):
                c()

            # LN front: aggr/sqrt/recip/murs per F
            mvs, rstds, murss = [], [], []
            for F in range(NBLK):
                mv = lns.tile([P, 2], fp32, tag="mv", name="mv")
                nc.vector.bn_aggr(mv, statsA[:, F, :, :])
                rstd = lns.tile([P, 1], fp32, tag="rstd", name="rstd")
                nc.scalar.activation(out=rstd, in_=mv[:, 1:2],
                                     func=Sqrt, bias=epsT, scale=1.0)
                mvs.append(mv)
                rstds.append(rstd)
            for F in range(NBLK):
                murs = lns.tile([P, 1], fp32, tag="murs", name="murs")
                nc.vector.reciprocal(rstds[F], rstds[F])
                if F % 2 == 0:
                    nc.vector.tensor_tensor(murs, mvs[F][:, 0:1],
                                            rstds[F], mult)
                else:
                    nc.vector.tensor_scalar(murs, mvs[F][:, 0:1],
                                            rstds[F], -1.0,
                                            op0=mult, op1=mult)
                murss.append(murs)

            # LN pass 2: applies alternate Pool / Act with out-DMA per F.
            for F in range(NBLK):
                of = lnof.tile([P, E], fp16, tag="of", name="of")
                if F % 2 == 0:
                    nc.gpsimd.tensor_scalar(of, natSB[:, F, :], rstds[F],
                                            murss[F], op0=mult, op1=sub)
                else:
                    nc.scalar.activation(out=of, in_=natSB[:, F, :],
                                         func=Identity, bias=murss[F],
                                         scale=rstds[F])
                if not trivial_ln:
                    nc.vector.tensor_tensor(of, of, gamT, mult)
                    nc.vector.tensor_tensor(of, of, betT, add)
                nc.sync.dma_start(out[F * P:(F + 1) * P, :], of)

    nc.compile()
    return nc


def get_nc(trivial_ln: bool = True):
    if trivial_ln not in _BUILT:
        _BUILT[trivial_ln] = _build(trivial_ln)
    return _BUILT[trivial_ln]


def make_in_maps(inputs):
    x = np.asarray(inputs["x"])
    rel = np.asarray(inputs["rel_table"])
    gamma = np.asarray(inputs["gamma"])
    beta = np.asarray(inputs["beta"])
    trivial_ln = bool(np.all(gamma == 1.0) and np.all(beta == 0.0))

    import ml_dtypes
    f8 = ml_dtypes.float8_e4m3fn
    x16 = x.astype(np.float16)
    xt16 = np.ascontiguousarray(x16.transpose(0, 2, 1))          # (B, E, S)
    xt8 = np.ascontiguousarray(x.transpose(0, 2, 1).astype(f8))
    xtp8 = np.ascontiguousarray(x[:, SIGMA, :].transpose(0, 2, 1).astype(f8))
    # q/k weights pre-scaled by 32 (fp8 sweet spot); exp scale absorbs 1/1024
    wq8 = (np.asarray(inputs["Wq"]) * 32.0).astype(f8)
    wk8 = (np.asarray(inputs["Wk"]) * 32.0).astype(f8)
    wv16 = np.asarray(inputs["Wv"]).astype(np.float16)
    wv8 = (np.asarray(inputs["Wv"]) * 32.0).astype(f8)
    flat16 = np.ascontiguousarray(rel.reshape(-1).astype(np.float16))

    in_maps = []
    for b in range(x.shape[0]):
        m = {"xt16": xt16[b], "xt8": xt8[b], "xtp8": xtp8[b],
             "wq8": wq8, "wk8": wk8, "wv16": wv16, "wv8": wv8,
             "flat16": flat16}
        if not trivial_ln:
            m["gamma"] = gamma.reshape(1, E).astype(np.float32)
            m["beta"] = beta.reshape(1, E).astype(np.float32)
        in_maps.append(m)
    return in_maps, trivial_ln


def unpermute(raw):
    """raw: (..., S, E) rows in processing order -> natural order."""
    fixed = np.empty_like(raw)
    fixed[..., SIGMA, :] = raw
    return fixed


def kernel(**inputs) -> np.ndarray:
    from concourse import bass_utils

    in_maps, trivial_ln = make_in_maps(inputs)
    nc = get_nc(trivial_ln)
    res = bass_utils.run_bass_kernel_spmd(nc, in_maps,
                                          core_ids=list(range(len(in_maps))))
    outs = np.stack([r["out"] for r in res.results])
    return unpermute(outs).astype(np.float32)



# revision 49
# speedup vs baseline: 1.2193x; 1.0061x over previous
"""Trainium2 Bass kernel for nn_Attention_Rel_Scl (B=8,S=1024,E=1024,H=16).

Data-parallel over batch: one batch element per NeuronCore (8 cores).

v8 (164976ns model, from v6's 198352): Act/exp (133us busy) is the
hard floor — TimelineSim charges matmuls out_free x 0.4167ns x cpr
(fp8e4 DoubleRow cpr=0.5, contraction length free) and activations
free_size x 0.833ns; psum (16KB) caps exp tiles at [128,1024].
  (a) QK^T as fp8 DoubleRow: QT/KT fp8 [P, G, 2, S], r=1 a copy of r=0
  (DVE for g0/g1 warmup, Pool after; Pool cannot read PSUM so the dup
  chains off the SBUF r=0 slice). The DR matmul contracts the 64 head
  dims twice; exp scale absorbs the 2x. PE 139->112us.
  (b) bias@V inside the exp window: per-head [P,512] psum accumulation.
  T2 split in two sliding windows (T2b 27.7KB loads at t~16us beside
  the stage-1 inputs, T2a 39.7KB reuses s1fix right after vproj), and
  vproj runs on the psProj ring so VE completes ~iter 3. Heads 6-15:
  bias COPIES into natSB (fp16) before their PV, whose add rides on
  top (tmp = pv*srec; natSB += tmp) — drains psum without waiting
  natv; heads 0-5 add after their early natv. Scheduler note: PE static
  order ~= creation order, so bias units are spread per-iter as chunks
  sized to each iter's Act budget (a block created too early starves
  Act for its full duration).
  (c) tail: per-(F,head) bn_stats ride each bias-add; LN front fused
  into head 15's PV; applies alternate Pool / Act(Identity, bias=-mu*
  rstd); fp16 'of'/output (host casts) halves the out-DMA; wide lnof
  ring avoids WAR ping-pong. Tail ~15us after the last exp.
  First exp ~13us (w8q/w8k loaded per-g block; DMA device serializes).

v6: v5 + fp8e4 DoubleRow matmuls for Q/K projections and PV.
  - exp(QK^T/sqrt(E)) is the *stationary* operand of PV / colsum / biasV
    matmuls, so those cost only (out free size) PE cycles and the result
    lands directly in natural [row, feature] orientation (no transposes,
    no gathers). V carries an interleaved 1.0 column per head so PV and
    the softmax denominator come from one moving stream.
  - Emission interleaves 2 QK+exp J-steps between every ~2us PE chunk
    (projection half-chains, PV half-blocks): the in-order engines then
    pace each other without head-of-line stalls; Act (the 133us exp
    budget) starts ~15us in and stays ~full.
  - QT/KT/VE psum->SBUF copies run on GpSimd (Pool) so the DVE's
    reciprocal (which waits on PV groups) never blocks them.
  - T2 (bias table, 63.7KB/part) is DMA-filled into the region freed by
    the projection inputs, overlapping the back half of stage 2.
  - bias[h,i,j] = flat[(16368-1024h) + 1024*(i%16) - 16*(i//16) + j]
    (flat = rel_table.reshape(-1), clip never fires); rows processed in
    order f -> SIGMA[f] = 16*(63-f%64) + f//64 make the bias block for
    (hh, F, J) the T2 view at offset 15360-1024*hh+2048*F+128*J with
    ap [[1,128],[1024,2],[16,64]], T2[p,w] = flat[p+w].
  - LayerNorm in natural layout; combine-add + normalize-apply on Pool,
    bn_stats/aggr/recip on DVE, Sqrt on Act. Contiguous output DMA; host
    un-permutes rows (SIGMA).
"""

import sys

if "/opt/trn_rl_repo" not in sys.path:
    sys.path.insert(0, "/opt/trn_rl_repo")

import numpy as np

B, S, E, H = 8, 1024, 1024, 16
D = E // H          # 64 head dim
P = 128             # partitions
G = H // 2          # 8 head pairs
NBLK = S // P       # 8 key/query blocks
KBLK = E // P       # 8 contraction blocks
EPS = 1e-3
SCALE = float(E) ** -0.5
FLAT = (2 * S - 1) * H   # 32752
T2W = 32625              # max free offset 32624 (+p<=127 -> 32751 = FLAT-1)
DE = D + 1               # 65: V column block plus ones column

_f = np.arange(S)
SIGMA = 16 * (63 - _f % 64) + _f // 64

_BUILT = {}


def _build(trivial_ln: bool):
    import concourse.bass as bass
    import concourse.tile as tile
    from concourse import bacc, mybir
    from contextlib import ExitStack

    fp16 = mybir.dt.float16
    fp32 = mybir.dt.float32
    Exp = mybir.ActivationFunctionType.Exp
    Sqrt = mybir.ActivationFunctionType.Sqrt
    Identity = mybir.ActivationFunctionType.Identity
    mult = mybir.AluOpType.mult
    add = mybir.AluOpType.add
    sub = mybir.AluOpType.subtract

    nc = bacc.Bacc("TRN2", target_bir_lowering=False, debug=False,
                   num_devices=8)

    fp8 = mybir.dt.float8e4
    DRow = mybir.MatmulPerfMode.DoubleRow
    xt16 = nc.dram_tensor("xt16", [E, S], fp16, kind="ExternalInput").ap()
    xt8 = nc.dram_tensor("xt8", [E, S], fp8, kind="ExternalInput").ap()
    xtp8 = nc.dram_tensor("xtp8", [E, S], fp8, kind="ExternalInput").ap()
    wq8 = nc.dram_tensor("wq8", [E, E], fp8, kind="ExternalInput").ap()
    wk8 = nc.dram_tensor("wk8", [E, E], fp8, kind="ExternalInput").ap()
    wv16 = nc.dram_tensor("wv16", [E, E], fp16, kind="ExternalInput").ap()
    wv8 = nc.dram_tensor("wv8", [E, E], fp8, kind="ExternalInput").ap()
    flat16 = nc.dram_tensor("flat16", [FLAT], fp16, kind="ExternalInput").ap()
    if not trivial_ln:
        gam = nc.dram_tensor("gamma", [1, E], fp32, kind="ExternalInput").ap()
        bet = nc.dram_tensor("beta", [1, E], fp32, kind="ExternalInput").ap()
    # fp16 output (host casts to fp32): LN output is ~N(0,1), fp16
    # rounding is ~5e-4 relative — halves the out-DMA tail.
    out = nc.dram_tensor("out", [S, E], fp16, kind="ExternalOutput").ap()

    with tile.TileContext(nc) as tc, ExitStack() as ctx:
        persist = ctx.enter_context(tc.tile_pool(name="persist", bufs=1))
        QT = persist.tile([P, G, S], fp8, name="QT")
        KT = persist.tile([P, G, S], fp8, name="KT")
        VE = persist.tile([P, NBLK, H * DE], fp16, name="VE")
        VE8 = persist.tile([P, NBLK, H * DE], fp8, name="VE8")
        natSB = persist.tile([P, NBLK, E], fp16, name="natSB")
        srecSB = persist.tile([P, G, 2, NBLK], fp32, name="srecSB")
        epsT = persist.tile([P, 1], fp32, name="epsT")

        nc.vector.memset(epsT, EPS)
        nc.vector.memset(
            bass.AP(tensor=VE.tensor, offset=VE.offset + D,
                    ap=[VE.ap[0], [H * DE, NBLK], [DE, H]]),
            1.0)
        nc.vector.memset(
            bass.AP(tensor=VE8.tensor, offset=VE8.offset + D,
                    ap=[VE8.ap[0], [H * DE, NBLK], [DE, H]]),
            32.0)

        if not trivial_ln:
            gamT = persist.tile([P, E], fp32, name="gamT")
            betT = persist.tile([P, E], fp32, name="betT")
            nc.sync.dma_start(
                out=gamT,
                in_=bass.AP(tensor=gam.tensor, offset=0, ap=[[0, P], [1, E]]),
            )
            nc.sync.dma_start(
                out=betT,
                in_=bass.AP(tensor=bet.tensor, offset=0, ap=[[0, P], [1, E]]),
            )

        expp = ctx.enter_context(tc.tile_pool(name="expp", bufs=5))
        # T2 bias table, split in two sliding windows so each can load as
        # early as SBUF frees: T2b (the high-offset window, 27.7KB) fits
        # beside the stage-1 inputs and loads right after them; T2a
        # (39.7KB) reuses the Q/K-input region that dies after iter 1.
        # Group (hh, F) with base = 15360-1024*hh+2048*F reads
        # T2a[base + 128J + p + 16w] if base <= 17408 (view max 20463),
        # else T2b at offset base-18432 (flat index 18432 + ...).
        T2AW = 20352
        T2BO = 18432
        T2BW = FLAT - T2BO - 127   # 14193
        t2bp = ctx.enter_context(tc.tile_pool(name="t2bp", bufs=1))
        T2b = t2bp.tile([P, T2BW], fp16, name="T2b")
        psQK = ctx.enter_context(
            tc.tile_pool(name="psQK", bufs=2, space="PSUM"))
        pvp = ctx.enter_context(
            tc.tile_pool(name="pvp", bufs=1, space="PSUM"))

        eP = {}
        pools = {}

        # ---- emission helpers: each returns a list of closures ("chunks");
        # E-units (one QK J-step + exp) are interleaved between chunks.
        def proj_chunks(g, w8get, dst, rhs8get):
            # fp8 DoubleRow: contraction 1024 as 4 steps of 2x128.
            # Per-ic [P,512] psum tiles (bufs=2) let the DVE copy of ic0
            # overlap the matmuls of ic1 / the next chain.
            # dst is [P, G, 2, S] fp8; the r=1 slice is a DMA duplicate of
            # r=0 so QK can run as a DoubleRow matmul (contracting the 64
            # head dims twice; exp scale absorbs the factor 2).
            def go():
                w8, rhs8 = w8get(), rhs8get()
                for ic in range(2):
                    pt = pools["psProj"].tile([P, 512], fp32, tag="proj",
                                              name="pt")
                    for kp in range(4):
                        nc.tensor.matmul(
                            pt,
                            w8[:, 2 * kp:2 * kp + 2, g * P:(g + 1) * P],
                            rhs8[:, 2 * kp:2 * kp + 2,
                                 ic * 512:(ic + 1) * 512],
                            start=(kp == 0), stop=(kp == 3),
                            perf_mode=DRow, skip_group_check=True,
                        )
                    nc.vector.tensor_copy(
                        dst[:, g, ic * 512:(ic + 1) * 512], pt)
            return [go]

        def v8proj_chunks(jb):
            # fp8 DR V projection feeding VE8 (PV path) only
            def mk(ic):
                def go():
                    bt = pvp.tile([P, NBLK, P], fp32, tag="pv", name="pv")
                    pt = bass.AP(tensor=bt.tensor, offset=bt.offset,
                                 ap=[bt.ap[0], [1, 512]])
                    for kp in range(4):
                        nc.tensor.matmul(
                            pt,
                            x8T[:, 2 * kp:2 * kp + 2, jb * P:(jb + 1) * P],
                            wv8_sb[:, 2 * kp:2 * kp + 2,
                                   ic * 512:(ic + 1) * 512],
                            start=(kp == 0), stop=(kp == 3),
                            perf_mode=DRow, skip_group_check=True,
                        )
                    dstv8 = bass.AP(
                        tensor=VE8.tensor,
                        offset=VE8.offset + jb * (H * DE) + ic * 8 * DE,
                        ap=[VE8.ap[0], [DE, 8], [1, D]],
                    )
                    nc.vector.tensor_copy(dstv8, pt)
                return go
            return [mk(0), mk(1)]

        def vproj_chunks(jb):
            # V16 runs on the psProj ring (idle after iter 1), decoupled
            # from the PV ring so all 16 chunks can finish by ~iter 3 and
            # unblock the bias matmuls (which read all of VE).
            def mk(ic):
                def go():
                    pt = pools["psProj"].tile([P, 512], fp32, tag="proj",
                                              name="pt")
                    for kb in range(KBLK):
                        nc.tensor.matmul(
                            pt,
                            xT[:, kb, jb * P:(jb + 1) * P],
                            wv_sb[:, kb, ic * 512:(ic + 1) * 512],
                            start=(kb == 0), stop=(kb == KBLK - 1),
                            skip_group_check=True,
                        )
                    dstv = bass.AP(
                        tensor=VE.tensor,
                        offset=VE.offset + jb * (H * DE) + ic * 8 * DE,
                        ap=[VE.ap[0], [DE, 8], [1, D]],
                    )
                    nc.vector.tensor_copy(dstv, pt)
                return go
            return [mk(0), mk(1)]

        def pv_chunks(g, half):
            u = 2 * g + half
            hh = u
            state = {}

            def mk(fh):
                def go(st):
                    if fh == 0:
                        st["pv"] = pvp.tile([P, NBLK, P], fp32, tag="pv", name="pv")
                    pv = st["pv"]
                    for F in range(4 * fh, 4 * fh + 4):
                        for Jp in range(4):
                            nc.tensor.matmul(
                                pv[:, F, 0:DE],
                                eP[u][:, 2 * Jp:2 * Jp + 2,
                                      F * P:(F + 1) * P],
                                VE8[:, 2 * Jp:2 * Jp + 2,
                                    hh * DE:(hh + 1) * DE],
                                start=(Jp == 0), stop=(Jp == 3),
                                perf_mode=DRow, skip_group_check=True,
                            )
                    if fh == 1:
                        del eP[u]
                        srec = srecSB[:, g, half, :]
                        nc.vector.reciprocal(
                            srec,
                            bass.AP(tensor=pv.tensor, offset=pv.offset + D,
                                    ap=[pv.ap[0], [P, NBLK]]))
                        natv = bass.AP(
                            tensor=natSB.tensor,
                            offset=natSB.offset + hh * D,
                            ap=[natSB.ap[0], [E, NBLK], [1, D]],
                        )
                        pvv = bass.AP(tensor=pv.tensor, offset=pv.offset,
                                      ap=[pv.ap[0], [P, NBLK], [1, D]])
                        srecb = bass.AP(
                            tensor=srecSB.tensor,
                            offset=srecSB.offset + u * NBLK,
                            ap=[srecSB.ap[0], [1, NBLK], [0, D]],
                        )
                        if u < 6:
                            # early heads: attn@V lands first, the bias
                            # unit later ADDS into natSB.
                            nc.vector.tensor_tensor(natv, pvv, srecb, mult)
                        else:
                            # late heads: the bias COPY (created earlier,
                            # eligible early) already filled natSB; add
                            # the normalized attention on top, then stats.
                            tmp = pools["tmpp"].tile([P, NBLK * D], fp32,
                                                     tag="tmp", name="tmp")
                            nc.vector.tensor_tensor(tmp, pvv, srecb, mult)
                            nc.vector.tensor_tensor(natv, natv, tmp, add)
                            for F in range(NBLK):
                                nc.vector.bn_stats(
                                    statsA[:, F, u, :],
                                    natSB[:, F, u * D:(u + 1) * D])
                return go
            return [lambda f=mk(0): f(state), lambda f=mk(1): f(state)]

        def e_units(g):
            units = []
            for half in range(2):
                for J in range(NBLK):
                    def go(half=half, J=J):
                        u = 2 * g + half
                        if J == 0:
                            eP[u] = expp.tile([P, NBLK, S], fp8, tag="ept", name="eP")
                        lo = D * half
                        pa = psQK.tile([P, E], fp32, tag="qk", name="pa")
                        # DoubleRow pair dim as a stride-0 AP dim: both
                        # k-tiles read the SAME 64 head dims (exp scale
                        # absorbs the factor 2) — no duplicate slice.
                        kv = KT[lo:lo + D, g, J * P:(J + 1) * P]
                        kst = bass.AP(tensor=kv.tensor, offset=kv.offset,
                                      ap=[kv.ap[0], [0, 2]] + list(kv.ap[1:]))
                        for ic in range(2):
                            qv = QT[lo:lo + D, g,
                                    ic * 512:(ic + 1) * 512]
                            qst = bass.AP(
                                tensor=qv.tensor, offset=qv.offset,
                                ap=[qv.ap[0], [0, 2]] + list(qv.ap[1:]))
                            nc.tensor.matmul(
                                pa[:, ic * 512:(ic + 1) * 512],
                                kst, qst,
                                start=True, stop=True,
                                perf_mode=DRow, skip_group_check=True,
                            )
                        nc.scalar.activation(
                            out=eP[u][:, J, :], in_=pa, func=Exp,
                            scale=SCALE / 2048.0)
                    units.append(go)
            return units

        def emit_interleaved(chunks, units, gates=None):
            # spread E-units evenly between chunks; unit k may only be
            # emitted once gates[k] chunks are done (WAR: the eP slot it
            # allocates must have its reader PV already emitted).
            nc_, nu = len(chunks), len(units)
            if gates is None:
                gates = [0] * nu
            ui = 0
            for ci, ch in enumerate(chunks):
                ch()
                done = ci + 1
                want = done * nu // nc_
                while ui < want and ui < nu and gates[ui] <= done:
                    units[ui]()
                    ui += 1
            while ui < nu:
                units[ui]()
                ui += 1

        # ---- fused stage 1+2 ----
        with tc.tile_pool(name="psProj", bufs=2, space="PSUM") as psProj, \
             tc.tile_pool(name="s1fix", bufs=1) as s1fix:
            pools["psProj"] = psProj
            xT = s1fix.tile([P, KBLK, S], fp16, name="xT")
            x8T = s1fix.tile([P, KBLK, S], fp8, name="x8T")
            x8Tp = s1fix.tile([P, KBLK, S], fp8, name="x8Tp")
            w8q = s1fix.tile([P, KBLK, E], fp8, name="w8q")
            w8k = s1fix.tile([P, KBLK, E], fp8, name="w8k")
            wv_sb = s1fix.tile([P, KBLK, E], fp16, name="wv_sb")
            wv8_sb = s1fix.tile([P, KBLK, E], fp8, name="wv8_sb")
            # Input DMAs serialize on the DMA-engine device, so arrival
            # order = creation order. Load per-g column blocks of Wq/Wk so
            # the g0 QK chain (and the first exp) is gated by ~7us of DMA
            # instead of ~12us.
            wqr = wq8.rearrange("(kb kp) e -> kp kb e", kp=P)
            wkr = wk8.rearrange("(kb kp) e -> kp kb e", kp=P)
            nc.sync.dma_start(
                out=x8Tp, in_=xtp8.rearrange("(kb kp) s -> kp kb s", kp=P))
            nc.sync.dma_start(out=w8q[:, :, 0:P], in_=wqr[:, :, 0:P])
            nc.sync.dma_start(
                out=x8T, in_=xt8.rearrange("(kb kp) s -> kp kb s", kp=P))
            nc.sync.dma_start(out=w8k[:, :, 0:P], in_=wkr[:, :, 0:P])
            for g in (1,):
                nc.sync.dma_start(out=w8q[:, :, g * P:(g + 1) * P],
                                  in_=wqr[:, :, g * P:(g + 1) * P])
                nc.sync.dma_start(out=w8k[:, :, g * P:(g + 1) * P],
                                  in_=wkr[:, :, g * P:(g + 1) * P])
            nc.sync.dma_start(
                out=wv8_sb, in_=wv8.rearrange("(kb kp) e -> kp kb e", kp=P))
            for g in range(2, G):
                nc.sync.dma_start(out=w8q[:, :, g * P:(g + 1) * P],
                                  in_=wqr[:, :, g * P:(g + 1) * P])
                nc.sync.dma_start(out=w8k[:, :, g * P:(g + 1) * P],
                                  in_=wkr[:, :, g * P:(g + 1) * P])
            nc.sync.dma_start(
                out=xT, in_=xt16.rearrange("(kb kp) s -> kp kb s", kp=P))
            nc.sync.dma_start(
                out=wv_sb,
                in_=wv16.rearrange("(kb kp) e -> kp kb e", kp=P),
            )
            # T2b has no region conflicts: loads right after the inputs
            nc.sync.dma_start(
                out=T2b,
                in_=bass.AP(tensor=flat16.tensor, offset=T2BO,
                            ap=[[1, P], [1, T2BW]]),
            )

            # Warmup feeds Act immediately: Q/K(0,1) projections first,
            # then E(0) units interleaved with the V chains; remaining
            # Q/K projections ride iter 1 alongside E(1). fp8 DR makes
            # projections cheap enough that the s1fix region (and the T2
            # fill) frees by ~60us into the run.
            for g in (0, 1):
                for c in proj_chunks(g, lambda: w8q, QT, lambda: x8Tp):
                    c()
                for c in proj_chunks(g, lambda: w8k, KT, lambda: x8T):
                    c()
            chunks = []
            for jb in range(4):
                chunks += v8proj_chunks(jb)
            emit_interleaved(chunks, e_units(0))
            chunks = []
            for g in range(2, G):
                chunks += proj_chunks(g, lambda: w8q, QT, lambda: x8Tp)
                chunks += proj_chunks(g, lambda: w8k, KT, lambda: x8T)
            for jb in range(4, 8):
                chunks += v8proj_chunks(jb)
            emit_interleaved(chunks, e_units(1))
            v16 = {2: [0, 1, 2, 3], 3: [4, 5, 6, 7], 4: []}
            for i in range(2, 5):
                chunks = pv_chunks(i - 2, 0) + pv_chunks(i - 2, 1)
                for jb in v16[i]:
                    chunks += vproj_chunks(jb)
                emit_interleaved(chunks, e_units(i),
                                 [2] * 8 + [4] * 8)

        # ---- tail of stage 2 + stage 3 (T2 reuses the s1fix region) ----
        with tc.tile_pool(name="t2p", bufs=1) as t2p, \
             tc.tile_pool(name="lns", bufs=8) as lns, \
             tc.tile_pool(name="lnof", bufs=8) as lnof, \
             tc.tile_pool(name="bps", bufs=2, space="PSUM") as bps, \
             tc.tile_pool(name="tmpp", bufs=3) as tmpp:
            pools["tmpp"] = tmpp
            statsA = t2p.tile([P, NBLK, H, 6], fp32, name="statsA")
            # T2a (covers groups with base <= 17408) reuses the freed
            # s1fix region; 3-slice fill starts as soon as vproj's last
            # read of xT/wv_sb retires (~iter 3 with vproj on psProj).
            T2a = t2p.tile([P, T2AW], fp16, name="T2a")
            for a, b in ((0, 6784), (6784, 13568), (13568, T2AW)):
                nc.sync.dma_start(
                    out=T2a[:, a:b],
                    in_=bass.AP(tensor=flat16.tensor, offset=a,
                                ap=[[1, P], [1, b - a]]),
                )

            def bias_units(hh):
                # bias@V for head hh over all 8 F blocks, accumulated in a
                # [P, 512] psum tile (64-col slice per F). Late heads
                # (>=6, created before their PV) COPY into natSB — the PV
                # then adds normalized attention on top; early heads (<6,
                # created last) ADD into natSB behind their natv + stats.
                def go():
                    bt = bps.tile([P, NBLK * D], fp32, tag="bias",
                                  name="bias")
                    for F in range(NBLK):
                        base = 15360 - 1024 * hh + 2048 * F
                        if base <= 17408:
                            tsr, off = T2a, base
                        else:
                            tsr, off = T2b, base - T2BO
                        for J in range(NBLK):
                            t2st = bass.AP(
                                tensor=tsr.tensor,
                                offset=tsr.offset + off + P * J,
                                ap=[tsr.ap[0], [16, P]],
                            )
                            nc.tensor.matmul(
                                bt[:, F * D:(F + 1) * D], t2st,
                                VE[:, J, hh * DE:hh * DE + D],
                                start=(J == 0), stop=(J == NBLK - 1),
                                skip_group_check=True,
                            )
                    nat = bass.AP(
                        tensor=natSB.tensor, offset=natSB.offset + hh * D,
                        ap=[natSB.ap[0], [E, NBLK], [1, D]],
                    )
                    if hh >= 6:
                        nc.vector.tensor_copy(nat, bt)
                    else:
                        nc.vector.tensor_tensor(nat, nat, bt, add)
                        for F in range(NBLK):
                            nc.vector.bn_stats(
                                statsA[:, F, hh, :],
                                natSB[:, F, hh * D:(hh + 1) * D])
                return go

            # iters 5..7 with bias units spread as chunks (PE order is
            # ~creation order, so each iter carries only what its Act
            # window affords). Copy-scheme heads (>=6) are created just
            # before the iter holding their PV; add-scheme heads (0-5)
            # ride along (their natv landed in iters 2-4).
            # The two copy-scheme units for THIS iter's PV heads must be
            # created BEFORE the pv chunks (the PV add reads natSB on top
            # of the bias copy); the eP-slot gates shift by 2 accordingly.
            biassched = {5: ([6, 7], [12, 0, 1]), 6: ([8, 9], [13, 2, 3]),
                         7: ([10, 11], [14, 15, 4, 5])}
            for i in range(5, G):
                pre, post = biassched[i]
                chunks = ([bias_units(pre[0])] + pv_chunks(i - 2, 0)
                          + [bias_units(pre[1])] + pv_chunks(i - 2, 1)
                          + [bias_units(hh) for hh in post])
                emit_interleaved(chunks, e_units(i), [3] * 8 + [6] * 8)
            for c in pv_chunks(G - 2, 0) + pv_chunks(G - 2, 1):
                c()
            for c in pv_chunks(G - 1, 0) + pv_chunks(G - 1, 1):
                c()

            # LN front: aggr/sqrt/recip/murs per F
            mvs, rstds, murss = [], [], []
            for F in range(NBLK):
                mv = lns.tile([P, 2], fp32, tag="mv", name="mv")
                nc.vector.bn_aggr(mv, statsA[:, F, :, :])
                rstd = lns.tile([P, 1], fp32, tag="rstd", name="rstd")
                nc.scalar.activation(out=rstd, in_=mv[:, 1:2],
                                     func=Sqrt, bias=epsT, scale=1.0)
                mvs.append(mv)
                rstds.append(rstd)
            for F in range(NBLK):
                murs = lns.tile([P, 1], fp32, tag="murs", name="murs")
                nc.vector.reciprocal(rstds[F], rstds[F])
                if F % 2 == 0:
                    nc.vector.tensor_tensor(murs, mvs[F][:, 0:1],
                                            rstds[F], mult)
                else:
                    nc.vector.tensor_scalar(murs, mvs[F][:, 0:1],
                                            rstds[F], -1.0,
                                            op0=mult, op1=mult)
                murss.append(murs)

            # LN pass 2: applies alternate Pool / Act with out-DMA per F.
            for F in range(NBLK):
                of = lnof.tile([P, E], fp16, tag="of", name="of")
                if F % 2 == 0:
                    nc.gpsimd.tensor_scalar(of, natSB[:, F, :], rstds[F],
                                            murss[F], op0=mult, op1=sub)
                else:
                    nc.scalar.activation(out=of, in_=natSB[:, F, :],
                                         func=Identity, bias=murss[F],
                                         scale=rstds[F])
                if not trivial_ln:
                    nc.vector.tensor_tensor(of, of, gamT, mult)
                    nc.vector.tensor_tensor(of, of, betT, add)
                nc.sync.dma_start(out[F * P:(F + 1) * P, :], of)

    nc.compile()
    return nc


def get_nc(trivial_ln: bool = True):
    if trivial_ln not in _BUILT:
        _BUILT[trivial_ln] = _build(trivial_ln)
    return _BUILT[trivial_ln]


def make_in_maps(inputs):
    x = np.asarray(inputs["x"])
    rel = np.asarray(inputs["rel_table"])
    gamma = np.asarray(inputs["gamma"])
    beta = np.asarray(inputs["beta"])
    trivial_ln = bool(np.all(gamma == 1.0) and np.all(beta == 0.0))

    import ml_dtypes
    f8 = ml_dtypes.float8_e4m3fn
    x16 = x.astype(np.float16)
    xt16 = np.ascontiguousarray(x16.transpose(0, 2, 1))          # (B, E, S)
    xt8 = np.ascontiguousarray(x.transpose(0, 2, 1).astype(f8))
    xtp8 = np.ascontiguousarray(x[:, SIGMA, :].transpose(0, 2, 1).astype(f8))
    # q/k weights pre-scaled by 32 (fp8 sweet spot); exp scale absorbs 1/1024
    wq8 = (np.asarray(inputs["Wq"]) * 32.0).astype(f8)
    wk8 = (np.asarray(inputs["Wk"]) * 32.0).astype(f8)
    wv16 = np.asarray(inputs["Wv"]).astype(np.float16)
    wv8 = (np.asarray(inputs["Wv"]) * 32.0).astype(f8)
    flat16 = np.ascontiguousarray(rel.reshape(-1).astype(np.float16))

    in_maps = []
    for b in range(x.shape[0]):
        m = {"xt16": xt16[b], "xt8": xt8[b], "xtp8": xtp8[b],
             "wq8": wq8, "wk8": wk8, "wv16": wv16, "wv8": wv8,
             "flat16": flat16}
        if not trivial_ln:
            m["gamma"] = gamma.reshape(1, E).astype(np.float32)
            m["beta"] = beta.reshape(1, E).astype(np.float32)
        in_maps.append(m)
    return in_maps, trivial_ln


def unpermute(raw):
    """raw: (..., S, E) rows in processing order -> natural order."""
    fixed = np.empty_like(raw)
    fixed[..., SIGMA, :] = raw
    return fixed


def kernel(**inputs) -> np.ndarray:
    from concourse import bass_utils

    in_maps, trivial_ln = make_in_maps(inputs)
    nc = get_nc(trivial_ln)
    res = bass_utils.run_bass_kernel_spmd(nc, in_maps,
                                          core_ids=list(range(len(in_maps))))
    outs = np.stack([r["out"] for r in res.results])
    return unpermute(outs).astype(np.float32)



# revision 54
# speedup vs baseline: 1.2193x; 1.0001x over previous
"""Trainium2 Bass kernel for nn_Attention_Rel_Scl (B=8,S=1024,E=1024,H=16).

Data-parallel over batch: one batch element per NeuronCore (8 cores).

v8 (164976ns model, from v6's 198352): Act/exp (133us busy) is the
hard floor — TimelineSim charges matmuls out_free x 0.4167ns x cpr
(fp8e4 DoubleRow cpr=0.5, contraction length free) and activations
free_size x 0.833ns; psum (16KB) caps exp tiles at [128,1024].
  (a) QK^T as fp8 DoubleRow: QT/KT fp8 [P, G, 2, S], r=1 a copy of r=0
  (DVE for g0/g1 warmup, Pool after; Pool cannot read PSUM so the dup
  chains off the SBUF r=0 slice). The DR matmul contracts the 64 head
  dims twice; exp scale absorbs the 2x. PE 139->112us.
  (b) bias@V inside the exp window: per-head [P,512] psum accumulation.
  T2 split in two sliding windows (T2b 27.7KB loads at t~16us beside
  the stage-1 inputs, T2a 39.7KB reuses s1fix right after vproj), and
  vproj runs on the psProj ring so VE completes ~iter 3. Heads 6-15:
  bias COPIES into natSB (fp16) before their PV, whose add rides on
  top (tmp = pv*srec; natSB += tmp) — drains psum without waiting
  natv; heads 0-5 add after their early natv. Scheduler note: PE static
  order ~= creation order, so bias units are spread per-iter as chunks
  sized to each iter's Act budget (a block created too early starves
  Act for its full duration).
  (c) tail: per-(F,head) bn_stats ride each bias-add; LN front fused
  into head 15's PV; applies alternate Pool / Act(Identity, bias=-mu*
  rstd); fp16 'of'/output (host casts) halves the out-DMA; wide lnof
  ring avoids WAR ping-pong. Tail ~15us after the last exp.
  First exp ~13us (w8q/w8k loaded per-g block; DMA device serializes).

v6: v5 + fp8e4 DoubleRow matmuls for Q/K projections and PV.
  - exp(QK^T/sqrt(E)) is the *stationary* operand of PV / colsum / biasV
    matmuls, so those cost only (out free size) PE cycles and the result
    lands directly in natural [row, feature] orientation (no transposes,
    no gathers). V carries an interleaved 1.0 column per head so PV and
    the softmax denominator come from one moving stream.
  - Emission interleaves 2 QK+exp J-steps between every ~2us PE chunk
    (projection half-chains, PV half-blocks): the in-order engines then
    pace each other without head-of-line stalls; Act (the 133us exp
    budget) starts ~15us in and stays ~full.
  - QT/KT/VE psum->SBUF copies run on GpSimd (Pool) so the DVE's
    reciprocal (which waits on PV groups) never blocks them.
  - T2 (bias table, 63.7KB/part) is DMA-filled into the region freed by
    the projection inputs, overlapping the back half of stage 2.
  - bias[h,i,j] = flat[(16368-1024h) + 1024*(i%16) - 16*(i//16) + j]
    (flat = rel_table.reshape(-1), clip never fires); rows processed in
    order f -> SIGMA[f] = 16*(63-f%64) + f//64 make the bias block for
    (hh, F, J) the T2 view at offset 15360-1024*hh+2048*F+128*J with
    ap [[1,128],[1024,2],[16,64]], T2[p,w] = flat[p+w].
  - LayerNorm in natural layout; combine-add + normalize-apply on Pool,
    bn_stats/aggr/recip on DVE, Sqrt on Act. Contiguous output DMA; host
    un-permutes rows (SIGMA).
"""

import sys

if "/opt/trn_rl_repo" not in sys.path:
    sys.path.insert(0, "/opt/trn_rl_repo")

import numpy as np

B, S, E, H = 8, 1024, 1024, 16
D = E // H          # 64 head dim
P = 128             # partitions
G = H // 2          # 8 head pairs
NBLK = S // P       # 8 key/query blocks
KBLK = E // P       # 8 contraction blocks
EPS = 1e-3
SCALE = float(E) ** -0.5
FLAT = (2 * S - 1) * H   # 32752
T2W = 32625              # max free offset 32624 (+p<=127 -> 32751 = FLAT-1)
DE = D + 1               # 65: V column block plus ones column

_f = np.arange(S)
SIGMA = 16 * (63 - _f % 64) + _f // 64

_BUILT = {}


def _build(trivial_ln: bool):
    import concourse.bass as bass
    import concourse.tile as tile
    from concourse import bacc, mybir
    from contextlib import ExitStack

    fp16 = mybir.dt.float16
    fp32 = mybir.dt.float32
    Exp = mybir.ActivationFunctionType.Exp
    Sqrt = mybir.ActivationFunctionType.Sqrt
    Identity = mybir.ActivationFunctionType.Identity
    mult = mybir.AluOpType.mult
    add = mybir.AluOpType.add
    sub = mybir.AluOpType.subtract

    nc = bacc.Bacc("TRN2", target_bir_lowering=False, debug=False,
                   num_devices=8)

    fp8 = mybir.dt.float8e4
    DRow = mybir.MatmulPerfMode.DoubleRow
    xt16 = nc.dram_tensor("xt16", [E, S], fp16, kind="ExternalInput").ap()
    xt8 = nc.dram_tensor("xt8", [E, S], fp8, kind="ExternalInput").ap()
    xtp8 = nc.dram_tensor("xtp8", [E, S], fp8, kind="ExternalInput").ap()
    wq8 = nc.dram_tensor("wq8", [E, E], fp8, kind="ExternalInput").ap()
    wk8 = nc.dram_tensor("wk8", [E, E], fp8, kind="ExternalInput").ap()
    wv16 = nc.dram_tensor("wv16", [E, E], fp16, kind="ExternalInput").ap()
    wv8 = nc.dram_tensor("wv8", [E, E], fp8, kind="ExternalInput").ap()
    flat16 = nc.dram_tensor("flat16", [FLAT], fp16, kind="ExternalInput").ap()
    if not trivial_ln:
        gam = nc.dram_tensor("gamma", [1, E], fp32, kind="ExternalInput").ap()
        bet = nc.dram_tensor("beta", [1, E], fp32, kind="ExternalInput").ap()
    # fp16 output (host casts to fp32): LN output is ~N(0,1), fp16
    # rounding is ~5e-4 relative — halves the out-DMA tail.
    out = nc.dram_tensor("out", [S, E], fp16, kind="ExternalOutput").ap()

    with tile.TileContext(nc) as tc, ExitStack() as ctx:
        persist = ctx.enter_context(tc.tile_pool(name="persist", bufs=1))
        QT = persist.tile([P, G, S], fp8, name="QT")
        KT = persist.tile([P, G, S], fp8, name="KT")
        VE = persist.tile([P, NBLK, H * DE], fp16, name="VE")
        VE8 = persist.tile([P, NBLK, H * DE], fp8, name="VE8")
        natSB = persist.tile([P, NBLK, E], fp16, name="natSB")
        srecSB = persist.tile([P, G, 2, NBLK], fp32, name="srecSB")
        epsT = persist.tile([P, 1], fp32, name="epsT")

        nc.vector.memset(epsT, EPS)
        nc.vector.memset(
            bass.AP(tensor=VE.tensor, offset=VE.offset + D,
                    ap=[VE.ap[0], [H * DE, NBLK], [DE, H]]),
            1.0)
        nc.vector.memset(
            bass.AP(tensor=VE8.tensor, offset=VE8.offset + D,
                    ap=[VE8.ap[0], [H * DE, NBLK], [DE, H]]),
            32.0)

        if not trivial_ln:
            gamT = persist.tile([P, E], fp32, name="gamT")
            betT = persist.tile([P, E], fp32, name="betT")
            nc.sync.dma_start(
                out=gamT,
                in_=bass.AP(tensor=gam.tensor, offset=0, ap=[[0, P], [1, E]]),
            )
            nc.sync.dma_start(
                out=betT,
                in_=bass.AP(tensor=bet.tensor, offset=0, ap=[[0, P], [1, E]]),
            )

        expp = ctx.enter_context(tc.tile_pool(name="expp", bufs=5))
        # T2 bias table, split in two sliding windows so each can load as
        # early as SBUF frees: T2b (the high-offset window, 27.7KB) fits
        # beside the stage-1 inputs and loads right after them; T2a
        # (39.7KB) reuses the Q/K-input region that dies after iter 1.
        # Group (hh, F) with base = 15360-1024*hh+2048*F reads
        # T2a[base + 128J + p + 16w] if base <= 17408 (view max 20463),
        # else T2b at offset base-18432 (flat index 18432 + ...).
        T2AW = 20352
        T2BO = 18432
        T2BW = FLAT - T2BO - 127   # 14193
        t2bp = ctx.enter_context(tc.tile_pool(name="t2bp", bufs=1))
        T2b = t2bp.tile([P, T2BW], fp16, name="T2b")
        psQK = ctx.enter_context(
            tc.tile_pool(name="psQK", bufs=2, space="PSUM"))
        pvp = ctx.enter_context(
            tc.tile_pool(name="pvp", bufs=1, space="PSUM"))

        eP = {}
        pools = {}

        # ---- emission helpers: each returns a list of closures ("chunks");
        # E-units (one QK J-step + exp) are interleaved between chunks.
        def proj_chunks(g, w8get, dst, rhs8get):
            # fp8 DoubleRow: contraction 1024 as 4 steps of 2x128.
            # Per-ic [P,512] psum tiles (bufs=2) let the DVE copy of ic0
            # overlap the matmuls of ic1 / the next chain.
            # dst is [P, G, 2, S] fp8; the r=1 slice is a DMA duplicate of
            # r=0 so QK can run as a DoubleRow matmul (contracting the 64
            # head dims twice; exp scale absorbs the factor 2).
            def go():
                w8, rhs8 = w8get(), rhs8get()
                for ic in range(2):
                    pt = pools["psProj"].tile([P, 512], fp32, tag="proj",
                                              name="pt")
                    for kp in range(4):
                        nc.tensor.matmul(
                            pt,
                            w8[:, 2 * kp:2 * kp + 2, g * P:(g + 1) * P],
                            rhs8[:, 2 * kp:2 * kp + 2,
                                 ic * 512:(ic + 1) * 512],
                            start=(kp == 0), stop=(kp == 3),
                            perf_mode=DRow, skip_group_check=True,
                        )
                    nc.vector.tensor_copy(
                        dst[:, g, ic * 512:(ic + 1) * 512], pt)
            return [go]

        def v8proj_chunks(jb):
            # fp8 DR V projection feeding VE8 (PV path) only
            def mk(ic):
                def go():
                    bt = pvp.tile([P, NBLK, P], fp32, tag="pv", name="pv")
                    pt = bass.AP(tensor=bt.tensor, offset=bt.offset,
                                 ap=[bt.ap[0], [1, 512]])
                    for kp in range(4):
                        nc.tensor.matmul(
                            pt,
                            x8T[:, 2 * kp:2 * kp + 2, jb * P:(jb + 1) * P],
                            wv8_sb[:, 2 * kp:2 * kp + 2,
                                   ic * 512:(ic + 1) * 512],
                            start=(kp == 0), stop=(kp == 3),
                            perf_mode=DRow, skip_group_check=True,
                        )
                    dstv8 = bass.AP(
                        tensor=VE8.tensor,
                        offset=VE8.offset + jb * (H * DE) + ic * 8 * DE,
                        ap=[VE8.ap[0], [DE, 8], [1, D]],
                    )
                    nc.vector.tensor_copy(dstv8, pt)
                return go
            return [mk(0), mk(1)]

        def vproj_chunks(jb):
            # V16 runs on the psProj ring (idle after iter 1), decoupled
            # from the PV ring so all 16 chunks can finish by ~iter 3 and
            # unblock the bias matmuls (which read all of VE).
            def mk(ic):
                def go():
                    pt = pools["psProj"].tile([P, 512], fp32, tag="proj",
                                              name="pt")
                    for kb in range(KBLK):
                        nc.tensor.matmul(
                            pt,
                            xT[:, kb, jb * P:(jb + 1) * P],
                            wv_sb[:, kb, ic * 512:(ic + 1) * 512],
                            start=(kb == 0), stop=(kb == KBLK - 1),
                            skip_group_check=True,
                        )
                    dstv = bass.AP(
                        tensor=VE.tensor,
                        offset=VE.offset + jb * (H * DE) + ic * 8 * DE,
                        ap=[VE.ap[0], [DE, 8], [1, D]],
                    )
                    nc.vector.tensor_copy(dstv, pt)
                return go
            return [mk(0), mk(1)]

        def pv_chunks(g, half):
            u = 2 * g + half
            hh = u
            state = {}

            def mk(fh):
                def go(st):
                    if fh == 0:
                        st["pv"] = pvp.tile([P, NBLK, P], fp32, tag="pv", name="pv")
                    pv = st["pv"]
                    for F in range(4 * fh, 4 * fh + 4):
                        for Jp in range(4):
                            nc.tensor.matmul(
                                pv[:, F, 0:DE],
                                eP[u][:, 2 * Jp:2 * Jp + 2,
                                      F * P:(F + 1) * P],
                                VE8[:, 2 * Jp:2 * Jp + 2,
                                    hh * DE:(hh + 1) * DE],
                                start=(Jp == 0), stop=(Jp == 3),
                                perf_mode=DRow, skip_group_check=True,
                            )
                    if fh == 1:
                        del eP[u]
                        srec = srecSB[:, g, half, :]
                        nc.vector.reciprocal(
                            srec,
                            bass.AP(tensor=pv.tensor, offset=pv.offset + D,
                                    ap=[pv.ap[0], [P, NBLK]]))
                        natv = bass.AP(
                            tensor=natSB.tensor,
                            offset=natSB.offset + hh * D,
                            ap=[natSB.ap[0], [E, NBLK], [1, D]],
                        )
                        pvv = bass.AP(tensor=pv.tensor, offset=pv.offset,
                                      ap=[pv.ap[0], [P, NBLK], [1, D]])
                        srecb = bass.AP(
                            tensor=srecSB.tensor,
                            offset=srecSB.offset + u * NBLK,
                            ap=[srecSB.ap[0], [1, NBLK], [0, D]],
                        )
                        if u < 6:
                            # early heads: attn@V lands first, the bias
                            # unit later ADDS into natSB.
                            nc.vector.tensor_tensor(natv, pvv, srecb, mult)
                        else:
                            # late heads: the bias COPY (created earlier,
                            # eligible early) already filled natSB; add
                            # the normalized attention on top, then stats.
                            tmp = pools["tmpp"].tile([P, NBLK * D], fp32,
                                                     tag="tmp", name="tmp")
                            nc.vector.tensor_tensor(tmp, pvv, srecb, mult)
                            nc.vector.tensor_tensor(natv, natv, tmp, add)
                            for F in range(NBLK):
                                nc.vector.bn_stats(
                                    statsA[:, F, u, :],
                                    natSB[:, F, u * D:(u + 1) * D])
                                if u == H - 1:
                                    # last head: LN front rides along,
                                    # aggr(F) right behind the stats
                                    # that complete it
                                    ln_front(F)
                            if u == H - 1:
                                for F in range(NBLK):
                                    ln_murs(F)
                return go
            return [lambda f=mk(0): f(state), lambda f=mk(1): f(state)]

        def e_units(g):
            units = []
            for half in range(2):
                for J in range(NBLK):
                    def go(half=half, J=J):
                        u = 2 * g + half
                        if J == 0:
                            eP[u] = expp.tile([P, NBLK, S], fp8, tag="ept", name="eP")
                        lo = D * half
                        pa = psQK.tile([P, E], fp32, tag="qk", name="pa")
                        # DoubleRow pair dim as a stride-0 AP dim: both
                        # k-tiles read the SAME 64 head dims (exp scale
                        # absorbs the factor 2) — no duplicate slice.
                        kv = KT[lo:lo + D, g, J * P:(J + 1) * P]
                        kst = bass.AP(tensor=kv.tensor, offset=kv.offset,
                                      ap=[kv.ap[0], [0, 2]] + list(kv.ap[1:]))
                        for ic in range(2):
                            qv = QT[lo:lo + D, g,
                                    ic * 512:(ic + 1) * 512]
                            qst = bass.AP(
                                tensor=qv.tensor, offset=qv.offset,
                                ap=[qv.ap[0], [0, 2]] + list(qv.ap[1:]))
                            nc.tensor.matmul(
                                pa[:, ic * 512:(ic + 1) * 512],
                                kst, qst,
                                start=True, stop=True,
                                perf_mode=DRow, skip_group_check=True,
                            )
                        nc.scalar.activation(
                            out=eP[u][:, J, :], in_=pa, func=Exp,
                            scale=SCALE / 2048.0)
                    units.append(go)
            return units

        def emit_interleaved(chunks, units, gates=None):
            # spread E-units evenly between chunks; unit k may only be
            # emitted once gates[k] chunks are done (WAR: the eP slot it
            # allocates must have its reader PV already emitted).
            nc_, nu = len(chunks), len(units)
            if gates is None:
                gates = [0] * nu
            ui = 0
            for ci, ch in enumerate(chunks):
                ch()
                done = ci + 1
                want = done * nu // nc_
                while ui < want and ui < nu and gates[ui] <= done:
                    units[ui]()
                    ui += 1
            while ui < nu:
                units[ui]()
                ui += 1

        # ---- fused stage 1+2 ----
        with tc.tile_pool(name="psProj", bufs=2, space="PSUM") as psProj, \
             tc.tile_pool(name="s1fix", bufs=1) as s1fix:
            pools["psProj"] = psProj
            xT = s1fix.tile([P, KBLK, S], fp16, name="xT")
            x8T = s1fix.tile([P, KBLK, S], fp8, name="x8T")
            x8Tp = s1fix.tile([P, KBLK, S], fp8, name="x8Tp")
            w8q = s1fix.tile([P, KBLK, E], fp8, name="w8q")
            w8k = s1fix.tile([P, KBLK, E], fp8, name="w8k")
            wv_sb = s1fix.tile([P, KBLK, E], fp16, name="wv_sb")
            wv8_sb = s1fix.tile([P, KBLK, E], fp8, name="wv8_sb")
            # Input DMAs serialize on the DMA-engine device, so arrival
            # order = creation order. Load per-g column blocks of Wq/Wk so
            # the g0 QK chain (and the first exp) is gated by ~7us of DMA
            # instead of ~12us.
            wqr = wq8.rearrange("(kb kp) e -> kp kb e", kp=P)
            wkr = wk8.rearrange("(kb kp) e -> kp kb e", kp=P)
            nc.sync.dma_start(
                out=x8Tp, in_=xtp8.rearrange("(kb kp) s -> kp kb s", kp=P))
            nc.sync.dma_start(out=w8q[:, :, 0:P], in_=wqr[:, :, 0:P])
            nc.sync.dma_start(
                out=x8T, in_=xt8.rearrange("(kb kp) s -> kp kb s", kp=P))
            nc.sync.dma_start(out=w8k[:, :, 0:P], in_=wkr[:, :, 0:P])
            for g in (1,):
                nc.sync.dma_start(out=w8q[:, :, g * P:(g + 1) * P],
                                  in_=wqr[:, :, g * P:(g + 1) * P])
                nc.sync.dma_start(out=w8k[:, :, g * P:(g + 1) * P],
                                  in_=wkr[:, :, g * P:(g + 1) * P])
            nc.sync.dma_start(
                out=wv8_sb, in_=wv8.rearrange("(kb kp) e -> kp kb e", kp=P))
            for g in range(2, G):
                nc.sync.dma_start(out=w8q[:, :, g * P:(g + 1) * P],
                                  in_=wqr[:, :, g * P:(g + 1) * P])
                nc.sync.dma_start(out=w8k[:, :, g * P:(g + 1) * P],
                                  in_=wkr[:, :, g * P:(g + 1) * P])
            nc.sync.dma_start(
                out=xT, in_=xt16.rearrange("(kb kp) s -> kp kb s", kp=P))
            nc.sync.dma_start(
                out=wv_sb,
                in_=wv16.rearrange("(kb kp) e -> kp kb e", kp=P),
            )
            # T2b has no region conflicts: loads right after the inputs
            nc.sync.dma_start(
                out=T2b,
                in_=bass.AP(tensor=flat16.tensor, offset=T2BO,
                            ap=[[1, P], [1, T2BW]]),
            )

            # Warmup feeds Act immediately: Q/K(0,1) projections first,
            # then E(0) units interleaved with the V chains; remaining
            # Q/K projections ride iter 1 alongside E(1). fp8 DR makes
            # projections cheap enough that the s1fix region (and the T2
            # fill) frees by ~60us into the run.
            for g in (0, 1):
                for c in proj_chunks(g, lambda: w8q, QT, lambda: x8Tp):
                    c()
                for c in proj_chunks(g, lambda: w8k, KT, lambda: x8T):
                    c()
            chunks = []
            for jb in range(4):
                chunks += v8proj_chunks(jb)
            emit_interleaved(chunks, e_units(0))
            chunks = []
            for g in range(2, G):
                chunks += proj_chunks(g, lambda: w8q, QT, lambda: x8Tp)
                chunks += proj_chunks(g, lambda: w8k, KT, lambda: x8T)
            for jb in range(4, 8):
                chunks += v8proj_chunks(jb)
            emit_interleaved(chunks, e_units(1))
            v16 = {2: [0, 1, 2, 3], 3: [4, 5, 6, 7], 4: []}
            for i in range(2, 5):
                chunks = pv_chunks(i - 2, 0) + pv_chunks(i - 2, 1)
                for jb in v16[i]:
                    chunks += vproj_chunks(jb)
                emit_interleaved(chunks, e_units(i),
                                 [2] * 8 + [4] * 8)

        # ---- tail of stage 2 + stage 3 (T2 reuses the s1fix region) ----
        with tc.tile_pool(name="t2p", bufs=1) as t2p, \
             tc.tile_pool(name="lns", bufs=8) as lns, \
             tc.tile_pool(name="lnof", bufs=8) as lnof, \
             tc.tile_pool(name="bps", bufs=2, space="PSUM") as bps, \
             tc.tile_pool(name="tmpp", bufs=3) as tmpp:
            pools["tmpp"] = tmpp
            statsA = t2p.tile([P, NBLK, H, 6], fp32, name="statsA")
            # T2a (covers groups with base <= 17408) reuses the freed
            # s1fix region; 3-slice fill starts as soon as vproj's last
            # read of xT/wv_sb retires (~iter 3 with vproj on psProj).
            T2a = t2p.tile([P, T2AW], fp16, name="T2a")
            for a, b in ((0, 6784), (6784, 13568), (13568, T2AW)):
                nc.sync.dma_start(
                    out=T2a[:, a:b],
                    in_=bass.AP(tensor=flat16.tensor, offset=a,
                                ap=[[1, P], [1, b - a]]),
                )

            def bias_units(hh):
                # bias@V for head hh over all 8 F blocks, accumulated in a
                # [P, 512] psum tile (64-col slice per F). Late heads
                # (>=6, created before their PV) COPY into natSB — the PV
                # then adds normalized attention on top; early heads (<6,
                # created last) ADD into natSB behind their natv + stats.
                def go():
                    bt = bps.tile([P, NBLK * D], fp32, tag="bias",
                                  name="bias")
                    for F in range(NBLK):
                        base = 15360 - 1024 * hh + 2048 * F
                        if base <= 17408:
                            tsr, off = T2a, base
                        else:
                            tsr, off = T2b, base - T2BO
                        for J in range(NBLK):
                            t2st = bass.AP(
                                tensor=tsr.tensor,
                                offset=tsr.offset + off + P * J,
                                ap=[tsr.ap[0], [16, P]],
                            )
                            nc.tensor.matmul(
                                bt[:, F * D:(F + 1) * D], t2st,
                                VE[:, J, hh * DE:hh * DE + D],
                                start=(J == 0), stop=(J == NBLK - 1),
                                skip_group_check=True,
                            )
                    nat = bass.AP(
                        tensor=natSB.tensor, offset=natSB.offset + hh * D,
                        ap=[natSB.ap[0], [E, NBLK], [1, D]],
                    )
                    if hh >= 6:
                        nc.vector.tensor_copy(nat, bt)
                    else:
                        nc.vector.tensor_tensor(nat, nat, bt, add)
                        for F in range(NBLK):
                            nc.vector.bn_stats(
                                statsA[:, F, hh, :],
                                natSB[:, F, hh * D:(hh + 1) * D])
                return go

            # iters 5..7 with bias units spread as chunks (PE order is
            # ~creation order, so each iter carries only what its Act
            # window affords). Copy-scheme heads (>=6) are created just
            # before the iter holding their PV; add-scheme heads (0-5)
            # ride along (their natv landed in iters 2-4).
            # LN front helpers, emitted from inside head-15's PV chunk:
            # rstd = exp(-0.5*ln(var+eps)) — Ln/Exp/Identity share one
            # act table set, so no Sqrt table switch after the last exp.
            mvs, rstds, murss = [], [], []

            def ln_front(F):
                mv = lns.tile([P, 2], fp32, tag="mv", name="mv")
                nc.vector.bn_aggr(mv, statsA[:, F, :, :])
                rstd = lns.tile([P, 1], fp32, tag="rstd", name="rstd")
                nc.scalar.activation(out=rstd, in_=mv[:, 1:2],
                                     func=Sqrt, bias=epsT, scale=1.0)
                mvs.append(mv)
                rstds.append(rstd)

            def ln_murs(F):
                murs = lns.tile([P, 1], fp32, tag="murs", name="murs")
                nc.vector.reciprocal(rstds[F], rstds[F])
                if F % 2 == 0:
                    nc.vector.tensor_tensor(murs, mvs[F][:, 0:1],
                                            rstds[F], mult)
                else:
                    nc.vector.tensor_scalar(murs, mvs[F][:, 0:1],
                                            rstds[F], -1.0,
                                            op0=mult, op1=mult)
                murss.append(murs)

            # The two copy-scheme units for THIS iter's PV heads must be
            # created BEFORE the pv chunks (the PV add reads natSB on top
            # of the bias copy); the eP-slot gates shift by 2 accordingly.
            biassched = {5: ([6, 7], [12, 0, 1]), 6: ([8, 9], [13, 2, 3]),
                         7: ([10, 11], [14, 15, 4, 5])}
            for i in range(5, G):
                pre, post = biassched[i]
                chunks = ([bias_units(pre[0])] + pv_chunks(i - 2, 0)
                          + [bias_units(pre[1])] + pv_chunks(i - 2, 1)
                          + [bias_units(hh) for hh in post])
                emit_interleaved(chunks, e_units(i), [3] * 8 + [6] * 8)
            for c in pv_chunks(G - 2, 0) + pv_chunks(G - 2, 1):
                c()
            for c in pv_chunks(G - 1, 0) + pv_chunks(G - 1, 1):
                c()

            # LN pass 2: applies alternate Pool / Act with out-DMA per F.
            for F in range(NBLK):
                of = lnof.tile([P, E], fp16, tag="of", name="of")
                if F % 2 == 0:
                    nc.gpsimd.tensor_scalar(of, natSB[:, F, :], rstds[F],
                                            murss[F], op0=mult, op1=sub)
                else:
                    nc.scalar.activation(out=of, in_=natSB[:, F, :],
                                         func=Identity, bias=murss[F],
                                         scale=rstds[F])
                if not trivial_ln:
                    nc.vector.tensor_tensor(of, of, gamT, mult)
                    nc.vector.tensor_tensor(of, of, betT, add)
                nc.sync.dma_start(out[F * P:(F + 1) * P, :], of)

    nc.compile()
    return nc


def get_nc(trivial_ln: bool = True):
    if trivial_ln not in _BUILT:
        _BUILT[trivial_ln] = _build(trivial_ln)
    return _BUILT[trivial_ln]


def make_in_maps(inputs):
    x = np.asarray(inputs["x"])
    rel = np.asarray(inputs["rel_table"])
    gamma = np.asarray(inputs["gamma"])
    beta = np.asarray(inputs["beta"])
    trivial_ln = bool(np.all(gamma == 1.0) and np.all(beta == 0.0))

    import ml_dtypes
    f8 = ml_dtypes.float8_e4m3fn
    x16 = x.astype(np.float16)
    xt16 = np.ascontiguousarray(x16.transpose(0, 2, 1))          # (B, E, S)
    xt8 = np.ascontiguousarray(x.transpose(0, 2, 1).astype(f8))
    xtp8 = np.ascontiguousarray(x[:, SIGMA, :].transpose(0, 2, 1).astype(f8))
    # q/k weights pre-scaled by 32 (fp8 sweet spot); exp scale absorbs 1/1024
    wq8 = (np.asarray(inputs["Wq"]) * 32.0).astype(f8)
    wk8 = (np.asarray(inputs["Wk"]) * 32.0).astype(f8)
    wv16 = np.asarray(inputs["Wv"]).astype(np.float16)
    wv8 = (np.asarray(inputs["Wv"]) * 32.0).astype(f8)
    flat16 = np.ascontiguousarray(rel.reshape(-1).astype(np.float16))

    in_maps = []
    for b in range(x.shape[0]):
        m = {"xt16": xt16[b], "xt8": xt8[b], "xtp8": xtp8[b],
             "wq8": wq8, "wk8": wk8, "wv16": wv16, "wv8": wv8,
             "flat16": flat16}
        if not trivial_ln:
            m["gamma"] = gamma.reshape(1, E).astype(np.float32)
            m["beta"] = beta.reshape(1, E).astype(np.float32)
        in_maps.append(m)
    return in_maps, trivial_ln


def unpermute(raw):
    """raw: (..., S, E) rows in processing order -> natural order."""
    fixed = np.empty_like(raw)
    fixed[..., SIGMA, :] = raw
    return fixed


def kernel(**inputs) -> np.ndarray:
    from concourse import bass_utils

    in_maps, trivial_ln = make_in_maps(inputs)
    nc = get_nc(trivial_ln)
    res = bass_utils.run_bass_kernel_spmd(nc, in_maps,
                                          core_ids=list(range(len(in_maps))))
    outs = np.stack([r["out"] for r in res.results])
    return unpermute(outs).astype(np.float32)



# revision 60
# speedup vs baseline: 1.2207x; 1.0011x over previous
"""Trainium2 Bass kernel for nn_Attention_Rel_Scl (B=8,S=1024,E=1024,H=16).

Data-parallel over batch: one batch element per NeuronCore (8 cores).

v9 (162673ns hw-validated, from v6's 198352): Act/exp (133us busy) is
the hard floor — TimelineSim charges matmuls out_free x 0.4167ns x cpr
(fp8e4 DoubleRow cpr=0.5, contraction length free) and activations
free_size x 0.833ns; psum (16KB) caps exp tiles at [128,1024].
  (a) QK^T as fp8 DoubleRow with a STRIDE-0 pair dim: QT/KT fp8
  [P, G, S]; both k-tiles of the DR operands read the same 64 head
  dims (walrus+hw accept [0,2] AP dims), so the contraction runs
  twice and the exp scale absorbs the 2x. No duplicate slice, no dup
  copies. PE 139->112us.
  (b) bias@V inside the exp window: per-head [P,512] psum accumulation.
  T2 split in two sliding windows (T2b 27.7KB loads at t~16us beside
  the stage-1 inputs, T2a 39.7KB reuses s1fix right after vproj), and
  vproj runs on the psProj ring so VE completes ~iter 3. Heads 6-15:
  bias COPIES into natSB (fp16) before their PV, whose add rides on
  top (tmp = pv*srec; natSB += tmp) — drains psum without waiting
  natv; heads 0-5 add after their early natv. Scheduler note: PE static
  order ~= creation order, so bias units are spread per-iter as chunks
  sized to each iter's Act budget (a block created too early starves
  Act for its full duration).
  (c) tail: per-(F,head) bn_stats ride each bias-add; LN front (aggr/
  Sqrt/recip/murs) fused into head 15's PV; applies alternate Pool /
  Act(Identity, bias=-mu*rstd); fp16 'of'/output (host casts) halves
  the out-DMA; wide lnof ring avoids WAR ping-pong. Tail ~15us after
  the last exp. (Ln+Exp rstd was tried to kill the Sqrt table load but
  Bacc inserts per-pair table switches: +15us — reverted.)
  First exp ~13us (w8q/w8k loaded per-g block; DMA device serializes).

v6: v5 + fp8e4 DoubleRow matmuls for Q/K projections and PV.
  - exp(QK^T/sqrt(E)) is the *stationary* operand of PV / colsum / biasV
    matmuls, so those cost only (out free size) PE cycles and the result
    lands directly in natural [row, feature] orientation (no transposes,
    no gathers). V carries an interleaved 1.0 column per head so PV and
    the softmax denominator come from one moving stream.
  - Emission interleaves 2 QK+exp J-steps between every ~2us PE chunk
    (projection half-chains, PV half-blocks): the in-order engines then
    pace each other without head-of-line stalls; Act (the 133us exp
    budget) starts ~15us in and stays ~full.
  - QT/KT/VE psum->SBUF copies run on GpSimd (Pool) so the DVE's
    reciprocal (which waits on PV groups) never blocks them.
  - T2 (bias table, 63.7KB/part) is DMA-filled into the region freed by
    the projection inputs, overlapping the back half of stage 2.
  - bias[h,i,j] = flat[(16368-1024h) + 1024*(i%16) - 16*(i//16) + j]
    (flat = rel_table.reshape(-1), clip never fires); rows processed in
    order f -> SIGMA[f] = 16*(63-f%64) + f//64 make the bias block for
    (hh, F, J) the T2 view at offset 15360-1024*hh+2048*F+128*J with
    ap [[1,128],[1024,2],[16,64]], T2[p,w] = flat[p+w].
  - LayerNorm in natural layout; combine-add + normalize-apply on Pool,
    bn_stats/aggr/recip on DVE, Sqrt on Act. Contiguous output DMA; host
    un-permutes rows (SIGMA).
"""

import sys

if "/opt/trn_rl_repo" not in sys.path:
    sys.path.insert(0, "/opt/trn_rl_repo")

import numpy as np

B, S, E, H = 8, 1024, 1024, 16
D = E // H          # 64 head dim
P = 128             # partitions
G = H // 2          # 8 head pairs
NBLK = S // P       # 8 key/query blocks
KBLK = E // P       # 8 contraction blocks
EPS = 1e-3
SCALE = float(E) ** -0.5
FLAT = (2 * S - 1) * H   # 32752
T2W = 32625              # max free offset 32624 (+p<=127 -> 32751 = FLAT-1)
DE = D + 1               # 65: V column block plus ones column

_f = np.arange(S)
SIGMA = 16 * (63 - _f % 64) + _f // 64

_BUILT = {}


def _build(trivial_ln: bool):
    import concourse.bass as bass
    import concourse.tile as tile
    from concourse import bacc, mybir
    from contextlib import ExitStack

    fp16 = mybir.dt.float16
    fp32 = mybir.dt.float32
    Exp = mybir.ActivationFunctionType.Exp
    Sqrt = mybir.ActivationFunctionType.Sqrt
    Identity = mybir.ActivationFunctionType.Identity
    mult = mybir.AluOpType.mult
    add = mybir.AluOpType.add
    sub = mybir.AluOpType.subtract

    nc = bacc.Bacc("TRN2", target_bir_lowering=False, debug=False,
                   num_devices=8)

    fp8 = mybir.dt.float8e4
    DRow = mybir.MatmulPerfMode.DoubleRow
    xt16 = nc.dram_tensor("xt16", [E, S], fp16, kind="ExternalInput").ap()
    xt8 = nc.dram_tensor("xt8", [E, S], fp8, kind="ExternalInput").ap()
    xtp8 = nc.dram_tensor("xtp8", [E, S], fp8, kind="ExternalInput").ap()
    wq8 = nc.dram_tensor("wq8", [E, E], fp8, kind="ExternalInput").ap()
    wk8 = nc.dram_tensor("wk8", [E, E], fp8, kind="ExternalInput").ap()
    wv16 = nc.dram_tensor("wv16", [E, E], fp16, kind="ExternalInput").ap()
    wv8 = nc.dram_tensor("wv8", [E, E], fp8, kind="ExternalInput").ap()
    flat16 = nc.dram_tensor("flat16", [FLAT], fp16, kind="ExternalInput").ap()
    if not trivial_ln:
        gam = nc.dram_tensor("gamma", [1, E], fp32, kind="ExternalInput").ap()
        bet = nc.dram_tensor("beta", [1, E], fp32, kind="ExternalInput").ap()
    # fp16 output (host casts to fp32): LN output is ~N(0,1), fp16
    # rounding is ~5e-4 relative — halves the out-DMA tail.
    out = nc.dram_tensor("out", [S, E], fp16, kind="ExternalOutput").ap()

    with tile.TileContext(nc) as tc, ExitStack() as ctx:
        persist = ctx.enter_context(tc.tile_pool(name="persist", bufs=1))
        QT = persist.tile([P, G, S], fp8, name="QT")
        KT = persist.tile([P, G, S], fp8, name="KT")
        VE = persist.tile([P, NBLK, H * DE], fp16, name="VE")
        VE8 = persist.tile([P, NBLK, H * DE], fp8, name="VE8")
        natSB = persist.tile([P, NBLK, E], fp16, name="natSB")
        srecSB = persist.tile([P, G, 2, NBLK], fp32, name="srecSB")
        epsT = persist.tile([P, 1], fp32, name="epsT")

        nc.vector.memset(epsT, EPS)
        nc.vector.memset(
            bass.AP(tensor=VE.tensor, offset=VE.offset + D,
                    ap=[VE.ap[0], [H * DE, NBLK], [DE, H]]),
            1.0)
        nc.vector.memset(
            bass.AP(tensor=VE8.tensor, offset=VE8.offset + D,
                    ap=[VE8.ap[0], [H * DE, NBLK], [DE, H]]),
            32.0)

        if not trivial_ln:
            gamT = persist.tile([P, E], fp32, name="gamT")
            betT = persist.tile([P, E], fp32, name="betT")
            nc.sync.dma_start(
                out=gamT,
                in_=bass.AP(tensor=gam.tensor, offset=0, ap=[[0, P], [1, E]]),
            )
            nc.sync.dma_start(
                out=betT,
                in_=bass.AP(tensor=bet.tensor, offset=0, ap=[[0, P], [1, E]]),
            )

        expp = ctx.enter_context(tc.tile_pool(name="expp", bufs=5))
        # T2 bias table, split in two sliding windows so each can load as
        # early as SBUF frees: T2b (the high-offset window, 27.7KB) fits
        # beside the stage-1 inputs and loads right after them; T2a
        # (39.7KB) reuses the Q/K-input region that dies after iter 1.
        # Group (hh, F) with base = 15360-1024*hh+2048*F reads
        # T2a[base + 128J + p + 16w] if base <= 17408 (view max 20463),
        # else T2b at offset base-18432 (flat index 18432 + ...).
        T2AW = 20352
        T2BO = 18432
        T2BW = FLAT - T2BO - 127   # 14193
        t2bp = ctx.enter_context(tc.tile_pool(name="t2bp", bufs=1))
        T2b = t2bp.tile([P, T2BW], fp16, name="T2b")
        psQK = ctx.enter_context(
            tc.tile_pool(name="psQK", bufs=2, space="PSUM"))
        pvp = ctx.enter_context(
            tc.tile_pool(name="pvp", bufs=1, space="PSUM"))

        eP = {}
        pools = {}

        # ---- emission helpers: each returns a list of closures ("chunks");
        # E-units (one QK J-step + exp) are interleaved between chunks.
        def proj_chunks(g, w8get, dst, rhs8get):
            # fp8 DoubleRow: contraction 1024 as 4 steps of 2x128.
            # Per-ic [P,512] psum tiles (bufs=2) let the DVE copy of ic0
            # overlap the matmuls of ic1 / the next chain.
            # dst is [P, G, 2, S] fp8; the r=1 slice is a DMA duplicate of
            # r=0 so QK can run as a DoubleRow matmul (contracting the 64
            # head dims twice; exp scale absorbs the factor 2).
            def go():
                w8, rhs8 = w8get(), rhs8get()
                for ic in range(2):
                    pt = pools["psProj"].tile([P, 512], fp32, tag="proj",
                                              name="pt")
                    for kp in range(4):
                        nc.tensor.matmul(
                            pt,
                            w8[:, 2 * kp:2 * kp + 2, g * P:(g + 1) * P],
                            rhs8[:, 2 * kp:2 * kp + 2,
                                 ic * 512:(ic + 1) * 512],
                            start=(kp == 0), stop=(kp == 3),
                            perf_mode=DRow, skip_group_check=True,
                        )
                    nc.vector.tensor_copy(
                        dst[:, g, ic * 512:(ic + 1) * 512], pt)
            return [go]

        def v8proj_chunks(jb):
            # fp8 DR V projection feeding VE8 (PV path) only
            def mk(ic):
                def go():
                    bt = pvp.tile([P, NBLK, P], fp32, tag="pv", name="pv")
                    pt = bass.AP(tensor=bt.tensor, offset=bt.offset,
                                 ap=[bt.ap[0], [1, 512]])
                    for kp in range(4):
                        nc.tensor.matmul(
                            pt,
                            x8T[:, 2 * kp:2 * kp + 2, jb * P:(jb + 1) * P],
                            wv8_sb[:, 2 * kp:2 * kp + 2,
                                   ic * 512:(ic + 1) * 512],
                            start=(kp == 0), stop=(kp == 3),
                            perf_mode=DRow, skip_group_check=True,
                        )
                    dstv8 = bass.AP(
                        tensor=VE8.tensor,
                        offset=VE8.offset + jb * (H * DE) + ic * 8 * DE,
                        ap=[VE8.ap[0], [DE, 8], [1, D]],
                    )
                    nc.vector.tensor_copy(dstv8, pt)
                return go
            return [mk(0), mk(1)]

        def vproj_chunks(jb):
            # V16 runs on the psProj ring (idle after iter 1), decoupled
            # from the PV ring so all 16 chunks can finish by ~iter 3 and
            # unblock the bias matmuls (which read all of VE).
            def mk(ic):
                def go():
                    pt = pools["psProj"].tile([P, 512], fp32, tag="proj",
                                              name="pt")
                    for kb in range(KBLK):
                        nc.tensor.matmul(
                            pt,
                            xT[:, kb, jb * P:(jb + 1) * P],
                            wv_sb[:, kb, ic * 512:(ic + 1) * 512],
                            start=(kb == 0), stop=(kb == KBLK - 1),
                            skip_group_check=True,
                        )
                    dstv = bass.AP(
                        tensor=VE.tensor,
                        offset=VE.offset + jb * (H * DE) + ic * 8 * DE,
                        ap=[VE.ap[0], [DE, 8], [1, D]],
                    )
                    nc.vector.tensor_copy(dstv, pt)
                return go
            return [mk(0), mk(1)]

        def pv_chunks(g, half):
            u = 2 * g + half
            hh = u
            state = {}

            def mk(fh):
                def go(st):
                    if fh == 0:
                        st["pv"] = pvp.tile([P, NBLK, P], fp32, tag="pv", name="pv")
                    pv = st["pv"]
                    for F in range(4 * fh, 4 * fh + 4):
                        for Jp in range(4):
                            nc.tensor.matmul(
                                pv[:, F, 0:DE],
                                eP[u][:, 2 * Jp:2 * Jp + 2,
                                      F * P:(F + 1) * P],
                                VE8[:, 2 * Jp:2 * Jp + 2,
                                    hh * DE:(hh + 1) * DE],
                                start=(Jp == 0), stop=(Jp == 3),
                                perf_mode=DRow, skip_group_check=True,
                            )
                    if fh == 1:
                        del eP[u]
                        srec = srecSB[:, g, half, :]
                        nc.vector.reciprocal(
                            srec,
                            bass.AP(tensor=pv.tensor, offset=pv.offset + D,
                                    ap=[pv.ap[0], [P, NBLK]]))
                        natv = bass.AP(
                            tensor=natSB.tensor,
                            offset=natSB.offset + hh * D,
                            ap=[natSB.ap[0], [E, NBLK], [1, D]],
                        )
                        pvv = bass.AP(tensor=pv.tensor, offset=pv.offset,
                                      ap=[pv.ap[0], [P, NBLK], [1, D]])
                        srecb = bass.AP(
                            tensor=srecSB.tensor,
                            offset=srecSB.offset + u * NBLK,
                            ap=[srecSB.ap[0], [1, NBLK], [0, D]],
                        )
                        if u < 6:
                            # early heads: attn@V lands first, the bias
                            # unit later ADDS into natSB.
                            nc.vector.tensor_tensor(natv, pvv, srecb, mult)
                        else:
                            # late heads: the bias COPY (created earlier,
                            # eligible early) already filled natSB; add
                            # the normalized attention on top, then stats.
                            tmp = pools["tmpp"].tile([P, NBLK * D], fp32,
                                                     tag="tmp", name="tmp")
                            nc.vector.tensor_tensor(tmp, pvv, srecb, mult)
                            nc.vector.tensor_tensor(natv, natv, tmp, add)
                            for F in range(NBLK):
                                nc.vector.bn_stats(
                                    statsA[:, F, u, :],
                                    natSB[:, F, u * D:(u + 1) * D])
                                if u == H - 1:
                                    # last head: LN front rides along,
                                    # aggr(F) right behind the stats
                                    # that complete it
                                    ln_front(F)
                            if u == H - 1:
                                for F in range(NBLK):
                                    ln_murs(F)
                return go
            return [lambda f=mk(0): f(state), lambda f=mk(1): f(state)]

        def e_units(g):
            units = []
            for half in range(2):
                for J in range(NBLK):
                    def go(half=half, J=J):
                        u = 2 * g + half
                        if J == 0:
                            eP[u] = expp.tile([P, NBLK, S], fp8, tag="ept", name="eP")
                        lo = D * half
                        pa = psQK.tile([P, E], fp32, tag="qk", name="pa")
                        # DoubleRow pair dim as a stride-0 AP dim: both
                        # k-tiles read the SAME 64 head dims (exp scale
                        # absorbs the factor 2) — no duplicate slice.
                        kv = KT[lo:lo + D, g, J * P:(J + 1) * P]
                        kst = bass.AP(tensor=kv.tensor, offset=kv.offset,
                                      ap=[kv.ap[0], [0, 2]] + list(kv.ap[1:]))
                        for ic in range(2):
                            qv = QT[lo:lo + D, g,
                                    ic * 512:(ic + 1) * 512]
                            qst = bass.AP(
                                tensor=qv.tensor, offset=qv.offset,
                                ap=[qv.ap[0], [0, 2]] + list(qv.ap[1:]))
                            nc.tensor.matmul(
                                pa[:, ic * 512:(ic + 1) * 512],
                                kst, qst,
                                start=True, stop=True,
                                perf_mode=DRow, skip_group_check=True,
                            )
                        nc.scalar.activation(
                            out=eP[u][:, J, :], in_=pa, func=Exp,
                            scale=SCALE / 2048.0)
                    units.append(go)
            return units

        def emit_interleaved(chunks, units, gates=None):
            # spread E-units evenly between chunks; unit k may only be
            # emitted once gates[k] chunks are done (WAR: the eP slot it
            # allocates must have its reader PV already emitted).
            nc_, nu = len(chunks), len(units)
            if gates is None:
                gates = [0] * nu
            ui = 0
            for ci, ch in enumerate(chunks):
                ch()
                done = ci + 1
                want = done * nu // nc_
                while ui < want and ui < nu and gates[ui] <= done:
                    units[ui]()
                    ui += 1
            while ui < nu:
                units[ui]()
                ui += 1

        # ---- fused stage 1+2 ----
        with tc.tile_pool(name="psProj", bufs=2, space="PSUM") as psProj, \
             tc.tile_pool(name="s1fix", bufs=1) as s1fix:
            pools["psProj"] = psProj
            xT = s1fix.tile([P, KBLK, S], fp16, name="xT")
            x8T = s1fix.tile([P, KBLK, S], fp8, name="x8T")
            x8Tp = s1fix.tile([P, KBLK, S], fp8, name="x8Tp")
            w8q = s1fix.tile([P, KBLK, E], fp8, name="w8q")
            w8k = s1fix.tile([P, KBLK, E], fp8, name="w8k")
            wv_sb = s1fix.tile([P, KBLK, E], fp16, name="wv_sb")
            wv8_sb = s1fix.tile([P, KBLK, E], fp8, name="wv8_sb")
            # Input DMAs serialize on the DMA-engine device, so arrival
            # order = creation order. Load per-g column blocks of Wq/Wk so
            # the g0 QK chain (and the first exp) is gated by ~7us of DMA
            # instead of ~12us.
            wqr = wq8.rearrange("(kb kp) e -> kp kb e", kp=P)
            wkr = wk8.rearrange("(kb kp) e -> kp kb e", kp=P)
            nc.sync.dma_start(
                out=x8Tp, in_=xtp8.rearrange("(kb kp) s -> kp kb s", kp=P))
            nc.sync.dma_start(out=w8q[:, :, 0:P], in_=wqr[:, :, 0:P])
            nc.sync.dma_start(
                out=x8T, in_=xt8.rearrange("(kb kp) s -> kp kb s", kp=P))
            nc.sync.dma_start(out=w8k[:, :, 0:P], in_=wkr[:, :, 0:P])
            for g in (1,):
                nc.sync.dma_start(out=w8q[:, :, g * P:(g + 1) * P],
                                  in_=wqr[:, :, g * P:(g + 1) * P])
                nc.sync.dma_start(out=w8k[:, :, g * P:(g + 1) * P],
                                  in_=wkr[:, :, g * P:(g + 1) * P])
            nc.sync.dma_start(
                out=wv8_sb, in_=wv8.rearrange("(kb kp) e -> kp kb e", kp=P))
            for g in range(2, G):
                nc.sync.dma_start(out=w8q[:, :, g * P:(g + 1) * P],
                                  in_=wqr[:, :, g * P:(g + 1) * P])
                nc.sync.dma_start(out=w8k[:, :, g * P:(g + 1) * P],
                                  in_=wkr[:, :, g * P:(g + 1) * P])
            nc.sync.dma_start(
                out=xT, in_=xt16.rearrange("(kb kp) s -> kp kb s", kp=P))
            nc.sync.dma_start(
                out=wv_sb,
                in_=wv16.rearrange("(kb kp) e -> kp kb e", kp=P),
            )
            # T2b has no region conflicts: loads right after the inputs
            nc.sync.dma_start(
                out=T2b,
                in_=bass.AP(tensor=flat16.tensor, offset=T2BO,
                            ap=[[1, P], [1, T2BW]]),
            )

            # Warmup feeds Act immediately: Q/K(0,1) projections first,
            # then E(0) units interleaved with the V chains; remaining
            # Q/K projections ride iter 1 alongside E(1). fp8 DR makes
            # projections cheap enough that the s1fix region (and the T2
            # fill) frees by ~60us into the run.
            for g in (0, 1):
                for c in proj_chunks(g, lambda: w8q, QT, lambda: x8Tp):
                    c()
                for c in proj_chunks(g, lambda: w8k, KT, lambda: x8T):
                    c()
            chunks = []
            for jb in range(4):
                chunks += v8proj_chunks(jb)
            emit_interleaved(chunks, e_units(0))
            chunks = []
            for g in range(2, G):
                chunks += proj_chunks(g, lambda: w8q, QT, lambda: x8Tp)
                chunks += proj_chunks(g, lambda: w8k, KT, lambda: x8T)
            for jb in range(4, 8):
                chunks += v8proj_chunks(jb)
            emit_interleaved(chunks, e_units(1))
            v16 = {2: [0, 1, 2, 3], 3: [4, 5, 6, 7], 4: []}
            for i in range(2, 5):
                chunks = pv_chunks(i - 2, 0) + pv_chunks(i - 2, 1)
                for jb in v16[i]:
                    chunks += vproj_chunks(jb)
                emit_interleaved(chunks, e_units(i),
                                 [2] * 8 + [4] * 8)

        # ---- tail of stage 2 + stage 3 (T2 reuses the s1fix region) ----
        with tc.tile_pool(name="t2p", bufs=1) as t2p, \
             tc.tile_pool(name="lns", bufs=8) as lns, \
             tc.tile_pool(name="lnof", bufs=8) as lnof, \
             tc.tile_pool(name="bps", bufs=2, space="PSUM") as bps, \
             tc.tile_pool(name="tmpp", bufs=3) as tmpp:
            pools["tmpp"] = tmpp
            statsA = t2p.tile([P, NBLK, H, 6], fp32, name="statsA")
            # T2a (covers groups with base <= 17408) reuses the freed
            # s1fix region; 3-slice fill starts as soon as vproj's last
            # read of xT/wv_sb retires (~iter 3 with vproj on psProj).
            T2a = t2p.tile([P, T2AW], fp16, name="T2a")
            for a, b in ((0, 6784), (6784, 13568), (13568, T2AW)):
                nc.sync.dma_start(
                    out=T2a[:, a:b],
                    in_=bass.AP(tensor=flat16.tensor, offset=a,
                                ap=[[1, P], [1, b - a]]),
                )

            def bias_units(hh):
                # bias@V for head hh over all 8 F blocks, accumulated in a
                # [P, 512] psum tile (64-col slice per F). Late heads
                # (>=6, created before their PV) COPY into natSB — the PV
                # then adds normalized attention on top; early heads (<6,
                # created last) ADD into natSB behind their natv + stats.
                def go():
                    bt = bps.tile([P, NBLK * D], fp32, tag="bias",
                                  name="bias")
                    for F in range(NBLK):
                        base = 15360 - 1024 * hh + 2048 * F
                        if base <= 17408:
                            tsr, off = T2a, base
                        else:
                            tsr, off = T2b, base - T2BO
                        for J in range(NBLK):
                            t2st = bass.AP(
                                tensor=tsr.tensor,
                                offset=tsr.offset + off + P * J,
                                ap=[tsr.ap[0], [16, P]],
                            )
                            nc.tensor.matmul(
                                bt[:, F * D:(F + 1) * D], t2st,
                                VE[:, J, hh * DE:hh * DE + D],
                                start=(J == 0), stop=(J == NBLK - 1),
                                skip_group_check=True,
                            )
                    nat = bass.AP(
                        tensor=natSB.tensor, offset=natSB.offset + hh * D,
                        ap=[natSB.ap[0], [E, NBLK], [1, D]],
                    )
                    if hh >= 6:
                        nc.vector.tensor_copy(nat, bt)
                    else:
                        nc.vector.tensor_tensor(nat, nat, bt, add)
                        for F in range(NBLK):
                            nc.vector.bn_stats(
                                statsA[:, F, hh, :],
                                natSB[:, F, hh * D:(hh + 1) * D])
                return go

            # iters 5..7 with bias units spread as chunks (PE order is
            # ~creation order, so each iter carries only what its Act
            # window affords). Copy-scheme heads (>=6) are created just
            # before the iter holding their PV; add-scheme heads (0-5)
            # ride along (their natv landed in iters 2-4).
            # LN front helpers, emitted from inside head-15's PV chunk:
            # rstd = exp(-0.5*ln(var+eps)) — Ln/Exp/Identity share one
            # act table set, so no Sqrt table switch after the last exp.
            mvs, rstds, murss = [], [], []

            def ln_front(F):
                mv = lns.tile([P, 2], fp32, tag="mv", name="mv")
                nc.vector.bn_aggr(mv, statsA[:, F, :, :])
                rstd = lns.tile([P, 1], fp32, tag="rstd", name="rstd")
                nc.scalar.activation(out=rstd, in_=mv[:, 1:2],
                                     func=Sqrt, bias=epsT, scale=1.0)
                mvs.append(mv)
                rstds.append(rstd)

            def ln_murs(F):
                murs = lns.tile([P, 1], fp32, tag="murs", name="murs")
                nc.vector.reciprocal(rstds[F], rstds[F])
                if F % 3 == 0:
                    nc.vector.tensor_tensor(murs, mvs[F][:, 0:1],
                                            rstds[F], mult)
                else:
                    nc.vector.tensor_scalar(murs, mvs[F][:, 0:1],
                                            rstds[F], -1.0,
                                            op0=mult, op1=mult)
                murss.append(murs)

            # The two copy-scheme units for THIS iter's PV heads must be
            # created BEFORE the pv chunks (the PV add reads natSB on top
            # of the bias copy); the eP-slot gates shift by 2 accordingly.
            biassched = {5: ([6, 7], [12, 0, 1]), 6: ([8, 9], [13, 2, 3]),
                         7: ([10, 11], [14, 15, 4, 5])}
            for i in range(5, G):
                pre, post = biassched[i]
                chunks = ([bias_units(pre[0])] + pv_chunks(i - 2, 0)
                          + [bias_units(pre[1])] + pv_chunks(i - 2, 1)
                          + [bias_units(hh) for hh in post])
                emit_interleaved(chunks, e_units(i), [3] * 8 + [6] * 8)
            for c in pv_chunks(G - 2, 0) + pv_chunks(G - 2, 1):
                c()
            for c in pv_chunks(G - 1, 0) + pv_chunks(G - 1, 1):
                c()

            # LN pass 2: applies alternate Pool / Act with out-DMA per F.
            for F in range(NBLK):
                of = lnof.tile([P, E], fp16, tag="of", name="of")
                if F % 3 == 0:
                    nc.gpsimd.tensor_scalar(of, natSB[:, F, :], rstds[F],
                                            murss[F], op0=mult, op1=sub)
                else:
                    nc.scalar.activation(out=of, in_=natSB[:, F, :],
                                         func=Identity, bias=murss[F],
                                         scale=rstds[F])
                if not trivial_ln:
                    nc.vector.tensor_tensor(of, of, gamT, mult)
                    nc.vector.tensor_tensor(of, of, betT, add)
                nc.sync.dma_start(out[F * P:(F + 1) * P, :], of)

    nc.compile()
    return nc


def get_nc(trivial_ln: bool = True):
    if trivial_ln not in _BUILT:
        _BUILT[trivial_ln] = _build(trivial_ln)
    return _BUILT[trivial_ln]


def make_in_maps(inputs):
    x = np.asarray(inputs["x"])
    rel = np.asarray(inputs["rel_table"])
    gamma = np.asarray(inputs["gamma"])
    beta = np.asarray(inputs["beta"])
    trivial_ln = bool(np.all(gamma == 1.0) and np.all(beta == 0.0))

    import ml_dtypes
    f8 = ml_dtypes.float8_e4m3fn
    x16 = x.astype(np.float16)
    xt16 = np.ascontiguousarray(x16.transpose(0, 2, 1))          # (B, E, S)
    xt8 = np.ascontiguousarray(x.transpose(0, 2, 1).astype(f8))
    xtp8 = np.ascontiguousarray(x[:, SIGMA, :].transpose(0, 2, 1).astype(f8))
    # q/k weights pre-scaled by 32 (fp8 sweet spot); exp scale absorbs 1/1024
    wq8 = (np.asarray(inputs["Wq"]) * 32.0).astype(f8)
    wk8 = (np.asarray(inputs["Wk"]) * 32.0).astype(f8)
    wv16 = np.asarray(inputs["Wv"]).astype(np.float16)
    wv8 = (np.asarray(inputs["Wv"]) * 32.0).astype(f8)
    flat16 = np.ascontiguousarray(rel.reshape(-1).astype(np.float16))

    in_maps = []
    for b in range(x.shape[0]):
        m = {"xt16": xt16[b], "xt8": xt8[b], "xtp8": xtp8[b],
             "wq8": wq8, "wk8": wk8, "wv16": wv16, "wv8": wv8,
             "flat16": flat16}
        if not trivial_ln:
            m["gamma"] = gamma.reshape(1, E).astype(np.float32)
            m["beta"] = beta.reshape(1, E).astype(np.float32)
        in_maps.append(m)
    return in_maps, trivial_ln


def unpermute(raw):
    """raw: (..., S, E) rows in processing order -> natural order."""
    fixed = np.empty_like(raw)
    fixed[..., SIGMA, :] = raw
    return fixed


def kernel(**inputs) -> np.ndarray:
    from concourse import bass_utils

    in_maps, trivial_ln = make_in_maps(inputs)
    nc = get_nc(trivial_ln)
    res = bass_utils.run_bass_kernel_spmd(nc, in_maps,
                                          core_ids=list(range(len(in_maps))))
    outs = np.stack([r["out"] for r in res.results])
    return unpermute(outs).astype(np.float32)



# revision 64
# speedup vs baseline: 1.2257x; 1.0041x over previous
"""Trainium2 Bass kernel for nn_Attention_Rel_Scl (B=8,S=1024,E=1024,H=16).

Data-parallel over batch: one batch element per NeuronCore (8 cores).

v9 (162673ns hw-validated, from v6's 198352): Act/exp (133us busy) is
the hard floor — TimelineSim charges matmuls out_free x 0.4167ns x cpr
(fp8e4 DoubleRow cpr=0.5, contraction length free) and activations
free_size x 0.833ns; psum (16KB) caps exp tiles at [128,1024].
  (a) QK^T as fp8 DoubleRow with a STRIDE-0 pair dim: QT/KT fp8
  [P, G, S]; both k-tiles of the DR operands read the same 64 head
  dims (walrus+hw accept [0,2] AP dims), so the contraction runs
  twice and the exp scale absorbs the 2x. No duplicate slice, no dup
  copies. PE 139->112us.
  (b) bias@V inside the exp window: per-head [P,512] psum accumulation.
  T2 split in two sliding windows (T2b 27.7KB loads at t~16us beside
  the stage-1 inputs, T2a 39.7KB reuses s1fix right after vproj), and
  vproj runs on the psProj ring so VE completes ~iter 3. Heads 6-15:
  bias COPIES into natSB (fp16) before their PV, whose add rides on
  top (tmp = pv*srec; natSB += tmp) — drains psum without waiting
  natv; heads 0-5 add after their early natv. Scheduler note: PE static
  order ~= creation order, so bias units are spread per-iter as chunks
  sized to each iter's Act budget (a block created too early starves
  Act for its full duration).
  (c) tail: per-(F,head) bn_stats ride each bias-add; LN front (aggr/
  Sqrt/recip/murs) fused into head 15's PV; applies alternate Pool /
  Act(Identity, bias=-mu*rstd); fp16 'of'/output (host casts) halves
  the out-DMA; wide lnof ring avoids WAR ping-pong. Tail ~15us after
  the last exp. (Ln+Exp rstd was tried to kill the Sqrt table load but
  Bacc inserts per-pair table switches: +15us — reverted.)
  First exp ~13us (w8q/w8k loaded per-g block; DMA device serializes).

v6: v5 + fp8e4 DoubleRow matmuls for Q/K projections and PV.
  - exp(QK^T/sqrt(E)) is the *stationary* operand of PV / colsum / biasV
    matmuls, so those cost only (out free size) PE cycles and the result
    lands directly in natural [row, feature] orientation (no transposes,
    no gathers). V carries an interleaved 1.0 column per head so PV and
    the softmax denominator come from one moving stream.
  - Emission interleaves 2 QK+exp J-steps between every ~2us PE chunk
    (projection half-chains, PV half-blocks): the in-order engines then
    pace each other without head-of-line stalls; Act (the 133us exp
    budget) starts ~15us in and stays ~full.
  - QT/KT/VE psum->SBUF copies run on GpSimd (Pool) so the DVE's
    reciprocal (which waits on PV groups) never blocks them.
  - T2 (bias table, 63.7KB/part) is DMA-filled into the region freed by
    the projection inputs, overlapping the back half of stage 2.
  - bias[h,i,j] = flat[(16368-1024h) + 1024*(i%16) - 16*(i//16) + j]
    (flat = rel_table.reshape(-1), clip never fires); rows processed in
    order f -> SIGMA[f] = 16*(63-f%64) + f//64 make the bias block for
    (hh, F, J) the T2 view at offset 15360-1024*hh+2048*F+128*J with
    ap [[1,128],[1024,2],[16,64]], T2[p,w] = flat[p+w].
  - LayerNorm in natural layout; combine-add + normalize-apply on Pool,
    bn_stats/aggr/recip on DVE, Sqrt on Act. Contiguous output DMA; host
    un-permutes rows (SIGMA).
"""

import sys

if "/opt/trn_rl_repo" not in sys.path:
    sys.path.insert(0, "/opt/trn_rl_repo")

import numpy as np

B, S, E, H = 8, 1024, 1024, 16
D = E // H          # 64 head dim
P = 128             # partitions
G = H // 2          # 8 head pairs
NBLK = S // P       # 8 key/query blocks
KBLK = E // P       # 8 contraction blocks
EPS = 1e-3
SCALE = float(E) ** -0.5
FLAT = (2 * S - 1) * H   # 32752
T2W = 32625              # max free offset 32624 (+p<=127 -> 32751 = FLAT-1)
DE = D + 1               # 65: V column block plus ones column

_f = np.arange(S)
SIGMA = 16 * (63 - _f % 64) + _f // 64

_BUILT = {}


def _build(trivial_ln: bool):
    import concourse.bass as bass
    import concourse.tile as tile
    from concourse import bacc, mybir
    from contextlib import ExitStack

    fp16 = mybir.dt.float16
    fp32 = mybir.dt.float32
    Exp = mybir.ActivationFunctionType.Exp
    Sqrt = mybir.ActivationFunctionType.Sqrt
    Identity = mybir.ActivationFunctionType.Identity
    mult = mybir.AluOpType.mult
    add = mybir.AluOpType.add
    sub = mybir.AluOpType.subtract

    nc = bacc.Bacc("TRN2", target_bir_lowering=False, debug=False,
                   num_devices=8)

    fp8 = mybir.dt.float8e4
    DRow = mybir.MatmulPerfMode.DoubleRow
    xt16 = nc.dram_tensor("xt16", [E, S], fp16, kind="ExternalInput").ap()
    xt8 = nc.dram_tensor("xt8", [E, S], fp8, kind="ExternalInput").ap()
    wq8 = nc.dram_tensor("wq8", [E, E], fp8, kind="ExternalInput").ap()
    wk8 = nc.dram_tensor("wk8", [E, E], fp8, kind="ExternalInput").ap()
    wv16 = nc.dram_tensor("wv16", [E, E], fp16, kind="ExternalInput").ap()
    wv8 = nc.dram_tensor("wv8", [E, E], fp8, kind="ExternalInput").ap()
    flat16 = nc.dram_tensor("flat16", [FLAT], fp16, kind="ExternalInput").ap()
    if not trivial_ln:
        gam = nc.dram_tensor("gamma", [1, E], fp32, kind="ExternalInput").ap()
        bet = nc.dram_tensor("beta", [1, E], fp32, kind="ExternalInput").ap()
    # fp16 output (host casts to fp32): LN output is ~N(0,1), fp16
    # rounding is ~5e-4 relative — halves the out-DMA tail.
    out = nc.dram_tensor("out", [S, E], fp16, kind="ExternalOutput").ap()

    with tile.TileContext(nc) as tc, ExitStack() as ctx:
        persist = ctx.enter_context(tc.tile_pool(name="persist", bufs=1))
        QT = persist.tile([P, G, S], fp8, name="QT")
        KT = persist.tile([P, G, S], fp8, name="KT")
        VE = persist.tile([P, NBLK, H * DE], fp16, name="VE")
        VE8 = persist.tile([P, NBLK, H * DE], fp8, name="VE8")
        natSB = persist.tile([P, NBLK, E], fp16, name="natSB")
        srecSB = persist.tile([P, G, 2, NBLK], fp32, name="srecSB")
        epsT = persist.tile([P, 1], fp32, name="epsT")

        nc.vector.memset(epsT, EPS)
        nc.vector.memset(
            bass.AP(tensor=VE.tensor, offset=VE.offset + D,
                    ap=[VE.ap[0], [H * DE, NBLK], [DE, H]]),
            1.0)
        nc.vector.memset(
            bass.AP(tensor=VE8.tensor, offset=VE8.offset + D,
                    ap=[VE8.ap[0], [H * DE, NBLK], [DE, H]]),
            32.0)

        if not trivial_ln:
            gamT = persist.tile([P, E], fp32, name="gamT")
            betT = persist.tile([P, E], fp32, name="betT")
            nc.sync.dma_start(
                out=gamT,
                in_=bass.AP(tensor=gam.tensor, offset=0, ap=[[0, P], [1, E]]),
            )
            nc.sync.dma_start(
                out=betT,
                in_=bass.AP(tensor=bet.tensor, offset=0, ap=[[0, P], [1, E]]),
            )

        expp = ctx.enter_context(tc.tile_pool(name="expp", bufs=5))
        # T2 bias table, split in two sliding windows so each can load as
        # early as SBUF frees: T2b (the high-offset window, 27.7KB) fits
        # beside the stage-1 inputs and loads right after them; T2a
        # (39.7KB) reuses the Q/K-input region that dies after iter 1.
        # Group (hh, F) with base = 15360-1024*hh+2048*F reads
        # T2a[base + 128J + p + 16w] if base <= 17408 (view max 20463),
        # else T2b at offset base-18432 (flat index 18432 + ...).
        T2AW = 20352
        T2BO = 18432
        T2BW = FLAT - T2BO - 127   # 14193
        t2bp = ctx.enter_context(tc.tile_pool(name="t2bp", bufs=1))
        T2b = t2bp.tile([P, T2BW], fp16, name="T2b")
        psQK = ctx.enter_context(
            tc.tile_pool(name="psQK", bufs=2, space="PSUM"))
        pvp = ctx.enter_context(
            tc.tile_pool(name="pvp", bufs=1, space="PSUM"))

        eP = {}
        pools = {}

        # ---- emission helpers: each returns a list of closures ("chunks");
        # E-units (one QK J-step + exp) are interleaved between chunks.
        def proj_chunks(g, w8get, dst, rhs8get, perm=False):
            # fp8 DoubleRow: contraction 1024 as 4 steps of 2x128.
            # Per-ic [P,512] psum tiles (bufs=2) let the DVE copy of ic0
            # overlap the matmuls of ic1 / the next chain.
            # perm=True (Q): the projection runs off natural-order x8T
            # and the psum->SBUF copy applies the SIGMA query permutation
            # through its write AP: dst col for natural query q is
            # 64*(q%16) + 63 - q//16, i.e. dims [[-1,32],[64,16]] per
            # ic-half. This removes the separate permuted-x upload from
            # the serialized startup DMA chain.
            def go():
                w8, rhs8 = w8get(), rhs8get()
                for ic in range(2):
                    pt = pools["psProj"].tile([P, 512], fp32, tag="proj",
                                              name="pt")
                    for kp in range(4):
                        nc.tensor.matmul(
                            pt,
                            w8[:, 2 * kp:2 * kp + 2, g * P:(g + 1) * P],
                            rhs8[:, 2 * kp:2 * kp + 2,
                                 ic * 512:(ic + 1) * 512],
                            start=(kp == 0), stop=(kp == 3),
                            perf_mode=DRow, skip_group_check=True,
                        )
                    if perm:
                        dv = dst[:, g, 0:S]
                        dperm = bass.AP(
                            tensor=dv.tensor,
                            offset=dv.offset + 63 - 32 * ic,
                            ap=[dv.ap[0], [-1, 32], [64, 16]],
                        )
                        psplit = bass.AP(tensor=pt.tensor, offset=pt.offset,
                                         ap=[pt.ap[0], [16, 32], [1, 16]])
                        nc.vector.tensor_copy(dperm, psplit)
                    else:
                        nc.vector.tensor_copy(
                            dst[:, g, ic * 512:(ic + 1) * 512], pt)
            return [go]

        def v8proj_chunks(jb):
            # fp8 DR V projection feeding VE8 (PV path) only
            def mk(ic):
                def go():
                    bt = pvp.tile([P, NBLK, P], fp32, tag="pv", name="pv")
                    pt = bass.AP(tensor=bt.tensor, offset=bt.offset,
                                 ap=[bt.ap[0], [1, 512]])
                    for kp in range(4):
                        nc.tensor.matmul(
                            pt,
                            x8T[:, 2 * kp:2 * kp + 2, jb * P:(jb + 1) * P],
                            wv8_sb[:, 2 * kp:2 * kp + 2,
                                   ic * 512:(ic + 1) * 512],
                            start=(kp == 0), stop=(kp == 3),
                            perf_mode=DRow, skip_group_check=True,
                        )
                    dstv8 = bass.AP(
                        tensor=VE8.tensor,
                        offset=VE8.offset + jb * (H * DE) + ic * 8 * DE,
                        ap=[VE8.ap[0], [DE, 8], [1, D]],
                    )
                    nc.vector.tensor_copy(dstv8, pt)
                return go
            return [mk(0), mk(1)]

        def vproj_chunks(jb):
            # V16 runs on the psProj ring (idle after iter 1), decoupled
            # from the PV ring so all 16 chunks can finish by ~iter 3 and
            # unblock the bias matmuls (which read all of VE).
            def mk(ic):
                def go():
                    pt = pools["psProj"].tile([P, 512], fp32, tag="proj",
                                              name="pt")
                    for kb in range(KBLK):
                        nc.tensor.matmul(
                            pt,
                            xT[:, kb, jb * P:(jb + 1) * P],
                            wv_sb[:, kb, ic * 512:(ic + 1) * 512],
                            start=(kb == 0), stop=(kb == KBLK - 1),
                            skip_group_check=True,
                        )
                    dstv = bass.AP(
                        tensor=VE.tensor,
                        offset=VE.offset + jb * (H * DE) + ic * 8 * DE,
                        ap=[VE.ap[0], [DE, 8], [1, D]],
                    )
                    nc.vector.tensor_copy(dstv, pt)
                return go
            return [mk(0), mk(1)]

        def pv_chunks(g, half):
            u = 2 * g + half
            hh = u
            state = {}

            def mk(fh):
                def go(st):
                    if fh == 0:
                        st["pv"] = pvp.tile([P, NBLK, P], fp32, tag="pv", name="pv")
                    pv = st["pv"]
                    for F in range(4 * fh, 4 * fh + 4):
                        for Jp in range(4):
                            nc.tensor.matmul(
                                pv[:, F, 0:DE],
                                eP[u][:, 2 * Jp:2 * Jp + 2,
                                      F * P:(F + 1) * P],
                                VE8[:, 2 * Jp:2 * Jp + 2,
                                    hh * DE:(hh + 1) * DE],
                                start=(Jp == 0), stop=(Jp == 3),
                                perf_mode=DRow, skip_group_check=True,
                            )
                    if fh == 1:
                        del eP[u]
                        srec = srecSB[:, g, half, :]
                        nc.vector.reciprocal(
                            srec,
                            bass.AP(tensor=pv.tensor, offset=pv.offset + D,
                                    ap=[pv.ap[0], [P, NBLK]]))
                        natv = bass.AP(
                            tensor=natSB.tensor,
                            offset=natSB.offset + hh * D,
                            ap=[natSB.ap[0], [E, NBLK], [1, D]],
                        )
                        pvv = bass.AP(tensor=pv.tensor, offset=pv.offset,
                                      ap=[pv.ap[0], [P, NBLK], [1, D]])
                        srecb = bass.AP(
                            tensor=srecSB.tensor,
                            offset=srecSB.offset + u * NBLK,
                            ap=[srecSB.ap[0], [1, NBLK], [0, D]],
                        )
                        if u < 6:
                            # early heads: attn@V lands first, the bias
                            # unit later ADDS into natSB.
                            nc.vector.tensor_tensor(natv, pvv, srecb, mult)
                        else:
                            # late heads: the bias COPY (created earlier,
                            # eligible early) already filled natSB; add
                            # the normalized attention on top, then stats.
                            tmp = pools["tmpp"].tile([P, NBLK * D], fp32,
                                                     tag="tmp", name="tmp")
                            nc.vector.tensor_tensor(tmp, pvv, srecb, mult)
                            nc.vector.tensor_tensor(natv, natv, tmp, add)
                            for F in range(NBLK):
                                nc.vector.bn_stats(
                                    statsA[:, F, u, :],
                                    natSB[:, F, u * D:(u + 1) * D])
                                if u == H - 1:
                                    # last head: LN front rides along,
                                    # aggr(F) right behind the stats
                                    # that complete it
                                    ln_front(F)
                            if u == H - 1:
                                for F in range(NBLK):
                                    ln_murs(F)
                return go
            return [lambda f=mk(0): f(state), lambda f=mk(1): f(state)]

        def e_units(g):
            units = []
            for half in range(2):
                for J in range(NBLK):
                    def go(half=half, J=J):
                        u = 2 * g + half
                        if J == 0:
                            eP[u] = expp.tile([P, NBLK, S], fp8, tag="ept", name="eP")
                        lo = D * half
                        pa = psQK.tile([P, E], fp32, tag="qk", name="pa")
                        # DoubleRow pair dim as a stride-0 AP dim: both
                        # k-tiles read the SAME 64 head dims (exp scale
                        # absorbs the factor 2) — no duplicate slice.
                        kv = KT[lo:lo + D, g, J * P:(J + 1) * P]
                        kst = bass.AP(tensor=kv.tensor, offset=kv.offset,
                                      ap=[kv.ap[0], [0, 2]] + list(kv.ap[1:]))
                        for ic in range(2):
                            qv = QT[lo:lo + D, g,
                                    ic * 512:(ic + 1) * 512]
                            qst = bass.AP(
                                tensor=qv.tensor, offset=qv.offset,
                                ap=[qv.ap[0], [0, 2]] + list(qv.ap[1:]))
                            nc.tensor.matmul(
                                pa[:, ic * 512:(ic + 1) * 512],
                                kst, qst,
                                start=True, stop=True,
                                perf_mode=DRow, skip_group_check=True,
                            )
                        nc.scalar.activation(
                            out=eP[u][:, J, :], in_=pa, func=Exp,
                            scale=SCALE / 2048.0)
                    units.append(go)
            return units

        def emit_interleaved(chunks, units, gates=None):
            # spread E-units evenly between chunks; unit k may only be
            # emitted once gates[k] chunks are done (WAR: the eP slot it
            # allocates must have its reader PV already emitted).
            nc_, nu = len(chunks), len(units)
            if gates is None:
                gates = [0] * nu
            ui = 0
            for ci, ch in enumerate(chunks):
                ch()
                done = ci + 1
                want = done * nu // nc_
                while ui < want and ui < nu and gates[ui] <= done:
                    units[ui]()
                    ui += 1
            while ui < nu:
                units[ui]()
                ui += 1

        # ---- fused stage 1+2 ----
        with tc.tile_pool(name="psProj", bufs=2, space="PSUM") as psProj, \
             tc.tile_pool(name="s1fix", bufs=1) as s1fix:
            pools["psProj"] = psProj
            xT = s1fix.tile([P, KBLK, S], fp16, name="xT")
            x8T = s1fix.tile([P, KBLK, S], fp8, name="x8T")
            w8q = s1fix.tile([P, KBLK, E], fp8, name="w8q")
            w8k = s1fix.tile([P, KBLK, E], fp8, name="w8k")
            wv_sb = s1fix.tile([P, KBLK, E], fp16, name="wv_sb")
            wv8_sb = s1fix.tile([P, KBLK, E], fp8, name="wv8_sb")
            # Input DMAs serialize on the DMA-engine device, so arrival
            # order = creation order. Load per-g column blocks of Wq/Wk so
            # the g0 QK chain (and the first exp) is gated by ~7us of DMA
            # instead of ~12us.
            wqr = wq8.rearrange("(kb kp) e -> kp kb e", kp=P)
            wkr = wk8.rearrange("(kb kp) e -> kp kb e", kp=P)
            nc.sync.dma_start(
                out=x8T, in_=xt8.rearrange("(kb kp) s -> kp kb s", kp=P))
            nc.sync.dma_start(out=w8q[:, :, 0:P], in_=wqr[:, :, 0:P])
            nc.sync.dma_start(out=w8k[:, :, 0:P], in_=wkr[:, :, 0:P])
            for g in (1,):
                nc.sync.dma_start(out=w8q[:, :, g * P:(g + 1) * P],
                                  in_=wqr[:, :, g * P:(g + 1) * P])
                nc.sync.dma_start(out=w8k[:, :, g * P:(g + 1) * P],
                                  in_=wkr[:, :, g * P:(g + 1) * P])
            nc.sync.dma_start(
                out=wv8_sb, in_=wv8.rearrange("(kb kp) e -> kp kb e", kp=P))
            for g in range(2, G):
                nc.sync.dma_start(out=w8q[:, :, g * P:(g + 1) * P],
                                  in_=wqr[:, :, g * P:(g + 1) * P])
                nc.sync.dma_start(out=w8k[:, :, g * P:(g + 1) * P],
                                  in_=wkr[:, :, g * P:(g + 1) * P])
            nc.sync.dma_start(
                out=xT, in_=xt16.rearrange("(kb kp) s -> kp kb s", kp=P))
            nc.sync.dma_start(
                out=wv_sb,
                in_=wv16.rearrange("(kb kp) e -> kp kb e", kp=P),
            )
            # T2b has no region conflicts: loads right after the inputs
            nc.sync.dma_start(
                out=T2b,
                in_=bass.AP(tensor=flat16.tensor, offset=T2BO,
                            ap=[[1, P], [1, T2BW]]),
            )

            # Warmup feeds Act immediately: Q/K(0,1) projections first,
            # then E(0) units interleaved with the V chains; remaining
            # Q/K projections ride iter 1 alongside E(1). fp8 DR makes
            # projections cheap enough that the s1fix region (and the T2
            # fill) frees by ~60us into the run.
            for g in (0, 1):
                for c in proj_chunks(g, lambda: w8q, QT, lambda: x8T,
                                     perm=True):
                    c()
                for c in proj_chunks(g, lambda: w8k, KT, lambda: x8T):
                    c()
            chunks = []
            for jb in range(4):
                chunks += v8proj_chunks(jb)
            emit_interleaved(chunks, e_units(0))
            chunks = []
            for g in range(2, G):
                chunks += proj_chunks(g, lambda: w8q, QT, lambda: x8T,
                                      perm=True)
                chunks += proj_chunks(g, lambda: w8k, KT, lambda: x8T)
            for jb in range(4, 8):
                chunks += v8proj_chunks(jb)
            emit_interleaved(chunks, e_units(1))
            v16 = {2: [0, 1, 2, 3], 3: [4, 5, 6, 7], 4: []}
            for i in range(2, 5):
                chunks = pv_chunks(i - 2, 0) + pv_chunks(i - 2, 1)
                for jb in v16[i]:
                    chunks += vproj_chunks(jb)
                emit_interleaved(chunks, e_units(i),
                                 [2] * 8 + [4] * 8)

        # ---- tail of stage 2 + stage 3 (T2 reuses the s1fix region) ----
        with tc.tile_pool(name="t2p", bufs=1) as t2p, \
             tc.tile_pool(name="lns", bufs=8) as lns, \
             tc.tile_pool(name="lnof", bufs=8) as lnof, \
             tc.tile_pool(name="bps", bufs=2, space="PSUM") as bps, \
             tc.tile_pool(name="tmpp", bufs=3) as tmpp:
            pools["tmpp"] = tmpp
            statsA = t2p.tile([P, NBLK, H, 6], fp32, name="statsA")
            # T2a (covers groups with base <= 17408) reuses the freed
            # s1fix region; 3-slice fill starts as soon as vproj's last
            # read of xT/wv_sb retires (~iter 3 with vproj on psProj).
            T2a = t2p.tile([P, T2AW], fp16, name="T2a")
            for a, b in ((0, 6784), (6784, 13568), (13568, T2AW)):
                nc.sync.dma_start(
                    out=T2a[:, a:b],
                    in_=bass.AP(tensor=flat16.tensor, offset=a,
                                ap=[[1, P], [1, b - a]]),
                )

            def bias_units(hh):
                # bias@V for head hh over all 8 F blocks, accumulated in a
                # [P, 512] psum tile (64-col slice per F). Late heads
                # (>=6, created before their PV) COPY into natSB — the PV
                # then adds normalized attention on top; early heads (<6,
                # created last) ADD into natSB behind their natv + stats.
                def go():
                    bt = bps.tile([P, NBLK * D], fp32, tag="bias",
                                  name="bias")
                    for F in range(NBLK):
                        base = 15360 - 1024 * hh + 2048 * F
                        if base <= 17408:
                            tsr, off = T2a, base
                        else:
                            tsr, off = T2b, base - T2BO
                        for J in range(NBLK):
                            t2st = bass.AP(
                                tensor=tsr.tensor,
                                offset=tsr.offset + off + P * J,
                                ap=[tsr.ap[0], [16, P]],
                            )
                            nc.tensor.matmul(
                                bt[:, F * D:(F + 1) * D], t2st,
                                VE[:, J, hh * DE:hh * DE + D],
                                start=(J == 0), stop=(J == NBLK - 1),
                                skip_group_check=True,
                            )
                    nat = bass.AP(
                        tensor=natSB.tensor, offset=natSB.offset + hh * D,
                        ap=[natSB.ap[0], [E, NBLK], [1, D]],
                    )
                    if hh >= 6:
                        nc.vector.tensor_copy(nat, bt)
                    else:
                        nc.vector.tensor_tensor(nat, nat, bt, add)
                        for F in range(NBLK):
                            nc.vector.bn_stats(
                                statsA[:, F, hh, :],
                                natSB[:, F, hh * D:(hh + 1) * D])
                return go

            # iters 5..7 with bias units spread as chunks (PE order is
            # ~creation order, so each iter carries only what its Act
            # window affords). Copy-scheme heads (>=6) are created just
            # before the iter holding their PV; add-scheme heads (0-5)
            # ride along (their natv landed in iters 2-4).
            # LN front helpers, emitted from inside head-15's PV chunk:
            # rstd = exp(-0.5*ln(var+eps)) — Ln/Exp/Identity share one
            # act table set, so no Sqrt table switch after the last exp.
            mvs, rstds, murss = [], [], []

            def ln_front(F):
                mv = lns.tile([P, 2], fp32, tag="mv", name="mv")
                nc.vector.bn_aggr(mv, statsA[:, F, :, :])
                rstd = lns.tile([P, 1], fp32, tag="rstd", name="rstd")
                nc.scalar.activation(out=rstd, in_=mv[:, 1:2],
                                     func=Sqrt, bias=epsT, scale=1.0)
                mvs.append(mv)
                rstds.append(rstd)

            def ln_murs(F):
                murs = lns.tile([P, 1], fp32, tag="murs", name="murs")
                nc.vector.reciprocal(rstds[F], rstds[F])
                if F % 3 == 0:
                    nc.vector.tensor_tensor(murs, mvs[F][:, 0:1],
                                            rstds[F], mult)
                else:
                    nc.vector.tensor_scalar(murs, mvs[F][:, 0:1],
                                            rstds[F], -1.0,
                                            op0=mult, op1=mult)
                murss.append(murs)

            # The two copy-scheme units for THIS iter's PV heads must be
            # created BEFORE the pv chunks (the PV add reads natSB on top
            # of the bias copy); the eP-slot gates shift by 2 accordingly.
            biassched = {5: ([6, 7], [12, 0, 1]), 6: ([8, 9], [13, 2, 3]),
                         7: ([10, 11], [14, 15, 4, 5])}
            for i in range(5, G):
                pre, post = biassched[i]
                chunks = ([bias_units(pre[0])] + pv_chunks(i - 2, 0)
                          + [bias_units(pre[1])] + pv_chunks(i - 2, 1)
                          + [bias_units(hh) for hh in post])
                emit_interleaved(chunks, e_units(i), [3] * 8 + [6] * 8)
            for c in pv_chunks(G - 2, 0) + pv_chunks(G - 2, 1):
                c()
            for c in pv_chunks(G - 1, 0) + pv_chunks(G - 1, 1):
                c()

            # LN pass 2: applies alternate Pool / Act with out-DMA per F.
            for F in range(NBLK):
                of = lnof.tile([P, E], fp16, tag="of", name="of")
                if F % 3 == 0:
                    nc.gpsimd.tensor_scalar(of, natSB[:, F, :], rstds[F],
                                            murss[F], op0=mult, op1=sub)
                else:
                    nc.scalar.activation(out=of, in_=natSB[:, F, :],
                                         func=Identity, bias=murss[F],
                                         scale=rstds[F])
                if not trivial_ln:
                    nc.vector.tensor_tensor(of, of, gamT, mult)
                    nc.vector.tensor_tensor(of, of, betT, add)
                nc.sync.dma_start(out[F * P:(F + 1) * P, :], of)

    nc.compile()
    return nc


def get_nc(trivial_ln: bool = True):
    if trivial_ln not in _BUILT:
        _BUILT[trivial_ln] = _build(trivial_ln)
    return _BUILT[trivial_ln]


def make_in_maps(inputs):
    x = np.asarray(inputs["x"])
    rel = np.asarray(inputs["rel_table"])
    gamma = np.asarray(inputs["gamma"])
    beta = np.asarray(inputs["beta"])
    trivial_ln = bool(np.all(gamma == 1.0) and np.all(beta == 0.0))

    import ml_dtypes
    f8 = ml_dtypes.float8_e4m3fn
    x16 = x.astype(np.float16)
    xt16 = np.ascontiguousarray(x16.transpose(0, 2, 1))          # (B, E, S)
    xt8 = np.ascontiguousarray(x.transpose(0, 2, 1).astype(f8))
    # q/k weights pre-scaled by 32 (fp8 sweet spot); exp scale absorbs 1/1024
    wq8 = (np.asarray(inputs["Wq"]) * 32.0).astype(f8)
    wk8 = (np.asarray(inputs["Wk"]) * 32.0).astype(f8)
    wv16 = np.asarray(inputs["Wv"]).astype(np.float16)
    wv8 = (np.asarray(inputs["Wv"]) * 32.0).astype(f8)
    flat16 = np.ascontiguousarray(rel.reshape(-1).astype(np.float16))

    in_maps = []
    for b in range(x.shape[0]):
        m = {"xt16": xt16[b], "xt8": xt8[b],
             "wq8": wq8, "wk8": wk8, "wv16": wv16, "wv8": wv8,
             "flat16": flat16}
        if not trivial_ln:
            m["gamma"] = gamma.reshape(1, E).astype(np.float32)
            m["beta"] = beta.reshape(1, E).astype(np.float32)
        in_maps.append(m)
    return in_maps, trivial_ln


def unpermute(raw):
    """raw: (..., S, E) rows in processing order -> natural order."""
    fixed = np.empty_like(raw)
    fixed[..., SIGMA, :] = raw
    return fixed


def kernel(**inputs) -> np.ndarray:
    from concourse import bass_utils

    in_maps, trivial_ln = make_in_maps(inputs)
    nc = get_nc(trivial_ln)
    res = bass_utils.run_bass_kernel_spmd(nc, in_maps,
                                          core_ids=list(range(len(in_maps))))
    outs = np.stack([r["out"] for r in res.results])
    return unpermute(outs).astype(np.float32)



# revision 66
# speedup vs baseline: 1.2313x; 1.0045x over previous
"""Trainium2 Bass kernel for nn_Attention_Rel_Scl (B=8,S=1024,E=1024,H=16).

Data-parallel over batch: one batch element per NeuronCore (8 cores).

v10 (161826ns hw-validated, from v6's 198352): Act/exp (133us busy) is
the hard floor — TimelineSim charges matmuls out_free x 0.4167ns x cpr
(fp8e4 DoubleRow cpr=0.5, contraction length free) and activations
free_size x 0.833ns; psum (16KB) caps exp tiles at [128,1024].
  (0) No permuted-x upload: the Q projection runs off natural-order
  x8T and its psum->SBUF copy applies the SIGMA query permutation in
  the write AP (dims [[-1,32],[64,16]], negative stride OK on DVE) —
  one less 2.9us DMA in the serialized startup chain; first exp ~11.8us.
  LN applies split 3 Pool / 5 Act (Pool is 1.6x slower per apply).
  (a) QK^T as fp8 DoubleRow with a STRIDE-0 pair dim: QT/KT fp8
  [P, G, S]; both k-tiles of the DR operands read the same 64 head
  dims (walrus+hw accept [0,2] AP dims), so the contraction runs
  twice and the exp scale absorbs the 2x. No duplicate slice, no dup
  copies. PE 139->112us.
  (b) bias@V inside the exp window: per-head [P,512] psum accumulation.
  T2 split in two sliding windows (T2b 27.7KB loads at t~16us beside
  the stage-1 inputs, T2a 39.7KB reuses s1fix right after vproj), and
  vproj runs on the psProj ring so VE completes ~iter 3. Heads 6-15:
  bias COPIES into natSB (fp16) before their PV, whose add rides on
  top (tmp = pv*srec; natSB += tmp) — drains psum without waiting
  natv; heads 0-5 add after their early natv. Scheduler note: PE static
  order ~= creation order, so bias units are spread per-iter as chunks
  sized to each iter's Act budget (a block created too early starves
  Act for its full duration).
  (c) tail: per-(F,head) bn_stats ride each bias-add; LN front (aggr/
  Sqrt/recip/murs) fused into head 15's PV; applies alternate Pool /
  Act(Identity, bias=-mu*rstd); fp16 'of'/output (host casts) halves
  the out-DMA; wide lnof ring avoids WAR ping-pong. Tail ~15us after
  the last exp. (Ln+Exp rstd was tried to kill the Sqrt table load but
  Bacc inserts per-pair table switches: +15us — reverted.)
  First exp ~13us (w8q/w8k loaded per-g block; DMA device serializes).

v6: v5 + fp8e4 DoubleRow matmuls for Q/K projections and PV.
  - exp(QK^T/sqrt(E)) is the *stationary* operand of PV / colsum / biasV
    matmuls, so those cost only (out free size) PE cycles and the result
    lands directly in natural [row, feature] orientation (no transposes,
    no gathers). V carries an interleaved 1.0 column per head so PV and
    the softmax denominator come from one moving stream.
  - Emission interleaves 2 QK+exp J-steps between every ~2us PE chunk
    (projection half-chains, PV half-blocks): the in-order engines then
    pace each other without head-of-line stalls; Act (the 133us exp
    budget) starts ~15us in and stays ~full.
  - QT/KT/VE psum->SBUF copies run on GpSimd (Pool) so the DVE's
    reciprocal (which waits on PV groups) never blocks them.
  - T2 (bias table, 63.7KB/part) is DMA-filled into the region freed by
    the projection inputs, overlapping the back half of stage 2.
  - bias[h,i,j] = flat[(16368-1024h) + 1024*(i%16) - 16*(i//16) + j]
    (flat = rel_table.reshape(-1), clip never fires); rows processed in
    order f -> SIGMA[f] = 16*(63-f%64) + f//64 make the bias block for
    (hh, F, J) the T2 view at offset 15360-1024*hh+2048*F+128*J with
    ap [[1,128],[1024,2],[16,64]], T2[p,w] = flat[p+w].
  - LayerNorm in natural layout; combine-add + normalize-apply on Pool,
    bn_stats/aggr/recip on DVE, Sqrt on Act. Contiguous output DMA; host
    un-permutes rows (SIGMA).
"""

import sys

if "/opt/trn_rl_repo" not in sys.path:
    sys.path.insert(0, "/opt/trn_rl_repo")

import numpy as np

B, S, E, H = 8, 1024, 1024, 16
D = E // H          # 64 head dim
P = 128             # partitions
G = H // 2          # 8 head pairs
NBLK = S // P       # 8 key/query blocks
KBLK = E // P       # 8 contraction blocks
EPS = 1e-3
SCALE = float(E) ** -0.5
FLAT = (2 * S - 1) * H   # 32752
T2W = 32625              # max free offset 32624 (+p<=127 -> 32751 = FLAT-1)
DE = D + 1               # 65: V column block plus ones column

_f = np.arange(S)
SIGMA = 16 * (63 - _f % 64) + _f // 64

_BUILT = {}


def _build(trivial_ln: bool):
    import concourse.bass as bass
    import concourse.tile as tile
    from concourse import bacc, mybir
    from contextlib import ExitStack

    fp16 = mybir.dt.float16
    fp32 = mybir.dt.float32
    Exp = mybir.ActivationFunctionType.Exp
    Sqrt = mybir.ActivationFunctionType.Sqrt
    Identity = mybir.ActivationFunctionType.Identity
    mult = mybir.AluOpType.mult
    add = mybir.AluOpType.add
    sub = mybir.AluOpType.subtract

    nc = bacc.Bacc("TRN2", target_bir_lowering=False, debug=False,
                   num_devices=8)

    fp8 = mybir.dt.float8e4
    DRow = mybir.MatmulPerfMode.DoubleRow
    xt16 = nc.dram_tensor("xt16", [E, S], fp16, kind="ExternalInput").ap()
    xt8 = nc.dram_tensor("xt8", [E, S], fp8, kind="ExternalInput").ap()
    wq8 = nc.dram_tensor("wq8", [E, E], fp8, kind="ExternalInput").ap()
    wk8 = nc.dram_tensor("wk8", [E, E], fp8, kind="ExternalInput").ap()
    wv16 = nc.dram_tensor("wv16", [E, E], fp16, kind="ExternalInput").ap()
    wv8 = nc.dram_tensor("wv8", [E, E], fp8, kind="ExternalInput").ap()
    flat16 = nc.dram_tensor("flat16", [FLAT], fp16, kind="ExternalInput").ap()
    if not trivial_ln:
        gam = nc.dram_tensor("gamma", [1, E], fp32, kind="ExternalInput").ap()
        bet = nc.dram_tensor("beta", [1, E], fp32, kind="ExternalInput").ap()
    # fp16 output (host casts to fp32): LN output is ~N(0,1), fp16
    # rounding is ~5e-4 relative — halves the out-DMA tail.
    out = nc.dram_tensor("out", [S, E], fp16, kind="ExternalOutput").ap()

    with tile.TileContext(nc) as tc, ExitStack() as ctx:
        persist = ctx.enter_context(tc.tile_pool(name="persist", bufs=1))
        QT = persist.tile([P, G, S], fp8, name="QT")
        KT = persist.tile([P, G, S], fp8, name="KT")
        VE = persist.tile([P, NBLK, H * DE], fp16, name="VE")
        VE8 = persist.tile([P, NBLK, H * DE], fp8, name="VE8")
        natSB = persist.tile([P, NBLK, E], fp16, name="natSB")
        srecSB = persist.tile([P, G, 2, NBLK], fp32, name="srecSB")
        epsT = persist.tile([P, 1], fp32, name="epsT")

        nc.vector.memset(epsT, EPS)
        nc.vector.memset(
            bass.AP(tensor=VE.tensor, offset=VE.offset + D,
                    ap=[VE.ap[0], [H * DE, NBLK], [DE, H]]),
            1.0)
        nc.vector.memset(
            bass.AP(tensor=VE8.tensor, offset=VE8.offset + D,
                    ap=[VE8.ap[0], [H * DE, NBLK], [DE, H]]),
            32.0)

        if not trivial_ln:
            gamT = persist.tile([P, E], fp32, name="gamT")
            betT = persist.tile([P, E], fp32, name="betT")
            nc.sync.dma_start(
                out=gamT,
                in_=bass.AP(tensor=gam.tensor, offset=0, ap=[[0, P], [1, E]]),
            )
            nc.sync.dma_start(
                out=betT,
                in_=bass.AP(tensor=bet.tensor, offset=0, ap=[[0, P], [1, E]]),
            )

        expp = ctx.enter_context(tc.tile_pool(name="expp", bufs=5))
        # T2 bias table, split in two sliding windows so each can load as
        # early as SBUF frees: T2b (the high-offset window, 27.7KB) fits
        # beside the stage-1 inputs and loads right after them; T2a
        # (39.7KB) reuses the Q/K-input region that dies after iter 1.
        # Group (hh, F) with base = 15360-1024*hh+2048*F reads
        # T2a[base + 128J + p + 16w] if base <= 17408 (view max 20463),
        # else T2b at offset base-18432 (flat index 18432 + ...).
        T2AW = 20352
        T2BO = 18432
        T2BW = FLAT - T2BO - 127   # 14193
        t2bp = ctx.enter_context(tc.tile_pool(name="t2bp", bufs=1))
        T2b = t2bp.tile([P, T2BW], fp16, name="T2b")
        psQK = ctx.enter_context(
            tc.tile_pool(name="psQK", bufs=2, space="PSUM"))
        pvp = ctx.enter_context(
            tc.tile_pool(name="pvp", bufs=1, space="PSUM"))

        eP = {}
        pools = {}

        # ---- emission helpers: each returns a list of closures ("chunks");
        # E-units (one QK J-step + exp) are interleaved between chunks.
        def proj_chunks(g, w8get, dst, rhs8get, perm=False):
            # fp8 DoubleRow: contraction 1024 as 4 steps of 2x128.
            # Per-ic [P,512] psum tiles (bufs=2) let the DVE copy of ic0
            # overlap the matmuls of ic1 / the next chain.
            # perm=True (Q): the projection runs off natural-order x8T
            # and the psum->SBUF copy applies the SIGMA query permutation
            # through its write AP: dst col for natural query q is
            # 64*(q%16) + 63 - q//16, i.e. dims [[-1,32],[64,16]] per
            # ic-half. This removes the separate permuted-x upload from
            # the serialized startup DMA chain.
            def go():
                w8, rhs8 = w8get(), rhs8get()
                for ic in range(2):
                    pt = pools["psProj"].tile([P, 512], fp32, tag="proj",
                                              name="pt")
                    for kp in range(4):
                        nc.tensor.matmul(
                            pt,
                            w8[:, 2 * kp:2 * kp + 2, g * P:(g + 1) * P],
                            rhs8[:, 2 * kp:2 * kp + 2,
                                 ic * 512:(ic + 1) * 512],
                            start=(kp == 0), stop=(kp == 3),
                            perf_mode=DRow, skip_group_check=True,
                        )
                    if perm:
                        dv = dst[:, g, 0:S]
                        dperm = bass.AP(
                            tensor=dv.tensor,
                            offset=dv.offset + 63 - 32 * ic,
                            ap=[dv.ap[0], [-1, 32], [64, 16]],
                        )
                        psplit = bass.AP(tensor=pt.tensor, offset=pt.offset,
                                         ap=[pt.ap[0], [16, 32], [1, 16]])
                        nc.vector.tensor_copy(dperm, psplit)
                    else:
                        nc.vector.tensor_copy(
                            dst[:, g, ic * 512:(ic + 1) * 512], pt)
            return [go]

        def v8proj_chunks(jb):
            # fp8 DR V projection feeding VE8 (PV path) only
            def mk(ic):
                def go():
                    bt = pvp.tile([P, NBLK, P], fp32, tag="pv", name="pv")
                    pt = bass.AP(tensor=bt.tensor, offset=bt.offset,
                                 ap=[bt.ap[0], [1, 512]])
                    for kp in range(4):
                        nc.tensor.matmul(
                            pt,
                            x8T[:, 2 * kp:2 * kp + 2, jb * P:(jb + 1) * P],
                            wv8_sb[:, 2 * kp:2 * kp + 2,
                                   ic * 512:(ic + 1) * 512],
                            start=(kp == 0), stop=(kp == 3),
                            perf_mode=DRow, skip_group_check=True,
                        )
                    dstv8 = bass.AP(
                        tensor=VE8.tensor,
                        offset=VE8.offset + jb * (H * DE) + ic * 8 * DE,
                        ap=[VE8.ap[0], [DE, 8], [1, D]],
                    )
                    nc.vector.tensor_copy(dstv8, pt)
                return go
            return [mk(0), mk(1)]

        def vproj_chunks(jb):
            # V16 runs on the psProj ring (idle after iter 1), decoupled
            # from the PV ring so all 16 chunks can finish by ~iter 3 and
            # unblock the bias matmuls (which read all of VE).
            def mk(ic):
                def go():
                    pt = pools["psProj"].tile([P, 512], fp32, tag="proj",
                                              name="pt")
                    for kb in range(KBLK):
                        nc.tensor.matmul(
                            pt,
                            xT[:, kb, jb * P:(jb + 1) * P],
                            wv_sb[:, kb, ic * 512:(ic + 1) * 512],
                            start=(kb == 0), stop=(kb == KBLK - 1),
                            skip_group_check=True,
                        )
                    dstv = bass.AP(
                        tensor=VE.tensor,
                        offset=VE.offset + jb * (H * DE) + ic * 8 * DE,
                        ap=[VE.ap[0], [DE, 8], [1, D]],
                    )
                    nc.vector.tensor_copy(dstv, pt)
                return go
            return [mk(0), mk(1)]

        def pv_chunks(g, half):
            u = 2 * g + half
            hh = u
            state = {}

            def mk(fh):
                def go(st):
                    if fh == 0:
                        st["pv"] = pvp.tile([P, NBLK, P], fp32, tag="pv", name="pv")
                    pv = st["pv"]
                    for F in range(4 * fh, 4 * fh + 4):
                        for Jp in range(4):
                            nc.tensor.matmul(
                                pv[:, F, 0:DE],
                                eP[u][:, 2 * Jp:2 * Jp + 2,
                                      F * P:(F + 1) * P],
                                VE8[:, 2 * Jp:2 * Jp + 2,
                                    hh * DE:(hh + 1) * DE],
                                start=(Jp == 0), stop=(Jp == 3),
                                perf_mode=DRow, skip_group_check=True,
                            )
                    if fh == 1:
                        del eP[u]
                        srec = srecSB[:, g, half, :]
                        nc.vector.reciprocal(
                            srec,
                            bass.AP(tensor=pv.tensor, offset=pv.offset + D,
                                    ap=[pv.ap[0], [P, NBLK]]))
                        natv = bass.AP(
                            tensor=natSB.tensor,
                            offset=natSB.offset + hh * D,
                            ap=[natSB.ap[0], [E, NBLK], [1, D]],
                        )
                        pvv = bass.AP(tensor=pv.tensor, offset=pv.offset,
                                      ap=[pv.ap[0], [P, NBLK], [1, D]])
                        srecb = bass.AP(
                            tensor=srecSB.tensor,
                            offset=srecSB.offset + u * NBLK,
                            ap=[srecSB.ap[0], [1, NBLK], [0, D]],
                        )
                        if u < 6:
                            # early heads: attn@V lands first, the bias
                            # unit later ADDS into natSB.
                            nc.vector.tensor_tensor(natv, pvv, srecb, mult)
                        else:
                            # late heads: the bias COPY (created earlier,
                            # eligible early) already filled natSB; add
                            # the normalized attention on top, then stats.
                            tmp = pools["tmpp"].tile([P, NBLK * D], fp32,
                                                     tag="tmp", name="tmp")
                            nc.vector.tensor_tensor(tmp, pvv, srecb, mult)
                            nc.vector.tensor_tensor(natv, natv, tmp, add)
                            for F in range(NBLK):
                                nc.vector.bn_stats(
                                    statsA[:, F, u, :],
                                    natSB[:, F, u * D:(u + 1) * D])
                                if u == H - 1:
                                    # last head: LN front rides along,
                                    # aggr(F) right behind the stats
                                    # that complete it
                                    ln_front(F)
                            if u == H - 1:
                                for F in range(NBLK):
                                    ln_murs(F)
                return go
            return [lambda f=mk(0): f(state), lambda f=mk(1): f(state)]

        def e_units(g):
            units = []
            for half in range(2):
                for J in range(NBLK):
                    def go(half=half, J=J):
                        u = 2 * g + half
                        if J == 0:
                            eP[u] = expp.tile([P, NBLK, S], fp8, tag="ept", name="eP")
                        lo = D * half
                        pa = psQK.tile([P, E], fp32, tag="qk", name="pa")
                        # DoubleRow pair dim as a stride-0 AP dim: both
                        # k-tiles read the SAME 64 head dims (exp scale
                        # absorbs the factor 2) — no duplicate slice.
                        kv = KT[lo:lo + D, g, J * P:(J + 1) * P]
                        kst = bass.AP(tensor=kv.tensor, offset=kv.offset,
                                      ap=[kv.ap[0], [0, 2]] + list(kv.ap[1:]))
                        for ic in range(2):
                            qv = QT[lo:lo + D, g,
                                    ic * 512:(ic + 1) * 512]
                            qst = bass.AP(
                                tensor=qv.tensor, offset=qv.offset,
                                ap=[qv.ap[0], [0, 2]] + list(qv.ap[1:]))
                            nc.tensor.matmul(
                                pa[:, ic * 512:(ic + 1) * 512],
                                kst, qst,
                                start=True, stop=True,
                                perf_mode=DRow, skip_group_check=True,
                            )
                        nc.scalar.activation(
                            out=eP[u][:, J, :], in_=pa, func=Exp,
                            scale=SCALE / 2048.0)
                    units.append(go)
            return units

        def emit_interleaved(chunks, units, gates=None):
            # spread E-units evenly between chunks; unit k may only be
            # emitted once gates[k] chunks are done (WAR: the eP slot it
            # allocates must have its reader PV already emitted).
            nc_, nu = len(chunks), len(units)
            if gates is None:
                gates = [0] * nu
            ui = 0
            for ci, ch in enumerate(chunks):
                ch()
                done = ci + 1
                want = done * nu // nc_
                while ui < want and ui < nu and gates[ui] <= done:
                    units[ui]()
                    ui += 1
            while ui < nu:
                units[ui]()
                ui += 1

        # ---- fused stage 1+2 ----
        with tc.tile_pool(name="psProj", bufs=2, space="PSUM") as psProj, \
             tc.tile_pool(name="s1fix", bufs=1) as s1fix:
            pools["psProj"] = psProj
            xT = s1fix.tile([P, KBLK, S], fp16, name="xT")
            x8T = s1fix.tile([P, KBLK, S], fp8, name="x8T")
            w8q = s1fix.tile([P, KBLK, E], fp8, name="w8q")
            w8k = s1fix.tile([P, KBLK, E], fp8, name="w8k")
            wv_sb = s1fix.tile([P, KBLK, E], fp16, name="wv_sb")
            wv8_sb = s1fix.tile([P, KBLK, E], fp8, name="wv8_sb")
            # Input DMAs serialize on the DMA-engine device, so arrival
            # order = creation order. Load per-g column blocks of Wq/Wk so
            # the g0 QK chain (and the first exp) is gated by ~7us of DMA
            # instead of ~12us.
            wqr = wq8.rearrange("(kb kp) e -> kp kb e", kp=P)
            wkr = wk8.rearrange("(kb kp) e -> kp kb e", kp=P)
            nc.sync.dma_start(
                out=x8T, in_=xt8.rearrange("(kb kp) s -> kp kb s", kp=P))
            nc.sync.dma_start(out=w8q[:, :, 0:P], in_=wqr[:, :, 0:P])
            nc.sync.dma_start(out=w8k[:, :, 0:P], in_=wkr[:, :, 0:P])
            for g in (1,):
                nc.sync.dma_start(out=w8q[:, :, g * P:(g + 1) * P],
                                  in_=wqr[:, :, g * P:(g + 1) * P])
                nc.sync.dma_start(out=w8k[:, :, g * P:(g + 1) * P],
                                  in_=wkr[:, :, g * P:(g + 1) * P])
            nc.sync.dma_start(
                out=wv8_sb, in_=wv8.rearrange("(kb kp) e -> kp kb e", kp=P))
            for g in range(2, G):
                nc.sync.dma_start(out=w8q[:, :, g * P:(g + 1) * P],
                                  in_=wqr[:, :, g * P:(g + 1) * P])
                nc.sync.dma_start(out=w8k[:, :, g * P:(g + 1) * P],
                                  in_=wkr[:, :, g * P:(g + 1) * P])
            nc.sync.dma_start(
                out=xT, in_=xt16.rearrange("(kb kp) s -> kp kb s", kp=P))
            nc.sync.dma_start(
                out=wv_sb,
                in_=wv16.rearrange("(kb kp) e -> kp kb e", kp=P),
            )
            # T2b has no region conflicts: loads right after the inputs
            nc.sync.dma_start(
                out=T2b,
                in_=bass.AP(tensor=flat16.tensor, offset=T2BO,
                            ap=[[1, P], [1, T2BW]]),
            )

            # Warmup feeds Act immediately: Q/K(0,1) projections first,
            # then E(0) units interleaved with the V chains; remaining
            # Q/K projections ride iter 1 alongside E(1). fp8 DR makes
            # projections cheap enough that the s1fix region (and the T2
            # fill) frees by ~60us into the run.
            for g in (0, 1):
                for c in proj_chunks(g, lambda: w8q, QT, lambda: x8T,
                                     perm=True):
                    c()
                for c in proj_chunks(g, lambda: w8k, KT, lambda: x8T):
                    c()
            chunks = []
            for jb in range(4):
                chunks += v8proj_chunks(jb)
            emit_interleaved(chunks, e_units(0))
            chunks = []
            for g in range(2, G):
                chunks += proj_chunks(g, lambda: w8q, QT, lambda: x8T,
                                      perm=True)
                chunks += proj_chunks(g, lambda: w8k, KT, lambda: x8T)
            for jb in range(4, 8):
                chunks += v8proj_chunks(jb)
            emit_interleaved(chunks, e_units(1))
            v16 = {2: [0, 1, 2, 3], 3: [4, 5, 6, 7], 4: []}
            for i in range(2, 5):
                chunks = pv_chunks(i - 2, 0) + pv_chunks(i - 2, 1)
                for jb in v16[i]:
                    chunks += vproj_chunks(jb)
                emit_interleaved(chunks, e_units(i),
                                 [2] * 8 + [4] * 8)

        # ---- tail of stage 2 + stage 3 (T2 reuses the s1fix region) ----
        with tc.tile_pool(name="t2p", bufs=1) as t2p, \
             tc.tile_pool(name="lns", bufs=8) as lns, \
             tc.tile_pool(name="lnof", bufs=8) as lnof, \
             tc.tile_pool(name="bps", bufs=2, space="PSUM") as bps, \
             tc.tile_pool(name="tmpp", bufs=3) as tmpp:
            pools["tmpp"] = tmpp
            statsA = t2p.tile([P, NBLK, H, 6], fp32, name="statsA")
            # T2a (covers groups with base <= 17408) reuses the freed
            # s1fix region; 3-slice fill starts as soon as vproj's last
            # read of xT/wv_sb retires (~iter 3 with vproj on psProj).
            T2a = t2p.tile([P, T2AW], fp16, name="T2a")
            for a, b in ((0, 6784), (6784, 13568), (13568, T2AW)):
                nc.sync.dma_start(
                    out=T2a[:, a:b],
                    in_=bass.AP(tensor=flat16.tensor, offset=a,
                                ap=[[1, P], [1, b - a]]),
                )

            def bias_units(hh):
                # bias@V for head hh over all 8 F blocks, accumulated in a
                # [P, 512] psum tile (64-col slice per F). Late heads
                # (>=6, created before their PV) COPY into natSB — the PV
                # then adds normalized attention on top; early heads (<6,
                # created last) ADD into natSB behind their natv + stats.
                def go():
                    bt = bps.tile([P, NBLK * D], fp32, tag="bias",
                                  name="bias")
                    for F in range(NBLK):
                        base = 15360 - 1024 * hh + 2048 * F
                        if base <= 17408:
                            tsr, off = T2a, base
                        else:
                            tsr, off = T2b, base - T2BO
                        for J in range(NBLK):
                            t2st = bass.AP(
                                tensor=tsr.tensor,
                                offset=tsr.offset + off + P * J,
                                ap=[tsr.ap[0], [16, P]],
                            )
                            nc.tensor.matmul(
                                bt[:, F * D:(F + 1) * D], t2st,
                                VE[:, J, hh * DE:hh * DE + D],
                                start=(J == 0), stop=(J == NBLK - 1),
                                skip_group_check=True,
                            )
                    nat = bass.AP(
                        tensor=natSB.tensor, offset=natSB.offset + hh * D,
                        ap=[natSB.ap[0], [E, NBLK], [1, D]],
                    )
                    if hh >= 6:
                        nc.vector.tensor_copy(nat, bt)
                    else:
                        nc.vector.tensor_tensor(nat, nat, bt, add)
                        for F in range(NBLK):
                            nc.vector.bn_stats(
                                statsA[:, F, hh, :],
                                natSB[:, F, hh * D:(hh + 1) * D])
                return go

            # iters 5..7 with bias units spread as chunks (PE order is
            # ~creation order, so each iter carries only what its Act
            # window affords). Copy-scheme heads (>=6) are created just
            # before the iter holding their PV; add-scheme heads (0-5)
            # ride along (their natv landed in iters 2-4).
            # LN front helpers, emitted from inside head-15's PV chunk:
            # rstd = exp(-0.5*ln(var+eps)) — Ln/Exp/Identity share one
            # act table set, so no Sqrt table switch after the last exp.
            mvs, rstds, murss = [], [], []

            def ln_front(F):
                mv = lns.tile([P, 2], fp32, tag="mv", name="mv")
                nc.vector.bn_aggr(mv, statsA[:, F, :, :])
                rstd = lns.tile([P, 1], fp32, tag="rstd", name="rstd")
                nc.scalar.activation(out=rstd, in_=mv[:, 1:2],
                                     func=Sqrt, bias=epsT, scale=1.0)
                mvs.append(mv)
                rstds.append(rstd)

            def ln_murs(F):
                murs = lns.tile([P, 1], fp32, tag="murs", name="murs")
                nc.vector.reciprocal(rstds[F], rstds[F])
                if F in (1, 3, 6):
                    # Act applies want bias = -mu*rstd
                    nc.vector.tensor_scalar(murs, mvs[F][:, 0:1],
                                            rstds[F], -1.0,
                                            op0=mult, op1=mult)
                else:
                    nc.vector.tensor_tensor(murs, mvs[F][:, 0:1],
                                            rstds[F], mult)
                murss.append(murs)

            # The two copy-scheme units for THIS iter's PV heads must be
            # created BEFORE the pv chunks (the PV add reads natSB on top
            # of the bias copy); the eP-slot gates shift by 2 accordingly.
            biassched = {5: ([6, 7], [12, 0, 1]), 6: ([8, 9], [13, 2, 3]),
                         7: ([10, 11], [14, 15, 4, 5])}
            for i in range(5, G):
                pre, post = biassched[i]
                chunks = ([bias_units(pre[0])] + pv_chunks(i - 2, 0)
                          + [bias_units(pre[1])] + pv_chunks(i - 2, 1)
                          + [bias_units(hh) for hh in post])
                emit_interleaved(chunks, e_units(i), [3] * 8 + [6] * 8)
            for c in pv_chunks(G - 2, 0) + pv_chunks(G - 2, 1):
                c()
            for c in pv_chunks(G - 1, 0) + pv_chunks(G - 1, 1):
                c()

            # LN pass 2: applies split 3-way (Pool / DVE / Act are all
            # otherwise idle here) so the serialized out-DMAs start and
            # drain as early as possible.
            for F in range(NBLK):
                of = lnof.tile([P, E], fp16, tag="of", name="of")
                if F in (0, 4):
                    nc.gpsimd.tensor_scalar(of, natSB[:, F, :], rstds[F],
                                            murss[F], op0=mult, op1=sub)
                elif F in (2, 5, 7):
                    nc.vector.tensor_scalar(of, natSB[:, F, :], rstds[F],
                                            murss[F], op0=mult, op1=sub)
                else:
                    nc.scalar.activation(out=of, in_=natSB[:, F, :],
                                         func=Identity, bias=murss[F],
                                         scale=rstds[F])
                if not trivial_ln:
                    nc.vector.tensor_tensor(of, of, gamT, mult)
                    nc.vector.tensor_tensor(of, of, betT, add)
                nc.sync.dma_start(out[F * P:(F + 1) * P, :], of)

    nc.compile()
    return nc


def get_nc(trivial_ln: bool = True):
    if trivial_ln not in _BUILT:
        _BUILT[trivial_ln] = _build(trivial_ln)
    return _BUILT[trivial_ln]


def make_in_maps(inputs):
    x = np.asarray(inputs["x"])
    rel = np.asarray(inputs["rel_table"])
    gamma = np.asarray(inputs["gamma"])
    beta = np.asarray(inputs["beta"])
    trivial_ln = bool(np.all(gamma == 1.0) and np.all(beta == 0.0))

    import ml_dtypes
    f8 = ml_dtypes.float8_e4m3fn
    x16 = x.astype(np.float16)
    xt16 = np.ascontiguousarray(x16.transpose(0, 2, 1))          # (B, E, S)
    xt8 = np.ascontiguousarray(x.transpose(0, 2, 1).astype(f8))
    # q/k weights pre-scaled by 32 (fp8 sweet spot); exp scale absorbs 1/1024
    wq8 = (np.asarray(inputs["Wq"]) * 32.0).astype(f8)
    wk8 = (np.asarray(inputs["Wk"]) * 32.0).astype(f8)
    wv16 = np.asarray(inputs["Wv"]).astype(np.float16)
    wv8 = (np.asarray(inputs["Wv"]) * 32.0).astype(f8)
    flat16 = np.ascontiguousarray(rel.reshape(-1).astype(np.float16))

    in_maps = []
    for b in range(x.shape[0]):
        m = {"xt16": xt16[b], "xt8": xt8[b],
             "wq8": wq8, "wk8": wk8, "wv16": wv16, "wv8": wv8,
             "flat16": flat16}
        if not trivial_ln:
            m["gamma"] = gamma.reshape(1, E).astype(np.float32)
            m["beta"] = beta.reshape(1, E).astype(np.float32)
        in_maps.append(m)
    return in_maps, trivial_ln


def unpermute(raw):
    """raw: (..., S, E) rows in processing order -> natural order."""
    fixed = np.empty_like(raw)
    fixed[..., SIGMA, :] = raw
    return fixed


def kernel(**inputs) -> np.ndarray:
    from concourse import bass_utils

    in_maps, trivial_ln = make_in_maps(inputs)
    nc = get_nc(trivial_ln)
    res = bass_utils.run_bass_kernel_spmd(nc, in_maps,
                                          core_ids=list(range(len(in_maps))))
    outs = np.stack([r["out"] for r in res.results])
    return unpermute(outs).astype(np.float32)

